# revision 20
# baseline (speedup 1.0000x reference)
"""Trainium2 Bass kernel for nn_Block (dense transformer block).

  out = x + FFN(LN2(x + Attn(LN1(x))))   with causal single-head attention,
  B=4, T=2048, C=H=1024, FF=4096, fp32 reference.

Distribution: 8 NeuronCores = (batch b in 0..3) x (query-half in 0..1).
Each core handles one batch element's keys/values and HALF its query rows
(causally balanced interleaved block split), plus LN2+FFN+residual for those
rows.  No collectives; the per-core programs are IDENTICAL (SPMD) - all
per-core variation is input data.

All matmul OPERANDS are fp8e4m3 driven in DoubleRow perf mode (2 fp8
weights per PE cell, K=256 contraction per matmul); every accumulation is
f32 in PSUM, and LN stats / softmax denominators / residual adds are f32.
LN gains/biases are folded into the weight matrices host-side.
"""

import sys
import types

import numpy as np

# ---------------------------------------------------------------------------
# antenv.axon_hooks shim: the image's antenv lacks this module and
# run_bass_kernel_spmd imports it under axon when trace=True.
import antenv

if "antenv.axon_hooks" not in sys.modules:
    _mod = types.ModuleType("antenv.axon_hooks")
    _mod._hook = None
    _mod.set_axon_ntff_profile_hook = lambda h: setattr(_mod, "_hook", h)
    _mod.get_axon_ntff_profile_hook = lambda: _mod._hook
    sys.modules["antenv.axon_hooks"] = _mod
    antenv.axon_hooks = _mod

import ml_dtypes

import concourse.bass as bass
import concourse.mybir as mybir
import concourse.tile as tile
from concourse.bass_utils import run_bass_kernel_spmd

F32 = mybir.dt.float32
BF16 = mybir.dt.bfloat16
E4 = mybir.dt.float8e4
DR = mybir.MatmulPerfMode.DoubleRow

B, T, C = 4, 2048, 1024
H, FF = 1024, 4096
P = 128
NT = T // P  # 16 token blocks per batch element
NCT = C // P  # 8 contraction tiles
NH = H // P  # 8 head-dim tiles
NF = FF // P  # 32 ff tiles
TOWN = T // 2  # own tokens per core (1024)
NLOC = TOWN // P  # 8 own blocks
EPS = 1e-5
SCALE = 1.0 / np.sqrt(np.float32(C))  # 1/32
NEG = -1.0e30

# Causally balanced query-block assignment (sum of chunk counts = 20 each).
L_HALF = [
    [0, 2, 4, 6, 9, 11, 13, 15],
    [1, 3, 5, 7, 8, 10, 12, 14],
]
# ceil((i+1)/4) for i in L_HALF[h] - same sequence for both halves.
NCHUNKS = [1, 1, 2, 2, 3, 3, 4, 4]


def _split_multi_waits(nc):
    """walrus here accepts at most ONE sync-wait per instruction; hoist
    extras onto injected same-engine NoOps."""
    for fn in nc.m.functions:
        for blk in fn.blocks:
            new_insts = []
            changed = False
            for inst in blk.instructions:
                si = getattr(inst, "sync_info", None)
                ow = list(si.on_wait) if si is not None and si.on_wait else []
                if len(ow) > 1:
                    for i, cond in enumerate(ow[:-1]):
                        new_insts.append(
                            mybir.InstNoOp(
                                name=f"{inst.name}-wn{i}",
                                engine=inst.engine,
                                ins=[],
                                outs=[],
                                sync_info=mybir.SyncInfo(
                                    on_wait=[cond], on_update=[]
                                ),
                            )
                        )
                    inst.sync_info = mybir.SyncInfo(
                        on_wait=[ow[-1]], on_update=list(si.on_update or [])
                    )
                    changed = True
                new_insts.append(inst)
            if changed:
                blk.instructions = new_insts


def _layernorm(nc, pool, x_t, h_t, eps_t):
    """h_t = (x_t - mean) * rsqrt(var + eps), stats along the free dim."""
    xg = x_t[:].rearrange("p (s f) -> p s f", f=512)
    stats = pool.tile([P, 2, nc.vector.BN_STATS_DIM], F32, tag="ln_stats")
    for sg in range(2):
        nc.vector.bn_stats(out=stats[:, sg], in_=xg[:, sg])
    mv = pool.tile([P, nc.vector.BN_AGGR_DIM], F32, tag="ln_mv")
    nc.vector.bn_aggr(out=mv[:], in_=stats[:])
    rstd = pool.tile([P, 1], F32, tag="ln_rstd")
    nc.scalar.activation(
        out=rstd[:],
        in_=mv[:, 1:2],
        func=mybir.ActivationFunctionType.Sqrt,
        bias=eps_t[:],
        scale=1.0,
    )
    nc.vector.reciprocal(out=rstd[:], in_=rstd[:])
    nc.vector.tensor_scalar(
        out=h_t[:],
        in0=x_t[:],
        scalar1=mv[:, 0:1],
        scalar2=rstd[:],
        op0=mybir.AluOpType.subtract,
        op1=mybir.AluOpType.mult,
    )


def build_nc():
    from contextlib import ExitStack

    nc = bass.Bass()

    x = nc.declare_dram_parameter("x", [T, C], F32, isOutput=False)
    x_own = nc.declare_dram_parameter("x_own", [TOWN, C], F32, isOutput=False)
    wq = nc.declare_dram_parameter("wq", [C, H], E4, isOutput=False)
    wk = nc.declare_dram_parameter("wk", [C, H], E4, isOutput=False)
    wv = nc.declare_dram_parameter("wv", [C, H], E4, isOutput=False)
    wv16 = nc.declare_dram_parameter("wv16", [C, H], BF16, isOutput=False)
    w1 = nc.declare_dram_parameter("w1", [NF, P, NCT, P], E4, isOutput=False)
    # w2[0] = e4m3(W2), w2[1] = e4m3(256*(W2 - w2[0])) - two-digit fp8
    w2 = nc.declare_dram_parameter("w2", [2, FF, C], E4, isOutput=False)
    qb = nc.declare_dram_parameter("qb", [H], F32, isOutput=False)
    kb = nc.declare_dram_parameter("kb", [H], F32, isOutput=False)
    vb = nc.declare_dram_parameter("vb", [H], F32, isOutput=False)
    b1 = nc.declare_dram_parameter("b1", [FF], F32, isOutput=False)
    b2 = nc.declare_dram_parameter("b2", [C], F32, isOutput=False)
    ident = nc.declare_dram_parameter("ident", [P, P], BF16, isOutput=False)
    masks = nc.declare_dram_parameter("masks", [NLOC, P, 512], F32, isOutput=False)
    out = nc.declare_dram_parameter("out", [TOWN, C], F32, isOutput=True)

    x2_d = nc.dram_tensor("x2_d", [TOWN, C], F32)

    wq_r = wq.rearrange("(ko p) h -> p ko h", p=P)
    wk_r = wk.rearrange("(ko p) h -> p ko h", p=P)
    wv_r = wv.rearrange("(ko p) h -> p ko h", p=P)
    wv16_r = wv16.rearrange("(ko p) h -> p ko h", p=P)
    w2_r = w2.rearrange("t (ko p) c -> p t ko c", p=P)

    with tile.TileContext(nc) as tc, ExitStack() as top:
        cn = top.enter_context(tc.tile_pool(name="cn", bufs=1))
        ps = top.enter_context(tc.tile_pool(name="ps", bufs=1, space="PSUM"))
        ln = top.enter_context(tc.tile_pool(name="ln", bufs=4))
        # big resident tensors spanning several phases
        res = top.enter_context(tc.tile_pool(name="res", bufs=1))

        # critical-path-first: the very first LN tile and the transpose
        # identity go ahead of the bulk constant loads.
        x0_t = ln.tile([P, C], F32, tag="xt", name="x0t")
        nc.sync.dma_start(x0_t[:], x_own[0:P, :])
        id_t = cn.tile([P, P], BF16)
        nc.sync.dma_start(id_t[:], ident[:])
        # ---- constants
        qb_t = cn.tile([P, NH], F32)
        nc.sync.dma_start(qb_t[:], qb.rearrange("(m p) -> p m", p=P))
        kb_t = cn.tile([P, NH], F32)
        nc.sync.dma_start(kb_t[:], kb.rearrange("(m p) -> p m", p=P))
        b1_t = cn.tile([P, NF], F32)
        nc.sync.dma_start(b1_t[:], b1.rearrange("(m p) -> p m", p=P))
        vb_b = cn.tile([P, H], F32)
        nc.sync.dma_start(vb_b[:], vb[None, :].partition_broadcast(P))
        b2_b = cn.tile([P, C], F32)
        nc.sync.dma_start(b2_b[:], b2[None, :].partition_broadcast(P))
        eps_t = cn.tile([P, 1], F32)
        nc.vector.memset(eps_t, EPS)

        _ctr = [0]

        def psum(tag, shape=(P, 512), dt=F32, bufs=2):
            _ctr[0] += 1
            return ps.tile(list(shape), dt, tag=tag, bufs=bufs, name=f"ps{_ctr[0]}")

        qTo = res.tile([P, NH, TOWN], E4)   # q^T own tokens (local order)
        kT = res.tile([P, NH, T], E4)       # k^T all keys
        v_sb = res.tile([P, NT, H], E4)     # v token-major, all keys
        h2T = res.tile([P, NCT, TOWN], E4)  # LN2 output transposed
        # precise (bf16) head-block path: first own block attends few keys, so
        # fp8 noise passes straight through - keep that slice in bf16.
        qTb = res.tile([P, NH, P], BF16)    # q^T for local block 0
        kTb = res.tile([P, NH, 512], BF16)  # k^T for key chunk 0
        v_bf = res.tile([P, 2, H], BF16)    # v for key blocks 0-1 (bf16 matmul)

        # ================= Phase A: own-token LN1 -> hTo -> q^T ============
        with ExitStack() as sA:
            big_a = sA.enter_context(tc.tile_pool(name="biga", bufs=1))
            wka = sA.enter_context(tc.tile_pool(name="wka", bufs=2))
            hto = big_a.tile([P, NCT, TOWN], E4)
            wq_pre = []
            for qg in range(2):
                w_t = wka.tile([P, NCT, 256], E4, tag="w", name=f"wqp{qg}")
                nc.sync.dma_start(w_t[:], wq_r[:, :, qg * 256 : (qg + 1) * 256])
                wq_pre.append(w_t)
            for lt in range(NLOC):
                if lt == 0:
                    x_t = x0_t
                else:
                    x_t = ln.tile([P, C], F32, tag="xt")
                    nc.sync.dma_start(x_t[:], x_own[lt * P : (lt + 1) * P, :])
                h_t = ln.tile([P, C], BF16, tag="ht")
                _layernorm(nc, ln, x_t, h_t, eps_t)
                for c in range(NCT):
                    tp = psum("b", (P, P), BF16)
                    nc.tensor.transpose(tp[:], h_t[:, c * P : (c + 1) * P], id_t[:])
                    nc.vector.tensor_copy(hto[:, c, lt * P : (lt + 1) * P], tp[:])
            # q^T: quarter weight tiles, stationary reused over 2 token groups
            for qg in range(4):
                w_t = wq_pre[qg] if qg < 2 else wka.tile([P, NCT, 256], E4, tag="w")
                if qg >= 2:
                    nc.sync.dma_start(w_t[:], wq_r[:, :, qg * 256 : (qg + 1) * 256])
                for mi in range(2):
                    m = qg * 2 + mi
                    accs = [psum("a") for _ in range(2)]
                    for k in range(NCT // 2):
                        for g in range(2):
                            nc.tensor.matmul(
                                accs[g][:],
                                w_t[:, 2 * k : 2 * k + 2, mi * P : (mi + 1) * P],
                                hto[:, 2 * k : 2 * k + 2, g * 512 : (g + 1) * 512],
                                start=(k == 0),
                                stop=(k == NCT // 2 - 1),
                                perf_mode=DR,
                            )
                    for g in range(2):
                        nc.vector.tensor_scalar_add(
                            out=qTo[:, m, g * 512 : (g + 1) * 512],
                            in0=accs[g][:],
                            scalar1=qb_t[:, m : m + 1],
                        )
                    nc.vector.tensor_scalar_add(
                        out=qTb[:, m, :],
                        in0=accs[0][:, 0:P],
                        scalar1=qb_t[:, m : m + 1],
                    )

        # ================= Phase B: full LN1 -> hT; v interleaved; k^T =====
        with ExitStack() as sB:
            big_b = sB.enter_context(tc.tile_pool(name="bigb", bufs=1))
            wkb = sB.enter_context(tc.tile_pool(name="wkb", bufs=2))
            wvp = sB.enter_context(tc.tile_pool(name="wvp", bufs=1))
            hT = big_b.tile([P, NCT, T], E4)
            hT_bf = big_b.tile([P, NCT, 256], BF16)
            wv_t = wvp.tile([P, NCT, H], E4)
            nc.sync.dma_start(wv_t[:], wv_r[:])
            wv16_t = wvp.tile([P, NCT, H], BF16)
            nc.sync.dma_start(wv16_t[:], wv16_r[:])
            wk_pre = []
            for qg in range(2):
                w_t = wkb.tile([P, NCT, 256], E4, tag="w", name=f"wkp{qg}")
                nc.sync.dma_start(w_t[:], wk_r[:, :, qg * 256 : (qg + 1) * 256])
                wk_pre.append(w_t)
            # LN per global token tile; v row-block immediately after its tile
            for st in range(NT):
                x_t = ln.tile([P, C], F32, tag="xt")
                nc.sync.dma_start(x_t[:], x[st * P : (st + 1) * P, :])
                h_t = ln.tile([P, C], BF16, tag="ht")
                _layernorm(nc, ln, x_t, h_t, eps_t)
                for c in range(NCT):
                    tp = psum("b", (P, P), BF16)
                    nc.tensor.transpose(tp[:], h_t[:, c * P : (c + 1) * P], id_t[:])
                    nc.vector.tensor_copy(hT[:, c, st * P : (st + 1) * P], tp[:])
                    if st < 2:
                        nc.vector.tensor_copy(
                            hT_bf[:, c, st * P : (st + 1) * P], tp[:]
                        )
                accs = [psum("a") for _ in range(2)]
                for k in range(NCT // 2):
                    for hh in range(2):
                        nc.tensor.matmul(
                            accs[hh][:],
                            hT[:, 2 * k : 2 * k + 2, st * P : (st + 1) * P],
                            wv_t[:, 2 * k : 2 * k + 2, hh * 512 : (hh + 1) * 512],
                            start=(k == 0),
                            stop=(k == NCT // 2 - 1),
                            perf_mode=DR,
                        )
                for hh in range(2):
                    nc.vector.tensor_add(
                        out=v_sb[:, st, hh * 512 : (hh + 1) * 512],
                        in0=accs[hh][:],
                        in1=vb_b[:, hh * 512 : (hh + 1) * 512],
                    )
            # precise bf16 v for key blocks 0-1
            for blk in range(2):
                accs = [psum("a") for _ in range(2)]
                for k in range(NCT):
                    for hh in range(2):
                        nc.tensor.matmul(
                            accs[hh][:],
                            hT_bf[:, k, blk * P : (blk + 1) * P],
                            wv16_t[:, k, hh * 512 : (hh + 1) * 512],
                            start=(k == 0),
                            stop=(k == NCT - 1),
                        )
                for hh in range(2):
                    nc.vector.tensor_add(
                        out=v_bf[:, blk, hh * 512 : (hh + 1) * 512],
                        in0=accs[hh][:],
                        in1=vb_b[:, hh * 512 : (hh + 1) * 512],
                    )
            # k^T: quarter weight tiles, stationary reused over 4 key chunks
            for qg in range(4):
                w_t = wk_pre[qg] if qg < 2 else wkb.tile([P, NCT, 256], E4, tag="w")
                if qg >= 2:
                    nc.sync.dma_start(w_t[:], wk_r[:, :, qg * 256 : (qg + 1) * 256])
                for mi in range(2):
                    m = qg * 2 + mi
                    accs = [psum("c", bufs=4) for _ in range(4)]
                    for k in range(NCT // 2):
                        for ch in range(4):
                            nc.tensor.matmul(
                                accs[ch][:],
                                w_t[:, 2 * k : 2 * k + 2, mi * P : (mi + 1) * P],
                                hT[:, 2 * k : 2 * k + 2, ch * 512 : (ch + 1) * 512],
                                start=(k == 0),
                                stop=(k == NCT // 2 - 1),
                                perf_mode=DR,
                            )
                    for ch in range(4):
                        nc.vector.tensor_scalar_add(
                            out=kT[:, m, ch * 512 : (ch + 1) * 512],
                            in0=accs[ch][:],
                            scalar1=kb_t[:, m : m + 1],
                        )
                    nc.vector.tensor_scalar_add(
                        out=kTb[:, m, :],
                        in0=accs[0][:],
                        scalar1=kb_t[:, m : m + 1],
                    )

        # ============== Phase C: attention (software-pipelined) ============
        with ExitStack() as sC:
            att = sC.enter_context(tc.tile_pool(name="att", bufs=2))
            wtl = sC.enter_context(tc.tile_pool(name="wtl", bufs=16))
            state = {}

            def emit_scores(lp):
                nch = NCHUNKS[lp]
                mask_t = att.tile([P, 512], F32, tag="mask")
                nc.sync.dma_start(mask_t[:], masks[lp])
                # prefetch the residual rows for emit_tail(lp) two steps early
                x_t = att.tile([P, C], F32, tag="xo", bufs=3)
                nc.sync.dma_start(x_t[:], x_own[lp * P : (lp + 1) * P, :])
                p_t = att.tile([P, T], BF16, tag="pt", bufs=3)
                den = att.tile([P, 4], F32, tag="den")
                scs = [psum("c", bufs=4) for _ in range(nch)]
                if lp == 0:
                    # precise bf16 scores for the head block
                    for m in range(NH):
                        nc.tensor.matmul(
                            scs[0][:],
                            qTb[:, m, :],
                            kTb[:, m, :],
                            start=(m == 0),
                            stop=(m == NH - 1),
                        )
                else:
                    for m in range(NH // 2):
                        for j in range(nch):
                            nc.tensor.matmul(
                                scs[j][:],
                                qTo[:, 2 * m : 2 * m + 2, lp * P : (lp + 1) * P],
                                kT[:, 2 * m : 2 * m + 2, j * 512 : (j + 1) * 512],
                                start=(m == 0),
                                stop=(m == NH // 2 - 1),
                                perf_mode=DR,
                            )
                for j in range(nch):
                    if j == nch - 1:
                        nc.vector.tensor_add(
                            out=scs[j][:], in0=scs[j][:], in1=mask_t[:]
                        )
                    nc.scalar.activation(
                        out=p_t[:, j * 512 : (j + 1) * 512],
                        in_=scs[j][:],
                        func=mybir.ActivationFunctionType.Exp,
                        scale=float(SCALE),
                        accum_out=den[:, j : j + 1],
                    )
                state[lp] = (p_t, den, x_t)

            def emit_tail(lp):
                nch = NCHUNKS[lp]
                nst = 4 * nch
                p_t, den, x_t = state.pop(lp)
                dsum = att.tile([P, 1], F32, tag="dsum")
                nc.vector.reduce_sum(
                    out=dsum[:], in_=den[:, :nch], axis=mybir.AxisListType.X
                )
                nc.vector.reciprocal(out=dsum[:], in_=dsum[:])
                sa0 = psum("a")
                sa1 = psum("a")
                if lp == 0:
                    # precise bf16 p@v over key blocks 0-1 (rest masked to zero)
                    wtb = []
                    for st in range(2):
                        tp = psum("b", (P, P), BF16)
                        nc.tensor.transpose(
                            tp[:], p_t[:, st * P : (st + 1) * P], id_t[:]
                        )
                        wt = wtl.tile([P, P], BF16, tag="wtb")
                        nc.vector.tensor_copy(wt[:], tp[:])
                        wtb.append(wt)
                    for st in range(2):
                        nc.tensor.matmul(
                            sa0[:], wtb[st][:], v_bf[:, st, 0:512],
                            start=(st == 0), stop=(st == 1),
                        )
                        nc.tensor.matmul(
                            sa1[:], wtb[st][:], v_bf[:, st, 512:1024],
                            start=(st == 0), stop=(st == 1),
                        )
                else:
                    wtp = []
                    for sp in range(nst // 2):
                        wt = wtl.tile([P, 2, P], E4, tag="wt")
                        for u in range(2):
                            st = 2 * sp + u
                            tp = psum("b", (P, P), BF16)
                            nc.tensor.transpose(
                                tp[:], p_t[:, st * P : (st + 1) * P], id_t[:]
                            )
                            nc.vector.tensor_copy(wt[:, u], tp[:])
                        wtp.append(wt)
                    for sp in range(nst // 2):
                        nc.tensor.matmul(
                            sa0[:], wtp[sp][:], v_sb[:, 2 * sp : 2 * sp + 2, 0:512],
                            start=(sp == 0), stop=(sp == nst // 2 - 1),
                            perf_mode=DR,
                        )
                        nc.tensor.matmul(
                            sa1[:], wtp[sp][:],
                            v_sb[:, 2 * sp : 2 * sp + 2, 512:1024],
                            start=(sp == 0), stop=(sp == nst // 2 - 1),
                            perf_mode=DR,
                        )
                x2_t = att.tile([P, C], F32, tag="x2")
                nc.vector.tensor_scalar_mul(
                    out=x2_t[:, 0:512], in0=sa0[:], scalar1=dsum[:]
                )
                nc.vector.tensor_scalar_mul(
                    out=x2_t[:, 512:1024], in0=sa1[:], scalar1=dsum[:]
                )
                nc.vector.tensor_add(out=x2_t[:], in0=x2_t[:], in1=x_t[:])
                nc.sync.dma_start(x2_d[lp * P : (lp + 1) * P, :], x2_t[:])
                return x2_t

            def emit_ln2(lt, x2_t):
                # LN2 + h2^T straight from the in-SBUF x2 tile - no DRAM trip
                h2_t = ln.tile([P, C], BF16, tag="ht")
                _layernorm(nc, ln, x2_t, h2_t, eps_t)
                for c in range(NCT):
                    tp = psum("b", (P, P), BF16)
                    nc.tensor.transpose(
                        tp[:], h2_t[:, c * P : (c + 1) * P], id_t[:]
                    )
                    nc.vector.tensor_copy(
                        h2T[:, c, lt * P : (lt + 1) * P], tp[:]
                    )

            emit_scores(0)
            emit_scores(1)
            for lp in range(2, NLOC):
                emit_scores(lp)
                emit_ln2(lp - 2, emit_tail(lp - 2))
            emit_ln2(NLOC - 2, emit_tail(NLOC - 2))
            emit_ln2(NLOC - 1, emit_tail(NLOC - 1))

        # ================= Phase D: FFN (fp8 DoubleRow) ====================
        with ExitStack() as sD:
            big_d = sD.enter_context(tc.tile_pool(name="bigd", bufs=1))
            ffw = sD.enter_context(tc.tile_pool(name="ffw", bufs=3))
            aT = big_d.tile([P, NF, 512], E4)

            def emit_aT(tch):
                # a^T half = relu(W1^T h2^T + b1) for 512 local tokens
                for ft in range(NF):
                    w1_t = ffw.tile(
                        [P, NCT, P], E4, tag="w1t", bufs=4, name=f"w1t{tch}_{ft}"
                    )
                    nc.sync.dma_start(w1_t[:], w1[ft])
                    acc = psum("a")
                    for k in range(NCT // 2):
                        nc.tensor.matmul(
                            acc[:],
                            w1_t[:, 2 * k : 2 * k + 2],
                            h2T[:, 2 * k : 2 * k + 2, tch * 512 : (tch + 1) * 512],
                            start=(k == 0),
                            stop=(k == NCT // 2 - 1),
                            perf_mode=DR,
                        )
                    nc.scalar.activation(
                        out=aT[:, ft, :],
                        in_=acc[:],
                        func=mybir.ActivationFunctionType.Relu,
                        bias=b1_t[:, ft : ft + 1],
                        scale=1.0,
                    )

            def emit_ff(tbh):
                def mk_grp():
                    return [
                        [
                            psum(
                                ("a" if tb < 1 else "b" if tb < 2 else "c"),
                                bufs=(2 if tb < 2 else 4),
                            )
                            for cc in range(2)
                        ]
                        for tb in range(4)
                    ]

                def ff_pass(dig, grp):
                    for ft in range(NF // 2):
                        w2_t = ffw.tile(
                            [P, 2, C], E4, tag="w2t", bufs=4,
                            name=f"w2t{tbh}_{dig}_{ft}",
                        )
                        nc.sync.dma_start(
                            w2_t[:], w2_r[:, dig, 2 * ft : 2 * ft + 2, :]
                        )
                        for tb in range(4):
                            for cc in range(2):
                                nc.tensor.matmul(
                                    grp[tb][cc][:],
                                    aT[:, 2 * ft : 2 * ft + 2,
                                       tb * P : (tb + 1) * P],
                                    w2_t[:, :, cc * 512 : (cc + 1) * 512],
                                    start=(ft == 0),
                                    stop=(ft == NF // 2 - 1),
                                    perf_mode=DR,
                                )

                # hi-digit pass, evacuated (+b2) into o_t
                grp = mk_grp()
                ff_pass(0, grp)
                o_ts = []
                for tb in range(4):
                    o_t = ffw.tile([P, C], F32, tag="ot", bufs=5, name=f"ot{tbh}_{tb}")
                    for cc in range(2):
                        nc.vector.tensor_add(
                            out=o_t[:, cc * 512 : (cc + 1) * 512],
                            in0=grp[tb][cc][:],
                            in1=b2_b[:, cc * 512 : (cc + 1) * 512],
                        )
                    o_ts.append(o_t)
                # lo-digit pass (weights pre-scaled x256), combined at 1/256
                grp = mk_grp()
                ff_pass(1, grp)
                for tb in range(4):
                    lt = tbh * 4 + tb
                    x2_t = ffw.tile([P, C], F32, tag="x2r", name=f"x2r{tbh}_{tb}")
                    nc.sync.dma_start(x2_t[:], x2_d[lt * P : (lt + 1) * P, :])
                    o_t = o_ts[tb]
                    lo_t = ffw.tile([P, C], F32, tag="lot", name=f"lot{tbh}_{tb}")
                    for cc in range(2):
                        nc.scalar.activation(
                            out=lo_t[:, cc * 512 : (cc + 1) * 512],
                            in_=grp[tb][cc][:],
                            func=mybir.ActivationFunctionType.Copy,
                            scale=float(1.0 / 256.0),
                        )
                    nc.vector.tensor_add(out=o_t[:], in0=o_t[:], in1=lo_t[:])
                    nc.vector.tensor_add(out=o_t[:], in0=o_t[:], in1=x2_t[:])
                    nc.sync.dma_start(out[lt * P : (lt + 1) * P, :], o_t[:])

            emit_aT(0)
            emit_ff(0)
            emit_aT(1)
            emit_ff(1)

    _split_multi_waits(nc)
    return nc


_NC_CACHE = None


def _get_nc():
    global _NC_CACHE
    if _NC_CACHE is None:
        _NC_CACHE = build_nc()
    return _NC_CACHE


def _prep_host(inputs):
    """Fold LN gains/biases into weights; build per-core input maps."""
    x = np.asarray(inputs["x"], dtype=np.float32)
    Wk = np.asarray(inputs["Wk"], dtype=np.float32)
    Wq = np.asarray(inputs["Wq"], dtype=np.float32)
    Wv = np.asarray(inputs["Wv"], dtype=np.float32)
    W1 = np.asarray(inputs["W1"], dtype=np.float32)
    b1 = np.asarray(inputs["b1"], dtype=np.float32)
    W2 = np.asarray(inputs["W2"], dtype=np.float32)
    b2 = np.asarray(inputs["b2"], dtype=np.float32)
    g1 = np.asarray(inputs["g1"], dtype=np.float32)
    be1 = np.asarray(inputs["be1"], dtype=np.float32)
    g2 = np.asarray(inputs["g2"], dtype=np.float32)
    be2 = np.asarray(inputs["be2"], dtype=np.float32)

    f8 = ml_dtypes.float8_e4m3
    bf = ml_dtypes.bfloat16
    wq_f = np.ascontiguousarray((g1[:, None] * Wq).astype(f8))
    wk_f = np.ascontiguousarray((g1[:, None] * Wk).astype(f8))
    wv_full = g1[:, None] * Wv
    wv_f = np.ascontiguousarray(wv_full.astype(f8))
    wv16_f = np.ascontiguousarray(wv_full.astype(bf))
    qb = be1 @ Wq
    kb = be1 @ Wk
    vb = be1 @ Wv
    w1_full = (g2[:, None] * W1).astype(f8)
    w1_f = np.ascontiguousarray(
        w1_full.reshape(NCT, P, NF, P).transpose(2, 1, 0, 3)
    )
    w2_hi = W2.astype(f8)
    w2_lo = (256.0 * (W2 - w2_hi.astype(np.float32))).astype(f8)
    w2_f8 = np.ascontiguousarray(np.stack([w2_hi, w2_lo], axis=0))
    b1_f = b1 + be2 @ W1

    ident = np.eye(P, dtype=ml_dtypes.bfloat16)

    # per-half masks: for local position p with global block i, the diagonal
    # 512-key chunk mask is 0 where key-col j <= (i%4)*128 + row else -1e30.
    jj = np.arange(512)[None, :]
    rr = np.arange(P)[:, None]
    masks_h = []
    for half in range(2):
        mk = np.empty((NLOC, P, 512), dtype=np.float32)
        for ppos, i in enumerate(L_HALF[half]):
            m = i % 4
            mk[ppos] = np.where(jj <= m * P + rr, 0.0, NEG)
        masks_h.append(mk)

    shared = {
        "wq": wq_f, "wk": wk_f, "wv": wv_f, "wv16": wv16_f,
        "w1": w1_f, "w2": w2_f8,
        "qb": qb, "kb": kb, "vb": vb, "b1": b1_f, "b2": b2,
        "ident": ident,
    }
    in_maps = []
    for core in range(8):
        b, half = core // 2, core % 2
        L = L_HALF[half]
        rows = np.concatenate([np.arange(i * P, (i + 1) * P) for i in L])
        m = dict(shared)
        m["x"] = np.ascontiguousarray(x[b])
        m["x_own"] = np.ascontiguousarray(x[b][rows])
        m["masks"] = masks_h[half]
        in_maps.append(m)
    return in_maps


def _scatter_out(results):
    out = np.empty((B, T, C), dtype=np.float32)
    for core in range(8):
        b, half = core // 2, core % 2
        L = L_HALF[half]
        o = results[core]["out"]
        for ppos, i in enumerate(L):
            out[b, i * P : (i + 1) * P, :] = o[ppos * P : (ppos + 1) * P, :]
    return out


def run(inputs, trace=False, **kw):
    nc = _get_nc()
    in_maps = _prep_host(inputs)
    res = run_bass_kernel_spmd(
        nc, in_maps, core_ids=list(range(8)), trace=trace, **kw
    )
    return _scatter_out(res.results), res


def kernel(**inputs) -> np.ndarray:
    out, _ = run(inputs, trace=False)
    return out


# revision 22
# speedup vs baseline: 1.0849x; 1.0849x over previous
"""Trainium2 Bass kernel for nn_Block (dense transformer block).

  out = x + FFN(LN2(x + Attn(LN1(x))))   with causal single-head attention,
  B=4, T=2048, C=H=1024, FF=4096, fp32 reference.

Distribution: 8 NeuronCores = (batch b in 0..3) x (query-half in 0..1).
Each core handles one batch element's keys/values and HALF its query rows
(causally balanced interleaved block split), plus LN2+FFN+residual for those
rows.  No collectives; the per-core programs are IDENTICAL (SPMD) - all
per-core variation is input data.

All matmul OPERANDS are fp8e4m3 driven in DoubleRow perf mode (2 fp8
weights per PE cell, K=256 contraction per matmul); every accumulation is
f32 in PSUM, and LN stats / softmax denominators / residual adds are f32.
LN gains/biases are folded into the weight matrices host-side.
"""

import sys
import types

import numpy as np

# ---------------------------------------------------------------------------
# antenv.axon_hooks shim: the image's antenv lacks this module and
# run_bass_kernel_spmd imports it under axon when trace=True.
import antenv

if "antenv.axon_hooks" not in sys.modules:
    _mod = types.ModuleType("antenv.axon_hooks")
    _mod._hook = None
    _mod.set_axon_ntff_profile_hook = lambda h: setattr(_mod, "_hook", h)
    _mod.get_axon_ntff_profile_hook = lambda: _mod._hook
    sys.modules["antenv.axon_hooks"] = _mod
    antenv.axon_hooks = _mod

import ml_dtypes

import concourse.bass as bass
import concourse.mybir as mybir
import concourse.tile as tile
from concourse.bass_utils import run_bass_kernel_spmd

F32 = mybir.dt.float32
BF16 = mybir.dt.bfloat16
E4 = mybir.dt.float8e4
DR = mybir.MatmulPerfMode.DoubleRow

B, T, C = 4, 2048, 1024
H, FF = 1024, 4096
P = 128
NT = T // P  # 16 token blocks per batch element
NCT = C // P  # 8 contraction tiles
NH = H // P  # 8 head-dim tiles
NF = FF // P  # 32 ff tiles
TOWN = T // 2  # own tokens per core (1024)
NLOC = TOWN // P  # 8 own blocks
EPS = 1e-5
SCALE = 1.0 / np.sqrt(np.float32(C))  # 1/32
NEG = -1.0e30

# Causally balanced query-block assignment (sum of chunk counts = 20 each).
L_HALF = [
    [0, 2, 4, 6, 9, 11, 13, 15],
    [1, 3, 5, 7, 8, 10, 12, 14],
]
# ceil((i+1)/4) for i in L_HALF[h] - same sequence for both halves.
NCHUNKS = [1, 1, 2, 2, 3, 3, 4, 4]


def _split_multi_waits(nc):
    """walrus here accepts at most ONE sync-wait per instruction; hoist
    extras onto injected same-engine NoOps."""
    for fn in nc.m.functions:
        for blk in fn.blocks:
            new_insts = []
            changed = False
            for inst in blk.instructions:
                si = getattr(inst, "sync_info", None)
                ow = list(si.on_wait) if si is not None and si.on_wait else []
                if len(ow) > 1:
                    for i, cond in enumerate(ow[:-1]):
                        new_insts.append(
                            mybir.InstNoOp(
                                name=f"{inst.name}-wn{i}",
                                engine=inst.engine,
                                ins=[],
                                outs=[],
                                sync_info=mybir.SyncInfo(
                                    on_wait=[cond], on_update=[]
                                ),
                            )
                        )
                    inst.sync_info = mybir.SyncInfo(
                        on_wait=[ow[-1]], on_update=list(si.on_update or [])
                    )
                    changed = True
                new_insts.append(inst)
            if changed:
                blk.instructions = new_insts


def _layernorm(nc, pool, x_t, h_t, eps_t):
    """h_t = (x_t - mean) * rsqrt(var + eps), stats along the free dim."""
    xg = x_t[:].rearrange("p (s f) -> p s f", f=512)
    stats = pool.tile([P, 2, nc.vector.BN_STATS_DIM], F32, tag="ln_stats")
    for sg in range(2):
        nc.vector.bn_stats(out=stats[:, sg], in_=xg[:, sg])
    mv = pool.tile([P, nc.vector.BN_AGGR_DIM], F32, tag="ln_mv")
    nc.vector.bn_aggr(out=mv[:], in_=stats[:])
    rstd = pool.tile([P, 1], F32, tag="ln_rstd")
    nc.scalar.activation(
        out=rstd[:],
        in_=mv[:, 1:2],
        func=mybir.ActivationFunctionType.Sqrt,
        bias=eps_t[:],
        scale=1.0,
    )
    nc.vector.reciprocal(out=rstd[:], in_=rstd[:])
    nc.vector.tensor_scalar(
        out=h_t[:],
        in0=x_t[:],
        scalar1=mv[:, 0:1],
        scalar2=rstd[:],
        op0=mybir.AluOpType.subtract,
        op1=mybir.AluOpType.mult,
    )


def build_nc():
    from contextlib import ExitStack

    nc = bass.Bass()

    x = nc.declare_dram_parameter("x", [T, C], F32, isOutput=False)
    x_own = nc.declare_dram_parameter("x_own", [TOWN, C], F32, isOutput=False)
    wq = nc.declare_dram_parameter("wq", [C, H], E4, isOutput=False)
    wk = nc.declare_dram_parameter("wk", [C, H], E4, isOutput=False)
    wv = nc.declare_dram_parameter("wv", [C, H], E4, isOutput=False)
    wv16 = nc.declare_dram_parameter("wv16", [C, H], BF16, isOutput=False)
    w1 = nc.declare_dram_parameter("w1", [NF, P, NCT, P], E4, isOutput=False)
    # w2[0] = e4m3(W2), w2[1] = e4m3(256*(W2 - w2[0])) - two-digit fp8
    w2 = nc.declare_dram_parameter("w2", [2, FF, C], E4, isOutput=False)
    qb = nc.declare_dram_parameter("qb", [H], F32, isOutput=False)
    kb = nc.declare_dram_parameter("kb", [H], F32, isOutput=False)
    vb = nc.declare_dram_parameter("vb", [H], F32, isOutput=False)
    b1 = nc.declare_dram_parameter("b1", [FF], F32, isOutput=False)
    b2 = nc.declare_dram_parameter("b2", [C], F32, isOutput=False)
    ident = nc.declare_dram_parameter("ident", [P, P], BF16, isOutput=False)
    masks = nc.declare_dram_parameter("masks", [NLOC, P, 512], F32, isOutput=False)
    out = nc.declare_dram_parameter("out", [TOWN, C], F32, isOutput=True)

    x2_d = nc.dram_tensor("x2_d", [TOWN, C], F32)

    wq_r = wq.rearrange("(ko p) h -> p ko h", p=P)
    wk_r = wk.rearrange("(ko p) h -> p ko h", p=P)
    wv_r = wv.rearrange("(ko p) h -> p ko h", p=P)
    wv16_r = wv16.rearrange("(ko p) h -> p ko h", p=P)
    w2_r = w2.rearrange("t (ko p) c -> p t ko c", p=P)

    with tile.TileContext(nc) as tc, ExitStack() as top:
        cn = top.enter_context(tc.tile_pool(name="cn", bufs=1))
        ps = top.enter_context(tc.tile_pool(name="ps", bufs=1, space="PSUM"))
        ln = top.enter_context(tc.tile_pool(name="ln", bufs=4))
        # big resident tensors spanning several phases
        res = top.enter_context(tc.tile_pool(name="res", bufs=1))

        # critical-path-first: the very first LN tile and the transpose
        # identity go ahead of the bulk constant loads.
        x0_t = ln.tile([P, C], F32, tag="xt", name="x0t")
        nc.sync.dma_start(x0_t[:], x_own[0:P, :])
        id_t = cn.tile([P, P], BF16)
        nc.sync.dma_start(id_t[:], ident[:])
        # ---- constants
        qb_t = cn.tile([P, NH], F32)
        nc.sync.dma_start(qb_t[:], qb.rearrange("(m p) -> p m", p=P))
        kb_t = cn.tile([P, NH], F32)
        nc.sync.dma_start(kb_t[:], kb.rearrange("(m p) -> p m", p=P))
        b1_t = cn.tile([P, NF], F32)
        nc.sync.dma_start(b1_t[:], b1.rearrange("(m p) -> p m", p=P))
        vb_b = cn.tile([P, H], F32)
        nc.sync.dma_start(vb_b[:], vb[None, :].partition_broadcast(P))
        b2_b = cn.tile([P, C], F32)
        nc.sync.dma_start(b2_b[:], b2[None, :].partition_broadcast(P))
        eps_t = cn.tile([P, 1], F32)
        nc.vector.memset(eps_t, EPS)

        _ctr = [0]

        def psum(tag, shape=(P, 512), dt=F32, bufs=2):
            _ctr[0] += 1
            return ps.tile(list(shape), dt, tag=tag, bufs=bufs, name=f"ps{_ctr[0]}")

        qTo = res.tile([P, NH, TOWN], E4)   # q^T own tokens (local order)
        kT = res.tile([P, NH, T], E4)       # k^T all keys
        v_sb = res.tile([P, NT, H], E4)     # v token-major, all keys
        h2T = res.tile([P, NCT, TOWN], E4)  # LN2 output transposed
        # precise (bf16) head-block path: first own block attends few keys, so
        # fp8 noise passes straight through - keep that slice in bf16.
        qTb = res.tile([P, NH, P], BF16)    # q^T for local block 0
        kTb = res.tile([P, NH, 512], BF16)  # k^T for key chunk 0
        v_bf = res.tile([P, 2, H], BF16)    # v for key blocks 0-1 (bf16 matmul)

        # ================= Phase A: own-token LN1 -> hTo -> q^T ============
        with ExitStack() as sA:
            big_a = sA.enter_context(tc.tile_pool(name="biga", bufs=1))
            wka = sA.enter_context(tc.tile_pool(name="wka", bufs=2))
            hto = big_a.tile([P, NCT, TOWN], E4)
            wq_pre = []
            for qg in range(2):
                w_t = wka.tile([P, NCT, 256], E4, tag="w", name=f"wqp{qg}")
                nc.sync.dma_start(w_t[:], wq_r[:, :, qg * 256 : (qg + 1) * 256])
                wq_pre.append(w_t)
            for lt in range(NLOC):
                if lt == 0:
                    x_t = x0_t
                else:
                    x_t = ln.tile([P, C], F32, tag="xt")
                    nc.sync.dma_start(x_t[:], x_own[lt * P : (lt + 1) * P, :])
                h_t = ln.tile([P, C], BF16, tag="ht")
                _layernorm(nc, ln, x_t, h_t, eps_t)
                for c in range(NCT):
                    tp = psum("b", (P, P), BF16)
                    nc.tensor.transpose(tp[:], h_t[:, c * P : (c + 1) * P], id_t[:])
                    nc.vector.tensor_copy(hto[:, c, lt * P : (lt + 1) * P], tp[:])
            # q^T: quarter weight tiles, stationary reused over 2 token groups
            for qg in range(4):
                w_t = wq_pre[qg] if qg < 2 else wka.tile([P, NCT, 256], E4, tag="w")
                if qg >= 2:
                    nc.sync.dma_start(w_t[:], wq_r[:, :, qg * 256 : (qg + 1) * 256])
                for mi in range(2):
                    m = qg * 2 + mi
                    accs = [psum("a") for _ in range(2)]
                    for k in range(NCT // 2):
                        for g in range(2):
                            nc.tensor.matmul(
                                accs[g][:],
                                w_t[:, 2 * k : 2 * k + 2, mi * P : (mi + 1) * P],
                                hto[:, 2 * k : 2 * k + 2, g * 512 : (g + 1) * 512],
                                start=(k == 0),
                                stop=(k == NCT // 2 - 1),
                                perf_mode=DR,
                            )
                    for g in range(2):
                        nc.vector.tensor_scalar_add(
                            out=qTo[:, m, g * 512 : (g + 1) * 512],
                            in0=accs[g][:],
                            scalar1=qb_t[:, m : m + 1],
                        )
                    nc.vector.tensor_scalar_add(
                        out=qTb[:, m, :],
                        in0=accs[0][:, 0:P],
                        scalar1=qb_t[:, m : m + 1],
                    )

        # ================= Phase B: full LN1 -> hT; v interleaved; k^T =====
        with ExitStack() as sB:
            big_b = sB.enter_context(tc.tile_pool(name="bigb", bufs=1))
            wkb = sB.enter_context(tc.tile_pool(name="wkb", bufs=2))
            wvp = sB.enter_context(tc.tile_pool(name="wvp", bufs=1))
            hT = big_b.tile([P, NCT, T], E4)
            hT_bf = big_b.tile([P, NCT, 256], BF16)
            wv_t = wvp.tile([P, NCT, H], E4)
            nc.sync.dma_start(wv_t[:], wv_r[:])
            wv16_t = wvp.tile([P, NCT, H], BF16)
            nc.sync.dma_start(wv16_t[:], wv16_r[:])
            wk_pre = []
            for qg in range(2):
                w_t = wkb.tile([P, NCT, 256], E4, tag="w", name=f"wkp{qg}")
                nc.sync.dma_start(w_t[:], wk_r[:, :, qg * 256 : (qg + 1) * 256])
                wk_pre.append(w_t)
            # LN per global token tile; v row-block immediately after its tile
            for st in range(NT):
                x_t = ln.tile([P, C], F32, tag="xt")
                nc.sync.dma_start(x_t[:], x[st * P : (st + 1) * P, :])
                h_t = ln.tile([P, C], BF16, tag="ht")
                _layernorm(nc, ln, x_t, h_t, eps_t)
                for c in range(NCT):
                    tp = psum("b", (P, P), BF16)
                    nc.tensor.transpose(tp[:], h_t[:, c * P : (c + 1) * P], id_t[:])
                    nc.vector.tensor_copy(hT[:, c, st * P : (st + 1) * P], tp[:])
                    if st < 2:
                        nc.vector.tensor_copy(
                            hT_bf[:, c, st * P : (st + 1) * P], tp[:]
                        )
                accs = [psum("a") for _ in range(2)]
                for k in range(NCT // 2):
                    for hh in range(2):
                        nc.tensor.matmul(
                            accs[hh][:],
                            hT[:, 2 * k : 2 * k + 2, st * P : (st + 1) * P],
                            wv_t[:, 2 * k : 2 * k + 2, hh * 512 : (hh + 1) * 512],
                            start=(k == 0),
                            stop=(k == NCT // 2 - 1),
                            perf_mode=DR,
                        )
                for hh in range(2):
                    nc.vector.tensor_add(
                        out=v_sb[:, st, hh * 512 : (hh + 1) * 512],
                        in0=accs[hh][:],
                        in1=vb_b[:, hh * 512 : (hh + 1) * 512],
                    )
            # precise bf16 v for key blocks 0-1
            for blk in range(2):
                accs = [psum("a") for _ in range(2)]
                for k in range(NCT):
                    for hh in range(2):
                        nc.tensor.matmul(
                            accs[hh][:],
                            hT_bf[:, k, blk * P : (blk + 1) * P],
                            wv16_t[:, k, hh * 512 : (hh + 1) * 512],
                            start=(k == 0),
                            stop=(k == NCT - 1),
                        )
                for hh in range(2):
                    nc.vector.tensor_add(
                        out=v_bf[:, blk, hh * 512 : (hh + 1) * 512],
                        in0=accs[hh][:],
                        in1=vb_b[:, hh * 512 : (hh + 1) * 512],
                    )
            # k^T: quarter weight tiles, stationary reused over 4 key chunks
            for qg in range(4):
                w_t = wk_pre[qg] if qg < 2 else wkb.tile([P, NCT, 256], E4, tag="w")
                if qg >= 2:
                    nc.sync.dma_start(w_t[:], wk_r[:, :, qg * 256 : (qg + 1) * 256])
                for mi in range(2):
                    m = qg * 2 + mi
                    accs = [psum("c", bufs=4) for _ in range(4)]
                    for k in range(NCT // 2):
                        for ch in range(4):
                            nc.tensor.matmul(
                                accs[ch][:],
                                w_t[:, 2 * k : 2 * k + 2, mi * P : (mi + 1) * P],
                                hT[:, 2 * k : 2 * k + 2, ch * 512 : (ch + 1) * 512],
                                start=(k == 0),
                                stop=(k == NCT // 2 - 1),
                                perf_mode=DR,
                            )
                    for ch in range(4):
                        nc.vector.tensor_scalar_add(
                            out=kT[:, m, ch * 512 : (ch + 1) * 512],
                            in0=accs[ch][:],
                            scalar1=kb_t[:, m : m + 1],
                        )
                    nc.vector.tensor_scalar_add(
                        out=kTb[:, m, :],
                        in0=accs[0][:],
                        scalar1=kb_t[:, m : m + 1],
                    )

        # ============== Phase C: attention (software-pipelined) ============
        with ExitStack() as sC:
            att = sC.enter_context(tc.tile_pool(name="att", bufs=2))
            wtl = sC.enter_context(tc.tile_pool(name="wtl", bufs=16))
            state = {}

            def emit_scores(lp):
                nch = NCHUNKS[lp]
                mask_t = att.tile([P, 512], F32, tag="mask")
                nc.sync.dma_start(mask_t[:], masks[lp])
                p_t = att.tile([P, T], BF16, tag="pt", bufs=3)
                den = att.tile([P, 4], F32, tag="den")
                scs = [psum("c", bufs=4) for _ in range(nch)]
                if lp == 0:
                    # precise bf16 scores for the head block
                    for m in range(NH):
                        nc.tensor.matmul(
                            scs[0][:],
                            qTb[:, m, :],
                            kTb[:, m, :],
                            start=(m == 0),
                            stop=(m == NH - 1),
                        )
                else:
                    for m in range(NH // 2):
                        for j in range(nch):
                            nc.tensor.matmul(
                                scs[j][:],
                                qTo[:, 2 * m : 2 * m + 2, lp * P : (lp + 1) * P],
                                kT[:, 2 * m : 2 * m + 2, j * 512 : (j + 1) * 512],
                                start=(m == 0),
                                stop=(m == NH // 2 - 1),
                                perf_mode=DR,
                            )
                for j in range(nch):
                    if j == nch - 1:
                        nc.vector.tensor_add(
                            out=scs[j][:], in0=scs[j][:], in1=mask_t[:]
                        )
                    nc.scalar.activation(
                        out=p_t[:, j * 512 : (j + 1) * 512],
                        in_=scs[j][:],
                        func=mybir.ActivationFunctionType.Exp,
                        scale=float(SCALE),
                        accum_out=den[:, j : j + 1],
                    )
                state[lp] = (p_t, den)

            def emit_tail(lp):
                nch = NCHUNKS[lp]
                nst = 4 * nch
                p_t, den = state.pop(lp)
                dsum = att.tile([P, 1], F32, tag="dsum")
                nc.vector.reduce_sum(
                    out=dsum[:], in_=den[:, :nch], axis=mybir.AxisListType.X
                )
                nc.vector.reciprocal(out=dsum[:], in_=dsum[:])
                sa0 = psum("a")
                sa1 = psum("a")
                if lp == 0:
                    # precise bf16 p@v over key blocks 0-1 (rest masked to zero)
                    wtb = []
                    for st in range(2):
                        tp = psum("b", (P, P), BF16)
                        nc.tensor.transpose(
                            tp[:], p_t[:, st * P : (st + 1) * P], id_t[:]
                        )
                        wt = wtl.tile([P, P], BF16, tag="wtb")
                        nc.vector.tensor_copy(wt[:], tp[:])
                        wtb.append(wt)
                    for st in range(2):
                        nc.tensor.matmul(
                            sa0[:], wtb[st][:], v_bf[:, st, 0:512],
                            start=(st == 0), stop=(st == 1),
                        )
                        nc.tensor.matmul(
                            sa1[:], wtb[st][:], v_bf[:, st, 512:1024],
                            start=(st == 0), stop=(st == 1),
                        )
                else:
                    wtp = []
                    for sp in range(nst // 2):
                        wt = wtl.tile([P, 2, P], E4, tag="wt")
                        for u in range(2):
                            st = 2 * sp + u
                            tp = psum("b", (P, P), BF16)
                            nc.tensor.transpose(
                                tp[:], p_t[:, st * P : (st + 1) * P], id_t[:]
                            )
                            nc.vector.tensor_copy(wt[:, u], tp[:])
                        wtp.append(wt)
                    for sp in range(nst // 2):
                        nc.tensor.matmul(
                            sa0[:], wtp[sp][:], v_sb[:, 2 * sp : 2 * sp + 2, 0:512],
                            start=(sp == 0), stop=(sp == nst // 2 - 1),
                            perf_mode=DR,
                        )
                        nc.tensor.matmul(
                            sa1[:], wtp[sp][:],
                            v_sb[:, 2 * sp : 2 * sp + 2, 512:1024],
                            start=(sp == 0), stop=(sp == nst // 2 - 1),
                            perf_mode=DR,
                        )
                x_t = att.tile([P, C], F32, tag="xo")
                nc.sync.dma_start(x_t[:], x_own[lp * P : (lp + 1) * P, :])
                x2_t = att.tile([P, C], F32, tag="x2")
                nc.vector.tensor_scalar_mul(
                    out=x2_t[:, 0:512], in0=sa0[:], scalar1=dsum[:]
                )
                nc.vector.tensor_scalar_mul(
                    out=x2_t[:, 512:1024], in0=sa1[:], scalar1=dsum[:]
                )
                nc.vector.tensor_add(out=x2_t[:], in0=x2_t[:], in1=x_t[:])
                nc.sync.dma_start(x2_d[lp * P : (lp + 1) * P, :], x2_t[:])

            emit_scores(0)
            emit_scores(1)
            for lp in range(2, NLOC):
                emit_scores(lp)
                emit_tail(lp - 2)
            emit_tail(NLOC - 2)
            emit_tail(NLOC - 1)

        # ============== Phase C2: LN2 + h2^T ===============================
        for lt in range(NLOC):
            x2_t = ln.tile([P, C], F32, tag="xt")
            nc.sync.dma_start(x2_t[:], x2_d[lt * P : (lt + 1) * P, :])
            h2_t = ln.tile([P, C], BF16, tag="ht")
            _layernorm(nc, ln, x2_t, h2_t, eps_t)
            for c in range(NCT):
                tp = psum("b", (P, P), BF16)
                nc.tensor.transpose(tp[:], h2_t[:, c * P : (c + 1) * P], id_t[:])
                nc.vector.tensor_copy(h2T[:, c, lt * P : (lt + 1) * P], tp[:])

        # ================= Phase D: FFN (fp8 DoubleRow) ====================
        with ExitStack() as sD:
            big_d = sD.enter_context(tc.tile_pool(name="bigd", bufs=1))
            ffw = sD.enter_context(tc.tile_pool(name="ffw", bufs=3))
            aT = big_d.tile([P, NF, 512], E4)

            def emit_aT(tch):
                # a^T half = relu(W1^T h2^T + b1) for 512 local tokens
                for ft in range(NF):
                    w1_t = ffw.tile(
                        [P, NCT, P], E4, tag="w1t", bufs=4, name=f"w1t{tch}_{ft}"
                    )
                    nc.sync.dma_start(w1_t[:], w1[ft])
                    acc = psum("a")
                    for k in range(NCT // 2):
                        nc.tensor.matmul(
                            acc[:],
                            w1_t[:, 2 * k : 2 * k + 2],
                            h2T[:, 2 * k : 2 * k + 2, tch * 512 : (tch + 1) * 512],
                            start=(k == 0),
                            stop=(k == NCT // 2 - 1),
                            perf_mode=DR,
                        )
                    nc.scalar.activation(
                        out=aT[:, ft, :],
                        in_=acc[:],
                        func=mybir.ActivationFunctionType.Relu,
                        bias=b1_t[:, ft : ft + 1],
                        scale=1.0,
                    )

            def emit_ff(tbh):
                def mk_grp():
                    return [
                        [
                            psum(
                                ("a" if tb < 1 else "b" if tb < 2 else "c"),
                                bufs=(2 if tb < 2 else 4),
                            )
                            for cc in range(2)
                        ]
                        for tb in range(4)
                    ]

                def ff_pass(dig, grp):
                    for ft in range(NF // 2):
                        w2_t = ffw.tile(
                            [P, 2, C], E4, tag="w2t", bufs=4,
                            name=f"w2t{tbh}_{dig}_{ft}",
                        )
                        nc.sync.dma_start(
                            w2_t[:], w2_r[:, dig, 2 * ft : 2 * ft + 2, :]
                        )
                        for tb in range(4):
                            for cc in range(2):
                                nc.tensor.matmul(
                                    grp[tb][cc][:],
                                    aT[:, 2 * ft : 2 * ft + 2,
                                       tb * P : (tb + 1) * P],
                                    w2_t[:, :, cc * 512 : (cc + 1) * 512],
                                    start=(ft == 0),
                                    stop=(ft == NF // 2 - 1),
                                    perf_mode=DR,
                                )

                # hi-digit pass, evacuated (+b2) into o_t
                grp = mk_grp()
                ff_pass(0, grp)
                o_ts = []
                for tb in range(4):
                    o_t = ffw.tile([P, C], F32, tag="ot", bufs=5, name=f"ot{tbh}_{tb}")
                    for cc in range(2):
                        nc.vector.tensor_add(
                            out=o_t[:, cc * 512 : (cc + 1) * 512],
                            in0=grp[tb][cc][:],
                            in1=b2_b[:, cc * 512 : (cc + 1) * 512],
                        )
                    o_ts.append(o_t)
                # lo-digit pass (weights pre-scaled x256), combined at 1/256
                grp = mk_grp()
                ff_pass(1, grp)
                for tb in range(4):
                    lt = tbh * 4 + tb
                    x2_t = ffw.tile([P, C], F32, tag="x2r", name=f"x2r{tbh}_{tb}")
                    nc.sync.dma_start(x2_t[:], x2_d[lt * P : (lt + 1) * P, :])
                    o_t = o_ts[tb]
                    lo_t = ffw.tile([P, C], F32, tag="lot", name=f"lot{tbh}_{tb}")
                    for cc in range(2):
                        nc.scalar.activation(
                            out=lo_t[:, cc * 512 : (cc + 1) * 512],
                            in_=grp[tb][cc][:],
                            func=mybir.ActivationFunctionType.Copy,
                            scale=float(1.0 / 256.0),
                        )
                    nc.vector.tensor_add(out=o_t[:], in0=o_t[:], in1=lo_t[:])
                    nc.vector.tensor_add(out=o_t[:], in0=o_t[:], in1=x2_t[:])
                    nc.sync.dma_start(out[lt * P : (lt + 1) * P, :], o_t[:])

            emit_aT(0)
            emit_ff(0)
            emit_aT(1)
            emit_ff(1)

    _split_multi_waits(nc)
    return nc


_NC_CACHE = None


def _get_nc():
    global _NC_CACHE
    if _NC_CACHE is None:
        _NC_CACHE = build_nc()
    return _NC_CACHE


def _prep_host(inputs):
    """Fold LN gains/biases into weights; build per-core input maps."""
    x = np.asarray(inputs["x"], dtype=np.float32)
    Wk = np.asarray(inputs["Wk"], dtype=np.float32)
    Wq = np.asarray(inputs["Wq"], dtype=np.float32)
    Wv = np.asarray(inputs["Wv"], dtype=np.float32)
    W1 = np.asarray(inputs["W1"], dtype=np.float32)
    b1 = np.asarray(inputs["b1"], dtype=np.float32)
    W2 = np.asarray(inputs["W2"], dtype=np.float32)
    b2 = np.asarray(inputs["b2"], dtype=np.float32)
    g1 = np.asarray(inputs["g1"], dtype=np.float32)
    be1 = np.asarray(inputs["be1"], dtype=np.float32)
    g2 = np.asarray(inputs["g2"], dtype=np.float32)
    be2 = np.asarray(inputs["be2"], dtype=np.float32)

    f8 = ml_dtypes.float8_e4m3
    bf = ml_dtypes.bfloat16
    wq_f = np.ascontiguousarray((g1[:, None] * Wq).astype(f8))
    wk_f = np.ascontiguousarray((g1[:, None] * Wk).astype(f8))
    wv_full = g1[:, None] * Wv
    wv_f = np.ascontiguousarray(wv_full.astype(f8))
    wv16_f = np.ascontiguousarray(wv_full.astype(bf))
    qb = be1 @ Wq
    kb = be1 @ Wk
    vb = be1 @ Wv
    w1_full = (g2[:, None] * W1).astype(f8)
    w1_f = np.ascontiguousarray(
        w1_full.reshape(NCT, P, NF, P).transpose(2, 1, 0, 3)
    )
    w2_hi = W2.astype(f8)
    w2_lo = (256.0 * (W2 - w2_hi.astype(np.float32))).astype(f8)
    w2_f8 = np.ascontiguousarray(np.stack([w2_hi, w2_lo], axis=0))
    b1_f = b1 + be2 @ W1

    ident = np.eye(P, dtype=ml_dtypes.bfloat16)

    # per-half masks: for local position p with global block i, the diagonal
    # 512-key chunk mask is 0 where key-col j <= (i%4)*128 + row else -1e30.
    jj = np.arange(512)[None, :]
    rr = np.arange(P)[:, None]
    masks_h = []
    for half in range(2):
        mk = np.empty((NLOC, P, 512), dtype=np.float32)
        for ppos, i in enumerate(L_HALF[half]):
            m = i % 4
            mk[ppos] = np.where(jj <= m * P + rr, 0.0, NEG)
        masks_h.append(mk)

    shared = {
        "wq": wq_f, "wk": wk_f, "wv": wv_f, "wv16": wv16_f,
        "w1": w1_f, "w2": w2_f8,
        "qb": qb, "kb": kb, "vb": vb, "b1": b1_f, "b2": b2,
        "ident": ident,
    }
    in_maps = []
    for core in range(8):
        b, half = core // 2, core % 2
        L = L_HALF[half]
        rows = np.concatenate([np.arange(i * P, (i + 1) * P) for i in L])
        m = dict(shared)
        m["x"] = np.ascontiguousarray(x[b])
        m["x_own"] = np.ascontiguousarray(x[b][rows])
        m["masks"] = masks_h[half]
        in_maps.append(m)
    return in_maps


def _scatter_out(results):
    out = np.empty((B, T, C), dtype=np.float32)
    for core in range(8):
        b, half = core // 2, core % 2
        L = L_HALF[half]
        o = results[core]["out"]
        for ppos, i in enumerate(L):
            out[b, i * P : (i + 1) * P, :] = o[ppos * P : (ppos + 1) * P, :]
    return out


def run(inputs, trace=False, **kw):
    nc = _get_nc()
    in_maps = _prep_host(inputs)
    res = run_bass_kernel_spmd(
        nc, in_maps, core_ids=list(range(8)), trace=trace, **kw
    )
    return _scatter_out(res.results), res


def kernel(**inputs) -> np.ndarray:
    out, _ = run(inputs, trace=False)
    return out


# revision 31
# speedup vs baseline: 1.1786x; 1.0864x over previous
"""Trainium2 Bass kernel for nn_Block (dense transformer block).

  out = x + FFN(LN2(x + Attn(LN1(x))))   with causal single-head attention,
  B=4, T=2048, C=H=1024, FF=4096, fp32 reference.

Distribution: 8 NeuronCores = (batch b in 0..3) x (query-half in 0..1).
Each core handles one batch element's keys/values and HALF its query rows
(causally balanced interleaved block split), plus LN2+FFN+residual for those
rows.  No collectives; the per-core programs are IDENTICAL (SPMD) - all
per-core variation is input data.

All matmul OPERANDS are fp8e4m3 driven in DoubleRow perf mode (2 fp8
weights per PE cell, K=256 contraction per matmul); every accumulation is
f32 in PSUM, and LN stats / softmax denominators / residual adds are f32.
LN gains/biases are folded into the weight matrices host-side.
"""

import sys
import types

import numpy as np

# ---------------------------------------------------------------------------
# antenv.axon_hooks shim: the image's antenv lacks this module and
# run_bass_kernel_spmd imports it under axon when trace=True.
import antenv

if "antenv.axon_hooks" not in sys.modules:
    _mod = types.ModuleType("antenv.axon_hooks")
    _mod._hook = None
    _mod.set_axon_ntff_profile_hook = lambda h: setattr(_mod, "_hook", h)
    _mod.get_axon_ntff_profile_hook = lambda: _mod._hook
    sys.modules["antenv.axon_hooks"] = _mod
    antenv.axon_hooks = _mod

import ml_dtypes

import concourse.bass as bass
import concourse.mybir as mybir
import concourse.tile as tile
from concourse.bass_utils import run_bass_kernel_spmd

F32 = mybir.dt.float32
BF16 = mybir.dt.bfloat16
E4 = mybir.dt.float8e4
DR = mybir.MatmulPerfMode.DoubleRow

B, T, C = 4, 2048, 1024
H, FF = 1024, 4096
P = 128
NT = T // P  # 16 token blocks per batch element
NCT = C // P  # 8 contraction tiles
NH = H // P  # 8 head-dim tiles
NF = FF // P  # 32 ff tiles
TOWN = T // 2  # own tokens per core (1024)
NLOC = TOWN // P  # 8 own blocks
EPS = 1e-5
SCALE = 1.0 / np.sqrt(np.float32(C))  # 1/32
NEG = -1.0e30

# Causally balanced query-block assignment (sum of chunk counts = 20 each).
L_HALF = [
    [0, 2, 4, 6, 9, 11, 13, 15],
    [1, 3, 5, 7, 8, 10, 12, 14],
]
# ceil((i+1)/4) for i in L_HALF[h] - same sequence for both halves.
NCHUNKS = [1, 1, 2, 2, 3, 3, 4, 4]
# Per-core BLOCK PERMUTATION of x: own blocks sit at even positions, the
# other half's at odd positions.  The program then addresses own tokens at
# fixed (core-independent) offsets; all per-core variation stays in data.
PERMS = [
    [L_HALF[h][m // 2] if m % 2 == 0 else L_HALF[1 - h][m // 2] for m in range(NT)]
    for h in range(2)
]


def _split_multi_waits(nc):
    """walrus here accepts at most ONE sync-wait per instruction; hoist
    extras onto injected same-engine NoOps."""
    for fn in nc.m.functions:
        for blk in fn.blocks:
            new_insts = []
            changed = False
            for inst in blk.instructions:
                si = getattr(inst, "sync_info", None)
                ow = list(si.on_wait) if si is not None and si.on_wait else []
                if len(ow) > 1:
                    for i, cond in enumerate(ow[:-1]):
                        new_insts.append(
                            mybir.InstNoOp(
                                name=f"{inst.name}-wn{i}",
                                engine=inst.engine,
                                ins=[],
                                outs=[],
                                sync_info=mybir.SyncInfo(
                                    on_wait=[cond], on_update=[]
                                ),
                            )
                        )
                    inst.sync_info = mybir.SyncInfo(
                        on_wait=[ow[-1]], on_update=list(si.on_update or [])
                    )
                    changed = True
                new_insts.append(inst)
            if changed:
                blk.instructions = new_insts


def _layernorm(nc, pool, x_t, h_t, eps_t):
    """h_t = (x_t - mean) * rsqrt(var + eps), stats along the free dim."""
    xg = x_t[:].rearrange("p (s f) -> p s f", f=512)
    stats = pool.tile([P, 2, nc.vector.BN_STATS_DIM], F32, tag="ln_stats")
    for sg in range(2):
        nc.vector.bn_stats(out=stats[:, sg], in_=xg[:, sg])
    mv = pool.tile([P, nc.vector.BN_AGGR_DIM], F32, tag="ln_mv")
    nc.vector.bn_aggr(out=mv[:], in_=stats[:])
    rstd = pool.tile([P, 1], F32, tag="ln_rstd")
    nc.scalar.activation(
        out=rstd[:],
        in_=mv[:, 1:2],
        func=mybir.ActivationFunctionType.Sqrt,
        bias=eps_t[:],
        scale=1.0,
    )
    nc.vector.reciprocal(out=rstd[:], in_=rstd[:])
    nc.vector.tensor_scalar(
        out=h_t[:],
        in0=x_t[:],
        scalar1=mv[:, 0:1],
        scalar2=rstd[:],
        op0=mybir.AluOpType.subtract,
        op1=mybir.AluOpType.mult,
    )


def build_nc():
    from contextlib import ExitStack

    nc = bass.Bass()

    x = nc.declare_dram_parameter("x", [T, C], F32, isOutput=False)
    x_bf = nc.declare_dram_parameter("x_bf", [T, C], BF16, isOutput=False)
    wq = nc.declare_dram_parameter("wq", [C, H], E4, isOutput=False)
    wk = nc.declare_dram_parameter("wk", [C, H], E4, isOutput=False)
    wv = nc.declare_dram_parameter("wv", [C, H], E4, isOutput=False)
    wv16 = nc.declare_dram_parameter("wv16", [C, H], BF16, isOutput=False)
    w1 = nc.declare_dram_parameter("w1", [NF, P, NCT, P], E4, isOutput=False)
    # w2[0] = e4m3(W2), w2[1] = e4m3(256*(W2 - w2[0])) - two-digit fp8
    w2 = nc.declare_dram_parameter("w2", [2, FF, C], E4, isOutput=False)
    qb = nc.declare_dram_parameter("qb", [H], F32, isOutput=False)
    kb = nc.declare_dram_parameter("kb", [H], F32, isOutput=False)
    vb = nc.declare_dram_parameter("vb", [H], F32, isOutput=False)
    b1 = nc.declare_dram_parameter("b1", [FF], F32, isOutput=False)
    b2 = nc.declare_dram_parameter("b2", [C], F32, isOutput=False)
    ident = nc.declare_dram_parameter("ident", [P, P], BF16, isOutput=False)
    masks = nc.declare_dram_parameter("masks", [NLOC, P, 512], F32, isOutput=False)
    out = nc.declare_dram_parameter("out", [TOWN, C], F32, isOutput=True)

    x2_d = nc.dram_tensor("x2_d", [TOWN, C], F32)

    wq_r = wq.rearrange("(ko p) h -> p ko h", p=P)
    wk_r = wk.rearrange("(ko p) h -> p ko h", p=P)
    wv_r = wv.rearrange("(ko p) h -> p ko h", p=P)
    wv16_r = wv16.rearrange("(ko p) h -> p ko h", p=P)
    w2_r = w2.rearrange("t (ko p) c -> p t ko c", p=P)

    with tile.TileContext(nc) as tc, ExitStack() as top:
        cn = top.enter_context(tc.tile_pool(name="cn", bufs=1))
        ps = top.enter_context(tc.tile_pool(name="ps", bufs=1, space="PSUM"))
        ln = top.enter_context(tc.tile_pool(name="ln", bufs=4))
        # big resident tensors spanning several phases
        res = top.enter_context(tc.tile_pool(name="res", bufs=1))

        # critical-path-first: the very first LN tile and the transpose
        # identity go ahead of the bulk constant loads.
        x0_t = ln.tile([P, C], BF16, tag="xt", name="x0t")
        nc.sync.dma_start(x0_t[:], x_bf[0:P, :])
        id_t = cn.tile([P, P], BF16)
        nc.sync.dma_start(id_t[:], ident[:])
        # ---- constants
        qb_t = cn.tile([P, NH], F32)
        nc.sync.dma_start(qb_t[:], qb.rearrange("(m p) -> p m", p=P))
        kb_t = cn.tile([P, NH], F32)
        nc.sync.dma_start(kb_t[:], kb.rearrange("(m p) -> p m", p=P))
        b1_t = cn.tile([P, NF], F32)
        nc.sync.dma_start(b1_t[:], b1.rearrange("(m p) -> p m", p=P))
        vb_b = cn.tile([P, H], F32)
        nc.sync.dma_start(vb_b[:], vb[None, :].partition_broadcast(P))
        b2_b = cn.tile([P, C], F32)
        nc.sync.dma_start(b2_b[:], b2[None, :].partition_broadcast(P))
        eps_t = cn.tile([P, 1], F32)
        nc.vector.memset(eps_t, EPS)

        _ctr = [0]

        def psum(tag, shape=(P, 512), dt=F32, bufs=2):
            _ctr[0] += 1
            return ps.tile(list(shape), dt, tag=tag, bufs=bufs, name=f"ps{_ctr[0]}")

        qTo = res.tile([P, NH, TOWN], E4)   # q^T own tokens (local order)
        kT = res.tile([P, NH, T], E4)       # k^T all keys
        v_sb = res.tile([P, NT, H], E4)     # v token-major, all keys
        h2T = res.tile([P, NCT, TOWN], E4)  # LN2 output transposed
        # precise (bf16) head-block path: first own block attends few keys, so
        # fp8 noise passes straight through - keep that slice in bf16.
        qTb = res.tile([P, NH, P], BF16)    # q^T for local block 0
        kTb = res.tile([P, NH, 512], BF16)  # k^T for key chunk 0
        v_bf = res.tile([P, 2, H], BF16)    # v for key blocks 0-1 (bf16 matmul)

        # ===== Phase B: LN1 over permuted blocks -> hT/hto; v; q^T; k^T ====
        # x arrives block-PERMUTED per core (own blocks at even positions),
        # so one LN pass feeds hT (all keys) and hto (own tokens, gathered by
        # the scalar engine) - no duplicated LN/transposes, and the v matmuls
        # give the PE work from the first tile on.
        with ExitStack() as sB:
            big_b = sB.enter_context(tc.tile_pool(name="bigb", bufs=1))
            wvp = sB.enter_context(tc.tile_pool(name="wvp", bufs=1))
            hT = big_b.tile([P, NCT, T], E4)
            hto = big_b.tile([P, NCT, TOWN], E4)
            hT_bf = big_b.tile([P, NCT, 256], BF16)
            wv_t = wvp.tile([P, NCT, H], E4)
            nc.sync.dma_start(wv_t[:], wv_r[:])
            wq_t = wvp.tile([P, NCT, H], E4)
            nc.sync.dma_start(wq_t[:], wq_r[:])
            wk_t = wvp.tile([P, NCT, H], E4)
            nc.sync.dma_start(wk_t[:], wk_r[:])
            wv16_t = wvp.tile([P, NCT, H], BF16)
            nc.sync.dma_start(wv16_t[:], wv16_r[:])
            # LN per permuted token tile; v row-block right after its tile
            for st in range(NT):
                if st == 0:
                    x_t = x0_t
                else:
                    x_t = ln.tile([P, C], BF16, tag="xb")
                    nc.sync.dma_start(x_t[:], x_bf[st * P : (st + 1) * P, :])
                h_t = ln.tile([P, C], BF16, tag="ht")
                _layernorm(nc, ln, x_t, h_t, eps_t)
                for c in range(NCT):
                    tp = psum("b", (P, P), BF16)
                    nc.tensor.transpose(tp[:], h_t[:, c * P : (c + 1) * P], id_t[:])
                    nc.vector.tensor_copy(hT[:, c, st * P : (st + 1) * P], tp[:])
                    if st % 2 == 0:
                        nc.scalar.activation(
                            out=hto[:, c, (st // 2) * P : (st // 2 + 1) * P],
                            in_=tp[:],
                            func=mybir.ActivationFunctionType.Copy,
                            scale=1.0,
                        )
                    if st < 2:
                        nc.vector.tensor_copy(
                            hT_bf[:, c, st * P : (st + 1) * P], tp[:]
                        )
                accs = [psum("a") for _ in range(2)]
                for k in range(NCT // 2):
                    for hh in range(2):
                        nc.tensor.matmul(
                            accs[hh][:],
                            hT[:, 2 * k : 2 * k + 2, st * P : (st + 1) * P],
                            wv_t[:, 2 * k : 2 * k + 2, hh * 512 : (hh + 1) * 512],
                            start=(k == 0),
                            stop=(k == NCT // 2 - 1),
                            perf_mode=DR,
                        )
                for hh in range(2):
                    nc.vector.tensor_add(
                        out=v_sb[:, st, hh * 512 : (hh + 1) * 512],
                        in0=accs[hh][:],
                        in1=vb_b[:, hh * 512 : (hh + 1) * 512],
                    )
            # precise bf16 v for key blocks 0-1
            for blk in range(2):
                accs = [psum("a") for _ in range(2)]
                for k in range(NCT):
                    for hh in range(2):
                        nc.tensor.matmul(
                            accs[hh][:],
                            hT_bf[:, k, blk * P : (blk + 1) * P],
                            wv16_t[:, k, hh * 512 : (hh + 1) * 512],
                            start=(k == 0),
                            stop=(k == NCT - 1),
                        )
                for hh in range(2):
                    nc.vector.tensor_add(
                        out=v_bf[:, blk, hh * 512 : (hh + 1) * 512],
                        in0=accs[hh][:],
                        in1=vb_b[:, hh * 512 : (hh + 1) * 512],
                    )
            # q^T (own tokens, from hto)
            for m in range(NH):
                accs = [psum("a") for _ in range(2)]
                for k in range(NCT // 2):
                    for g in range(2):
                        nc.tensor.matmul(
                            accs[g][:],
                            wq_t[:, 2 * k : 2 * k + 2, m * P : (m + 1) * P],
                            hto[:, 2 * k : 2 * k + 2, g * 512 : (g + 1) * 512],
                            start=(k == 0),
                            stop=(k == NCT // 2 - 1),
                            perf_mode=DR,
                        )
                for g in range(2):
                    nc.vector.tensor_scalar_add(
                        out=qTo[:, m, g * 512 : (g + 1) * 512],
                        in0=accs[g][:],
                        scalar1=qb_t[:, m : m + 1],
                    )
                nc.vector.tensor_scalar_add(
                    out=qTb[:, m, :],
                    in0=accs[0][:, 0:P],
                    scalar1=qb_t[:, m : m + 1],
                )
            # k^T (all keys)
            for m in range(NH):
                accs = [psum("c", bufs=4) for _ in range(4)]
                for k in range(NCT // 2):
                    for ch in range(4):
                        nc.tensor.matmul(
                            accs[ch][:],
                            wk_t[:, 2 * k : 2 * k + 2, m * P : (m + 1) * P],
                            hT[:, 2 * k : 2 * k + 2, ch * 512 : (ch + 1) * 512],
                            start=(k == 0),
                            stop=(k == NCT // 2 - 1),
                            perf_mode=DR,
                        )
                for ch in range(4):
                    nc.vector.tensor_scalar_add(
                        out=kT[:, m, ch * 512 : (ch + 1) * 512],
                        in0=accs[ch][:],
                        scalar1=kb_t[:, m : m + 1],
                    )
                nc.vector.tensor_scalar_add(
                    out=kTb[:, m, :],
                    in0=accs[0][:],
                    scalar1=kb_t[:, m : m + 1],
                )

        # ============== Phase C: attention (software-pipelined) ============
        with ExitStack() as sC:
            att = sC.enter_context(tc.tile_pool(name="att", bufs=2))
            wtl = sC.enter_context(tc.tile_pool(name="wtl", bufs=16))
            state = {}

            def emit_scores(lp):
                nch = NCHUNKS[lp]
                mask_t = att.tile([P, 512], F32, tag="mask")
                nc.sync.dma_start(mask_t[:], masks[lp])
                p_t = att.tile([P, T], BF16, tag="pt", bufs=3)
                den = att.tile([P, 4], F32, tag="den")
                scs = [psum("c", bufs=4) for _ in range(nch)]
                if lp == 0:
                    # precise bf16 scores for the head block
                    for m in range(NH):
                        nc.tensor.matmul(
                            scs[0][:],
                            qTb[:, m, :],
                            kTb[:, m, :],
                            start=(m == 0),
                            stop=(m == NH - 1),
                        )
                else:
                    for m in range(NH // 2):
                        for j in range(nch):
                            nc.tensor.matmul(
                                scs[j][:],
                                qTo[:, 2 * m : 2 * m + 2, lp * P : (lp + 1) * P],
                                kT[:, 2 * m : 2 * m + 2, j * 512 : (j + 1) * 512],
                                start=(m == 0),
                                stop=(m == NH // 2 - 1),
                                perf_mode=DR,
                            )
                for j in range(nch):
                    if j == nch - 1:
                        nc.vector.tensor_add(
                            out=scs[j][:], in0=scs[j][:], in1=mask_t[:]
                        )
                    nc.scalar.activation(
                        out=p_t[:, j * 512 : (j + 1) * 512],
                        in_=scs[j][:],
                        func=mybir.ActivationFunctionType.Exp,
                        scale=float(SCALE),
                        accum_out=den[:, j : j + 1],
                    )
                state[lp] = (p_t, den)

            def emit_tail(lp):
                nch = NCHUNKS[lp]
                nst = 4 * nch
                p_t, den = state.pop(lp)
                dsum = att.tile([P, 1], F32, tag="dsum")
                nc.vector.reduce_sum(
                    out=dsum[:], in_=den[:, :nch], axis=mybir.AxisListType.X
                )
                nc.vector.reciprocal(out=dsum[:], in_=dsum[:])
                sa0 = psum("a")
                sa1 = psum("a")
                if lp == 0:
                    # precise bf16 p@v over key blocks 0-1 (rest masked to zero)
                    wtb = []
                    for st in range(2):
                        tp = psum("b", (P, P), BF16)
                        nc.tensor.transpose(
                            tp[:], p_t[:, st * P : (st + 1) * P], id_t[:]
                        )
                        wt = wtl.tile([P, P], BF16, tag="wtb")
                        nc.vector.tensor_copy(wt[:], tp[:])
                        wtb.append(wt)
                    for st in range(2):
                        nc.tensor.matmul(
                            sa0[:], wtb[st][:], v_bf[:, st, 0:512],
                            start=(st == 0), stop=(st == 1),
                        )
                        nc.tensor.matmul(
                            sa1[:], wtb[st][:], v_bf[:, st, 512:1024],
                            start=(st == 0), stop=(st == 1),
                        )
                else:
                    wtp = []
                    for sp in range(nst // 2):
                        wt = wtl.tile([P, 2, P], E4, tag="wt")
                        for u in range(2):
                            st = 2 * sp + u
                            tp = psum("b", (P, P), BF16)
                            nc.tensor.transpose(
                                tp[:], p_t[:, st * P : (st + 1) * P], id_t[:]
                            )
                            nc.vector.tensor_copy(wt[:, u], tp[:])
                        wtp.append(wt)
                    for sp in range(nst // 2):
                        nc.tensor.matmul(
                            sa0[:], wtp[sp][:], v_sb[:, 2 * sp : 2 * sp + 2, 0:512],
                            start=(sp == 0), stop=(sp == nst // 2 - 1),
                            perf_mode=DR,
                        )
                        nc.tensor.matmul(
                            sa1[:], wtp[sp][:],
                            v_sb[:, 2 * sp : 2 * sp + 2, 512:1024],
                            start=(sp == 0), stop=(sp == nst // 2 - 1),
                            perf_mode=DR,
                        )
                x_t = att.tile([P, C], F32, tag="xo")
                nc.sync.dma_start(x_t[:], x[2 * lp * P : (2 * lp + 1) * P, :])
                x2_t = att.tile([P, C], F32, tag="x2")
                nc.vector.tensor_scalar_mul(
                    out=x2_t[:, 0:512], in0=sa0[:], scalar1=dsum[:]
                )
                nc.vector.tensor_scalar_mul(
                    out=x2_t[:, 512:1024], in0=sa1[:], scalar1=dsum[:]
                )
                nc.vector.tensor_add(out=x2_t[:], in0=x2_t[:], in1=x_t[:])
                nc.sync.dma_start(x2_d[lp * P : (lp + 1) * P, :], x2_t[:])

            emit_scores(0)
            emit_scores(1)
            for lp in range(2, NLOC):
                emit_scores(lp)
                emit_tail(lp - 2)
            emit_tail(NLOC - 2)
            emit_tail(NLOC - 1)

        # ============== Phase C2: LN2 + h2^T ===============================
        for lt in range(NLOC):
            x2_t = ln.tile([P, C], F32, tag="xt")
            nc.sync.dma_start(x2_t[:], x2_d[lt * P : (lt + 1) * P, :])
            h2_t = ln.tile([P, C], BF16, tag="ht")
            _layernorm(nc, ln, x2_t, h2_t, eps_t)
            for c in range(NCT):
                tp = psum("b", (P, P), BF16)
                nc.tensor.transpose(tp[:], h2_t[:, c * P : (c + 1) * P], id_t[:])
                nc.vector.tensor_copy(h2T[:, c, lt * P : (lt + 1) * P], tp[:])

        # ================= Phase D: FFN (fp8 DoubleRow) ====================
        with ExitStack() as sD:
            big_d = sD.enter_context(tc.tile_pool(name="bigd", bufs=1))
            ffw = sD.enter_context(tc.tile_pool(name="ffw", bufs=3))
            aT = [
                big_d.tile([P, NF, 512], E4, name=f"aT{i}") for i in range(2)
            ]

            def emit_aT(tch):
                # a^T half = relu(W1^T h2^T + b1) for 512 local tokens
                for ft in range(NF):
                    w1_t = ffw.tile(
                        [P, NCT, P], E4, tag="w1t", bufs=4, name=f"w1t{tch}_{ft}"
                    )
                    nc.sync.dma_start(w1_t[:], w1[ft])
                    acc = psum("a")
                    for k in range(NCT // 2):
                        nc.tensor.matmul(
                            acc[:],
                            w1_t[:, 2 * k : 2 * k + 2],
                            h2T[:, 2 * k : 2 * k + 2, tch * 512 : (tch + 1) * 512],
                            start=(k == 0),
                            stop=(k == NCT // 2 - 1),
                            perf_mode=DR,
                        )
                    nc.scalar.activation(
                        out=aT[tch][:, ft, :],
                        in_=acc[:],
                        func=mybir.ActivationFunctionType.Relu,
                        bias=b1_t[:, ft : ft + 1],
                        scale=1.0,
                    )

            def mk_grp():
                return [
                    [
                        psum(
                            ("a" if tb < 1 else "b" if tb < 2 else "c"),
                            bufs=(2 if tb < 2 else 4),
                        )
                        for cc in range(2)
                    ]
                    for tb in range(4)
                ]

            def ff_pass(tbh, dig, grp):
                for ft in range(NF // 2):
                    w2_t = ffw.tile(
                        [P, 2, C], E4, tag="w2t", bufs=4,
                        name=f"w2t{tbh}_{dig}_{ft}",
                    )
                    nc.sync.dma_start(
                        w2_t[:], w2_r[:, dig, 2 * ft : 2 * ft + 2, :]
                    )
                    for tb in range(4):
                        for cc in range(2):
                            nc.tensor.matmul(
                                grp[tb][cc][:],
                                aT[tbh][:, 2 * ft : 2 * ft + 2,
                                        tb * P : (tb + 1) * P],
                                w2_t[:, :, cc * 512 : (cc + 1) * 512],
                                start=(ft == 0),
                                stop=(ft == NF // 2 - 1),
                                perf_mode=DR,
                            )

            def emit_ff_hi(tbh):
                # hi-digit pass, evacuated (+b2) into o_t
                grp = mk_grp()
                ff_pass(tbh, 0, grp)
                o_ts = []
                for tb in range(4):
                    o_t = ffw.tile(
                        [P, C], F32, tag="ot", bufs=5, name=f"ot{tbh}_{tb}"
                    )
                    for cc in range(2):
                        nc.vector.tensor_add(
                            out=o_t[:, cc * 512 : (cc + 1) * 512],
                            in0=grp[tb][cc][:],
                            in1=b2_b[:, cc * 512 : (cc + 1) * 512],
                        )
                    o_ts.append(o_t)
                return o_ts

            def emit_ff_lo(tbh, o_ts):
                # lo-digit pass (weights pre-scaled x256), combined at 1/256
                grp = mk_grp()
                ff_pass(tbh, 1, grp)
                for tb in range(4):
                    lt = tbh * 4 + tb
                    x2_t = ffw.tile([P, C], F32, tag="x2r", name=f"x2r{tbh}_{tb}")
                    nc.sync.dma_start(x2_t[:], x2_d[lt * P : (lt + 1) * P, :])
                    o_t = o_ts[tb]
                    lo_t = ffw.tile([P, C], F32, tag="lot", name=f"lot{tbh}_{tb}")
                    for cc in range(2):
                        nc.scalar.activation(
                            out=lo_t[:, cc * 512 : (cc + 1) * 512],
                            in_=grp[tb][cc][:],
                            func=mybir.ActivationFunctionType.Copy,
                            scale=float(1.0 / 256.0),
                        )
                    nc.vector.tensor_add(out=o_t[:], in0=o_t[:], in1=lo_t[:])
                    nc.vector.tensor_add(out=o_t[:], in0=o_t[:], in1=x2_t[:])
                    nc.sync.dma_start(out[lt * P : (lt + 1) * P, :], o_t[:])

            # aT(1) overlaps the hi(0) PSUM evacuation; lo(0) still reads
            # aT[0], so the two halves use separate aT buffers.
            emit_aT(0)
            o0 = emit_ff_hi(0)
            emit_aT(1)
            emit_ff_lo(0, o0)
            o1 = emit_ff_hi(1)
            emit_ff_lo(1, o1)

    _split_multi_waits(nc)
    return nc


_NC_CACHE = None


def _get_nc():
    global _NC_CACHE
    if _NC_CACHE is None:
        _NC_CACHE = build_nc()
    return _NC_CACHE


def _prep_host(inputs):
    """Fold LN gains/biases into weights; build per-core input maps."""
    x = np.asarray(inputs["x"], dtype=np.float32)
    Wk = np.asarray(inputs["Wk"], dtype=np.float32)
    Wq = np.asarray(inputs["Wq"], dtype=np.float32)
    Wv = np.asarray(inputs["Wv"], dtype=np.float32)
    W1 = np.asarray(inputs["W1"], dtype=np.float32)
    b1 = np.asarray(inputs["b1"], dtype=np.float32)
    W2 = np.asarray(inputs["W2"], dtype=np.float32)
    b2 = np.asarray(inputs["b2"], dtype=np.float32)
    g1 = np.asarray(inputs["g1"], dtype=np.float32)
    be1 = np.asarray(inputs["be1"], dtype=np.float32)
    g2 = np.asarray(inputs["g2"], dtype=np.float32)
    be2 = np.asarray(inputs["be2"], dtype=np.float32)

    f8 = ml_dtypes.float8_e4m3
    bf = ml_dtypes.bfloat16
    wq_f = np.ascontiguousarray((g1[:, None] * Wq).astype(f8))
    wk_f = np.ascontiguousarray((g1[:, None] * Wk).astype(f8))
    wv_full = g1[:, None] * Wv
    wv_f = np.ascontiguousarray(wv_full.astype(f8))
    wv16_f = np.ascontiguousarray(wv_full.astype(bf))
    qb = be1 @ Wq
    kb = be1 @ Wk
    vb = be1 @ Wv
    w1_full = (g2[:, None] * W1).astype(f8)
    w1_f = np.ascontiguousarray(
        w1_full.reshape(NCT, P, NF, P).transpose(2, 1, 0, 3)
    )
    w2_hi = W2.astype(f8)
    w2_lo = (256.0 * (W2 - w2_hi.astype(np.float32))).astype(f8)
    w2_f8 = np.ascontiguousarray(np.stack([w2_hi, w2_lo], axis=0))
    b1_f = b1 + be2 @ W1

    ident = np.eye(P, dtype=ml_dtypes.bfloat16)

    # per-half masks against the PERMUTED key order: for own block lp (global
    # g), only the last processed 512-chunk needs masking; each 128-block in
    # it is fully allowed (gb < g), fully masked (gb > g), or diagonal.
    rr = np.arange(P)[:, None]
    sub_diag = np.where(np.arange(P)[None, :] <= rr, 0.0, NEG).astype(np.float32)
    masks_h = []
    for half in range(2):
        perm = PERMS[half]
        mk = np.empty((NLOC, P, 512), dtype=np.float32)
        for m_i, g in enumerate(L_HALF[half]):
            nch = NCHUNKS[m_i]
            for pos in range(4 * (nch - 1)):
                assert perm[pos] < g, (half, m_i, pos)
            for pos in range(4 * nch, NT):
                assert perm[pos] > g, (half, m_i, pos)
            base = 4 * (nch - 1)
            for jb in range(4):
                gb = perm[base + jb]
                if gb < g:
                    mk[m_i, :, jb * P : (jb + 1) * P] = 0.0
                elif gb > g:
                    mk[m_i, :, jb * P : (jb + 1) * P] = NEG
                else:
                    mk[m_i, :, jb * P : (jb + 1) * P] = sub_diag
        masks_h.append(mk)

    shared = {
        "wq": wq_f, "wk": wk_f, "wv": wv_f, "wv16": wv16_f,
        "w1": w1_f, "w2": w2_f8,
        "qb": qb, "kb": kb, "vb": vb, "b1": b1_f, "b2": b2,
        "ident": ident,
    }
    in_maps = []
    for core in range(8):
        b, half = core // 2, core % 2
        rows = np.concatenate(
            [np.arange(i * P, (i + 1) * P) for i in PERMS[half]]
        )
        xp = np.ascontiguousarray(x[b][rows])
        m = dict(shared)
        m["x"] = xp
        m["x_bf"] = xp.astype(bf)
        m["masks"] = masks_h[half]
        in_maps.append(m)
    return in_maps


def _scatter_out(results):
    out = np.empty((B, T, C), dtype=np.float32)
    for core in range(8):
        b, half = core // 2, core % 2
        L = L_HALF[half]
        o = results[core]["out"]
        for ppos, i in enumerate(L):
            out[b, i * P : (i + 1) * P, :] = o[ppos * P : (ppos + 1) * P, :]
    return out


def run(inputs, trace=False, **kw):
    nc = _get_nc()
    in_maps = _prep_host(inputs)
    res = run_bass_kernel_spmd(
        nc, in_maps, core_ids=list(range(8)), trace=trace, **kw
    )
    return _scatter_out(res.results), res


def kernel(**inputs) -> np.ndarray:
    out, _ = run(inputs, trace=False)
    return out


# revision 35
# speedup vs baseline: 1.1985x; 1.0169x over previous
"""Trainium2 Bass kernel for nn_Block (dense transformer block).

  out = x + FFN(LN2(x + Attn(LN1(x))))   with causal single-head attention,
  B=4, T=2048, C=H=1024, FF=4096, fp32 reference.

Distribution: 8 NeuronCores = (batch b in 0..3) x (query-half in 0..1).
Each core handles one batch element's keys/values and HALF its query rows
(causally balanced interleaved block split), plus LN2+FFN+residual for those
rows.  No collectives; the per-core programs are IDENTICAL (SPMD) - all
per-core variation is input data.

All matmul OPERANDS are fp8e4m3 driven in DoubleRow perf mode (2 fp8
weights per PE cell, K=256 contraction per matmul); every accumulation is
f32 in PSUM, and LN stats / softmax denominators / residual adds are f32.
LN gains/biases are folded into the weight matrices host-side.
"""

import sys
import types

import numpy as np

# ---------------------------------------------------------------------------
# antenv.axon_hooks shim: the image's antenv lacks this module and
# run_bass_kernel_spmd imports it under axon when trace=True.
import antenv

if "antenv.axon_hooks" not in sys.modules:
    _mod = types.ModuleType("antenv.axon_hooks")
    _mod._hook = None
    _mod.set_axon_ntff_profile_hook = lambda h: setattr(_mod, "_hook", h)
    _mod.get_axon_ntff_profile_hook = lambda: _mod._hook
    sys.modules["antenv.axon_hooks"] = _mod
    antenv.axon_hooks = _mod

import ml_dtypes

import concourse.bass as bass
import concourse.mybir as mybir
import concourse.tile as tile
from concourse.bass_utils import run_bass_kernel_spmd

F32 = mybir.dt.float32
BF16 = mybir.dt.bfloat16
E4 = mybir.dt.float8e4
DR = mybir.MatmulPerfMode.DoubleRow

B, T, C = 4, 2048, 1024
H, FF = 1024, 4096
P = 128
NT = T // P  # 16 token blocks per batch element
NCT = C // P  # 8 contraction tiles
NH = H // P  # 8 head-dim tiles
NF = FF // P  # 32 ff tiles
TOWN = T // 2  # own tokens per core (1024)
NLOC = TOWN // P  # 8 own blocks
EPS = 1e-5
SCALE = 1.0 / np.sqrt(np.float32(C))  # 1/32
NEG = -1.0e30

# Causally balanced query-block assignment (sum of chunk counts = 20 each).
L_HALF = [
    [0, 2, 4, 6, 9, 11, 13, 15],
    [1, 3, 5, 7, 8, 10, 12, 14],
]
# ceil((i+1)/4) for i in L_HALF[h] - same sequence for both halves.
NCHUNKS = [1, 1, 2, 2, 3, 3, 4, 4]
# Per-core BLOCK PERMUTATION of x: own blocks sit at even positions, the
# other half's at odd positions.  The program then addresses own tokens at
# fixed (core-independent) offsets; all per-core variation stays in data.
PERMS = [
    [L_HALF[h][m // 2] if m % 2 == 0 else L_HALF[1 - h][m // 2] for m in range(NT)]
    for h in range(2)
]


def _split_multi_waits(nc):
    """walrus here accepts at most ONE sync-wait per instruction; hoist
    extras onto injected same-engine NoOps."""
    for fn in nc.m.functions:
        for blk in fn.blocks:
            new_insts = []
            changed = False
            for inst in blk.instructions:
                si = getattr(inst, "sync_info", None)
                ow = list(si.on_wait) if si is not None and si.on_wait else []
                if len(ow) > 1:
                    for i, cond in enumerate(ow[:-1]):
                        new_insts.append(
                            mybir.InstNoOp(
                                name=f"{inst.name}-wn{i}",
                                engine=inst.engine,
                                ins=[],
                                outs=[],
                                sync_info=mybir.SyncInfo(
                                    on_wait=[cond], on_update=[]
                                ),
                            )
                        )
                    inst.sync_info = mybir.SyncInfo(
                        on_wait=[ow[-1]], on_update=list(si.on_update or [])
                    )
                    changed = True
                new_insts.append(inst)
            if changed:
                blk.instructions = new_insts


def _layernorm(nc, pool, x_t, h_t, eps_t):
    """h_t = (x_t - mean) * rsqrt(var + eps), stats along the free dim."""
    xg = x_t[:].rearrange("p (s f) -> p s f", f=512)
    stats = pool.tile([P, 2, nc.vector.BN_STATS_DIM], F32, tag="ln_stats")
    for sg in range(2):
        nc.vector.bn_stats(out=stats[:, sg], in_=xg[:, sg])
    mv = pool.tile([P, nc.vector.BN_AGGR_DIM], F32, tag="ln_mv")
    nc.vector.bn_aggr(out=mv[:], in_=stats[:])
    rstd = pool.tile([P, 1], F32, tag="ln_rstd")
    nc.scalar.activation(
        out=rstd[:],
        in_=mv[:, 1:2],
        func=mybir.ActivationFunctionType.Sqrt,
        bias=eps_t[:],
        scale=1.0,
    )
    nc.vector.reciprocal(out=rstd[:], in_=rstd[:])
    nc.vector.tensor_scalar(
        out=h_t[:],
        in0=x_t[:],
        scalar1=mv[:, 0:1],
        scalar2=rstd[:],
        op0=mybir.AluOpType.subtract,
        op1=mybir.AluOpType.mult,
    )


def build_nc():
    from contextlib import ExitStack

    nc = bass.Bass()

    x = nc.declare_dram_parameter("x", [T, C], F32, isOutput=False)
    x_bf = nc.declare_dram_parameter("x_bf", [T, C], BF16, isOutput=False)
    wq = nc.declare_dram_parameter("wq", [C, H], E4, isOutput=False)
    wk = nc.declare_dram_parameter("wk", [C, H], E4, isOutput=False)
    wv = nc.declare_dram_parameter("wv", [C, H], E4, isOutput=False)
    wv16 = nc.declare_dram_parameter("wv16", [C, H], BF16, isOutput=False)
    w1 = nc.declare_dram_parameter("w1", [NF, P, NCT, P], E4, isOutput=False)
    # w2[0] = e4m3(W2), w2[1] = e4m3(256*(W2 - w2[0])) - two-digit fp8
    w2 = nc.declare_dram_parameter("w2", [2, FF, C], E4, isOutput=False)
    qb = nc.declare_dram_parameter("qb", [H], F32, isOutput=False)
    kb = nc.declare_dram_parameter("kb", [H], F32, isOutput=False)
    vb = nc.declare_dram_parameter("vb", [H], F32, isOutput=False)
    b1 = nc.declare_dram_parameter("b1", [FF], F32, isOutput=False)
    b2 = nc.declare_dram_parameter("b2", [C], F32, isOutput=False)
    ident = nc.declare_dram_parameter("ident", [P, P], BF16, isOutput=False)
    masks = nc.declare_dram_parameter("masks", [NLOC, P, 512], F32, isOutput=False)
    out = nc.declare_dram_parameter("out", [TOWN, C], F32, isOutput=True)

    x2_d = nc.dram_tensor("x2_d", [TOWN, C], F32)

    wq_r = wq.rearrange("(ko p) h -> p ko h", p=P)
    wk_r = wk.rearrange("(ko p) h -> p ko h", p=P)
    wv_r = wv.rearrange("(ko p) h -> p ko h", p=P)
    wv16_r = wv16.rearrange("(ko p) h -> p ko h", p=P)
    w2_r = w2.rearrange("t (ko p) c -> p t ko c", p=P)

    with tile.TileContext(nc) as tc, ExitStack() as top:
        cn = top.enter_context(tc.tile_pool(name="cn", bufs=1))
        ps = top.enter_context(tc.tile_pool(name="ps", bufs=1, space="PSUM"))
        ln = top.enter_context(tc.tile_pool(name="ln", bufs=4))
        # big resident tensors spanning several phases
        res = top.enter_context(tc.tile_pool(name="res", bufs=1))

        # critical-path-first: the very first LN tile and the transpose
        # identity go ahead of the bulk constant loads.
        x0_t = ln.tile([P, C], BF16, tag="xt", name="x0t")
        nc.sync.dma_start(x0_t[:], x_bf[0:P, :])
        id_t = cn.tile([P, P], BF16)
        nc.sync.dma_start(id_t[:], ident[:])
        # ---- constants
        qb_t = cn.tile([P, NH], F32)
        nc.sync.dma_start(qb_t[:], qb.rearrange("(m p) -> p m", p=P))
        kb_t = cn.tile([P, NH], F32)
        nc.sync.dma_start(kb_t[:], kb.rearrange("(m p) -> p m", p=P))
        b1_t = cn.tile([P, NF], F32)
        nc.sync.dma_start(b1_t[:], b1.rearrange("(m p) -> p m", p=P))
        vb_b = cn.tile([P, H], F32)
        nc.sync.dma_start(vb_b[:], vb[None, :].partition_broadcast(P))
        b2_b = cn.tile([P, C], F32)
        nc.sync.dma_start(b2_b[:], b2[None, :].partition_broadcast(P))
        eps_t = cn.tile([P, 1], F32)
        nc.vector.memset(eps_t, EPS)

        _ctr = [0]

        def psum(tag, shape=(P, 512), dt=F32, bufs=2):
            _ctr[0] += 1
            return ps.tile(list(shape), dt, tag=tag, bufs=bufs, name=f"ps{_ctr[0]}")

        qTo = res.tile([P, NH, TOWN], E4)   # q^T own tokens (local order)
        kT = res.tile([P, NH, T], E4)       # k^T all keys
        v_sb = res.tile([P, NT, H], E4)     # v token-major, all keys
        h2T = res.tile([P, NCT, TOWN], E4)  # LN2 output transposed
        # precise (bf16) head-block path: first own block attends few keys, so
        # fp8 noise passes straight through - keep that slice in bf16.
        qTb = res.tile([P, NH, P], BF16)    # q^T for local block 0
        kTb = res.tile([P, NH, 512], BF16)  # k^T for key chunk 0
        v_bf = res.tile([P, 2, H], BF16)    # v for key blocks 0-1 (bf16 matmul)

        # ===== Phase B: LN1 over permuted blocks -> hT/hto; v; q^T; k^T ====
        # x arrives block-PERMUTED per core (own blocks at even positions),
        # so one LN pass feeds hT (all keys) and hto (own tokens, gathered by
        # the scalar engine) - no duplicated LN/transposes, and the v matmuls
        # give the PE work from the first tile on.
        with ExitStack() as sB:
            big_b = sB.enter_context(tc.tile_pool(name="bigb", bufs=1))
            wvp = sB.enter_context(tc.tile_pool(name="wvp", bufs=1))
            hT = big_b.tile([P, NCT, T], E4)
            hto = big_b.tile([P, NCT, TOWN], E4)
            hT_bf = big_b.tile([P, NCT, 256], BF16)
            wv_t = wvp.tile([P, NCT, H], E4)
            nc.sync.dma_start(wv_t[:], wv_r[:])
            wq_t = wvp.tile([P, NCT, H], E4)
            nc.sync.dma_start(wq_t[:], wq_r[:])
            wk_t = wvp.tile([P, NCT, H], E4)
            nc.sync.dma_start(wk_t[:], wk_r[:])
            wv16_t = wvp.tile([P, NCT, H], BF16)
            nc.sync.dma_start(wv16_t[:], wv16_r[:])
            # LN per permuted token tile; v row-block right after its tile
            for st in range(NT):
                if st == 0:
                    x_t = x0_t
                else:
                    x_t = ln.tile([P, C], BF16, tag="xb")
                    nc.sync.dma_start(x_t[:], x_bf[st * P : (st + 1) * P, :])
                h_t = ln.tile([P, C], BF16, tag="ht")
                _layernorm(nc, ln, x_t, h_t, eps_t)
                for c in range(NCT):
                    tp = psum("b", (P, P), BF16)
                    nc.tensor.transpose(tp[:], h_t[:, c * P : (c + 1) * P], id_t[:])
                    nc.vector.tensor_copy(hT[:, c, st * P : (st + 1) * P], tp[:])
                    if st % 2 == 0:
                        nc.scalar.activation(
                            out=hto[:, c, (st // 2) * P : (st // 2 + 1) * P],
                            in_=tp[:],
                            func=mybir.ActivationFunctionType.Copy,
                            scale=1.0,
                        )
                    if st < 2:
                        nc.vector.tensor_copy(
                            hT_bf[:, c, st * P : (st + 1) * P], tp[:]
                        )
                accs = [psum("a") for _ in range(2)]
                for k in range(NCT // 2):
                    for hh in range(2):
                        nc.tensor.matmul(
                            accs[hh][:],
                            hT[:, 2 * k : 2 * k + 2, st * P : (st + 1) * P],
                            wv_t[:, 2 * k : 2 * k + 2, hh * 512 : (hh + 1) * 512],
                            start=(k == 0),
                            stop=(k == NCT // 2 - 1),
                            perf_mode=DR,
                        )
                for hh in range(2):
                    nc.vector.tensor_add(
                        out=v_sb[:, st, hh * 512 : (hh + 1) * 512],
                        in0=accs[hh][:],
                        in1=vb_b[:, hh * 512 : (hh + 1) * 512],
                    )
                if st == 1:
                    # precise bf16 v for key blocks 0-1 - emitted here so the
                    # PE has work while the LN stream (DVE-bound) warms up
                    for blk in range(2):
                        accs = [psum("a") for _ in range(2)]
                        for k in range(NCT):
                            for hh in range(2):
                                nc.tensor.matmul(
                                    accs[hh][:],
                                    hT_bf[:, k, blk * P : (blk + 1) * P],
                                    wv16_t[:, k, hh * 512 : (hh + 1) * 512],
                                    start=(k == 0),
                                    stop=(k == NCT - 1),
                                )
                        for hh in range(2):
                            nc.vector.tensor_add(
                                out=v_bf[:, blk, hh * 512 : (hh + 1) * 512],
                                in0=accs[hh][:],
                                in1=vb_b[:, hh * 512 : (hh + 1) * 512],
                            )
            # q^T (own tokens, from hto)
            for m in range(NH):
                accs = [psum("a") for _ in range(2)]
                for k in range(NCT // 2):
                    for g in range(2):
                        nc.tensor.matmul(
                            accs[g][:],
                            wq_t[:, 2 * k : 2 * k + 2, m * P : (m + 1) * P],
                            hto[:, 2 * k : 2 * k + 2, g * 512 : (g + 1) * 512],
                            start=(k == 0),
                            stop=(k == NCT // 2 - 1),
                            perf_mode=DR,
                        )
                for g in range(2):
                    nc.vector.tensor_scalar_add(
                        out=qTo[:, m, g * 512 : (g + 1) * 512],
                        in0=accs[g][:],
                        scalar1=qb_t[:, m : m + 1],
                    )
                nc.vector.tensor_scalar_add(
                    out=qTb[:, m, :],
                    in0=accs[0][:, 0:P],
                    scalar1=qb_t[:, m : m + 1],
                )
            # k^T (all keys)
            for m in range(NH):
                accs = [psum("c", bufs=4) for _ in range(4)]
                for k in range(NCT // 2):
                    for ch in range(4):
                        nc.tensor.matmul(
                            accs[ch][:],
                            wk_t[:, 2 * k : 2 * k + 2, m * P : (m + 1) * P],
                            hT[:, 2 * k : 2 * k + 2, ch * 512 : (ch + 1) * 512],
                            start=(k == 0),
                            stop=(k == NCT // 2 - 1),
                            perf_mode=DR,
                        )
                for ch in range(4):
                    nc.vector.tensor_scalar_add(
                        out=kT[:, m, ch * 512 : (ch + 1) * 512],
                        in0=accs[ch][:],
                        scalar1=kb_t[:, m : m + 1],
                    )
                nc.vector.tensor_scalar_add(
                    out=kTb[:, m, :],
                    in0=accs[0][:],
                    scalar1=kb_t[:, m : m + 1],
                )

        # ============== Phase C: attention (software-pipelined) ============
        with ExitStack() as sC:
            att = sC.enter_context(tc.tile_pool(name="att", bufs=2))
            wtl = sC.enter_context(tc.tile_pool(name="wtl", bufs=16))
            state = {}

            def emit_scores(lp):
                nch = NCHUNKS[lp]
                mask_t = att.tile([P, 512], F32, tag="mask")
                nc.sync.dma_start(mask_t[:], masks[lp])
                p_t = att.tile([P, T], BF16, tag="pt", bufs=3)
                den = att.tile([P, 4], F32, tag="den")
                scs = [psum("c", bufs=4) for _ in range(nch)]
                if lp == 0:
                    # precise bf16 scores for the head block
                    for m in range(NH):
                        nc.tensor.matmul(
                            scs[0][:],
                            qTb[:, m, :],
                            kTb[:, m, :],
                            start=(m == 0),
                            stop=(m == NH - 1),
                        )
                else:
                    for m in range(NH // 2):
                        for j in range(nch):
                            nc.tensor.matmul(
                                scs[j][:],
                                qTo[:, 2 * m : 2 * m + 2, lp * P : (lp + 1) * P],
                                kT[:, 2 * m : 2 * m + 2, j * 512 : (j + 1) * 512],
                                start=(m == 0),
                                stop=(m == NH // 2 - 1),
                                perf_mode=DR,
                            )
                for j in range(nch):
                    if j == nch - 1:
                        nc.vector.tensor_add(
                            out=scs[j][:], in0=scs[j][:], in1=mask_t[:]
                        )
                    nc.scalar.activation(
                        out=p_t[:, j * 512 : (j + 1) * 512],
                        in_=scs[j][:],
                        func=mybir.ActivationFunctionType.Exp,
                        scale=float(SCALE),
                        accum_out=den[:, j : j + 1],
                    )
                state[lp] = (p_t, den)

            def emit_tail(lp):
                nch = NCHUNKS[lp]
                nst = 4 * nch
                p_t, den = state.pop(lp)
                dsum = att.tile([P, 1], F32, tag="dsum")
                nc.vector.reduce_sum(
                    out=dsum[:], in_=den[:, :nch], axis=mybir.AxisListType.X
                )
                nc.vector.reciprocal(out=dsum[:], in_=dsum[:])
                sa0 = psum("a")
                sa1 = psum("a")
                if lp == 0:
                    # precise bf16 p@v over key blocks 0-1 (rest masked to zero)
                    wtb = []
                    for st in range(2):
                        tp = psum("b", (P, P), BF16)
                        nc.tensor.transpose(
                            tp[:], p_t[:, st * P : (st + 1) * P], id_t[:]
                        )
                        wt = wtl.tile([P, P], BF16, tag="wtb")
                        nc.vector.tensor_copy(wt[:], tp[:])
                        wtb.append(wt)
                    for st in range(2):
                        nc.tensor.matmul(
                            sa0[:], wtb[st][:], v_bf[:, st, 0:512],
                            start=(st == 0), stop=(st == 1),
                        )
                        nc.tensor.matmul(
                            sa1[:], wtb[st][:], v_bf[:, st, 512:1024],
                            start=(st == 0), stop=(st == 1),
                        )
                else:
                    wtp = []
                    for sp in range(nst // 2):
                        wt = wtl.tile([P, 2, P], E4, tag="wt")
                        for u in range(2):
                            st = 2 * sp + u
                            tp = psum("b", (P, P), BF16)
                            nc.tensor.transpose(
                                tp[:], p_t[:, st * P : (st + 1) * P], id_t[:]
                            )
                            nc.vector.tensor_copy(wt[:, u], tp[:])
                        wtp.append(wt)
                    for sp in range(nst // 2):
                        nc.tensor.matmul(
                            sa0[:], wtp[sp][:], v_sb[:, 2 * sp : 2 * sp + 2, 0:512],
                            start=(sp == 0), stop=(sp == nst // 2 - 1),
                            perf_mode=DR,
                        )
                        nc.tensor.matmul(
                            sa1[:], wtp[sp][:],
                            v_sb[:, 2 * sp : 2 * sp + 2, 512:1024],
                            start=(sp == 0), stop=(sp == nst // 2 - 1),
                            perf_mode=DR,
                        )
                x_t = att.tile([P, C], F32, tag="xo")
                nc.sync.dma_start(x_t[:], x[2 * lp * P : (2 * lp + 1) * P, :])
                x2_t = att.tile([P, C], F32, tag="x2")
                # sa/den scaling on the scalar engine - DVE gates the PE here
                nc.scalar.activation(
                    out=x2_t[:, 0:512], in_=sa0[:],
                    func=mybir.ActivationFunctionType.Copy, scale=dsum[:],
                )
                nc.scalar.activation(
                    out=x2_t[:, 512:1024], in_=sa1[:],
                    func=mybir.ActivationFunctionType.Copy, scale=dsum[:],
                )
                nc.vector.tensor_add(out=x2_t[:], in0=x2_t[:], in1=x_t[:])
                nc.sync.dma_start(x2_d[lp * P : (lp + 1) * P, :], x2_t[:])

            emit_scores(0)
            emit_scores(1)
            for lp in range(2, NLOC):
                emit_scores(lp)
                emit_tail(lp - 2)
            emit_tail(NLOC - 2)
            emit_tail(NLOC - 1)

        # ============== Phase C2: LN2 + h2^T ===============================
        def emit_ln2(lt):
            x2_t = ln.tile([P, C], F32, tag="xt")
            nc.sync.dma_start(x2_t[:], x2_d[lt * P : (lt + 1) * P, :])
            h2_t = ln.tile([P, C], BF16, tag="ht")
            _layernorm(nc, ln, x2_t, h2_t, eps_t)
            for c in range(NCT):
                tp = psum("b", (P, P), BF16)
                nc.tensor.transpose(tp[:], h2_t[:, c * P : (c + 1) * P], id_t[:])
                nc.scalar.activation(
                    out=h2T[:, c, lt * P : (lt + 1) * P],
                    in_=tp[:],
                    func=mybir.ActivationFunctionType.Copy,
                    scale=1.0,
                )

        for lt in range(4):
            emit_ln2(lt)

        # ================= Phase D: FFN (fp8 DoubleRow) ====================
        with ExitStack() as sD:
            big_d = sD.enter_context(tc.tile_pool(name="bigd", bufs=1))
            ffw = sD.enter_context(tc.tile_pool(name="ffw", bufs=3))
            aT = [
                big_d.tile([P, NF, 512], E4, name=f"aT{i}") for i in range(2)
            ]

            def emit_aT(tch):
                # a^T half = relu(W1^T h2^T + b1) for 512 local tokens
                for ft in range(NF):
                    w1_t = ffw.tile(
                        [P, NCT, P], E4, tag="w1t", bufs=4, name=f"w1t{tch}_{ft}"
                    )
                    nc.sync.dma_start(w1_t[:], w1[ft])
                    acc = psum("a")
                    for k in range(NCT // 2):
                        nc.tensor.matmul(
                            acc[:],
                            w1_t[:, 2 * k : 2 * k + 2],
                            h2T[:, 2 * k : 2 * k + 2, tch * 512 : (tch + 1) * 512],
                            start=(k == 0),
                            stop=(k == NCT // 2 - 1),
                            perf_mode=DR,
                        )
                    nc.scalar.activation(
                        out=aT[tch][:, ft, :],
                        in_=acc[:],
                        func=mybir.ActivationFunctionType.Relu,
                        bias=b1_t[:, ft : ft + 1],
                        scale=1.0,
                    )

            def mk_grp():
                return [
                    [
                        psum(
                            ("a" if tb < 1 else "b" if tb < 2 else "c"),
                            bufs=(2 if tb < 2 else 4),
                        )
                        for cc in range(2)
                    ]
                    for tb in range(4)
                ]

            def ff_pass(tbh, dig, grp):
                for ft in range(NF // 2):
                    w2_t = ffw.tile(
                        [P, 2, C], E4, tag="w2t", bufs=4,
                        name=f"w2t{tbh}_{dig}_{ft}",
                    )
                    nc.sync.dma_start(
                        w2_t[:], w2_r[:, dig, 2 * ft : 2 * ft + 2, :]
                    )
                    for tb in range(4):
                        for cc in range(2):
                            nc.tensor.matmul(
                                grp[tb][cc][:],
                                aT[tbh][:, 2 * ft : 2 * ft + 2,
                                        tb * P : (tb + 1) * P],
                                w2_t[:, :, cc * 512 : (cc + 1) * 512],
                                start=(ft == 0),
                                stop=(ft == NF // 2 - 1),
                                perf_mode=DR,
                            )

            def emit_ff_hi(tbh):
                # hi-digit pass, evacuated (+b2) into o_t
                grp = mk_grp()
                ff_pass(tbh, 0, grp)
                o_ts = []
                for tb in range(4):
                    o_t = ffw.tile(
                        [P, C], F32, tag="ot", bufs=5, name=f"ot{tbh}_{tb}"
                    )
                    for cc in range(2):
                        nc.vector.tensor_add(
                            out=o_t[:, cc * 512 : (cc + 1) * 512],
                            in0=grp[tb][cc][:],
                            in1=b2_b[:, cc * 512 : (cc + 1) * 512],
                        )
                    o_ts.append(o_t)
                return o_ts

            def emit_ff_lo(tbh, o_ts):
                # lo-digit pass (weights pre-scaled x256), combined at 1/256
                grp = mk_grp()
                ff_pass(tbh, 1, grp)
                for tb in range(4):
                    lt = tbh * 4 + tb
                    x2_t = ffw.tile([P, C], F32, tag="x2r", name=f"x2r{tbh}_{tb}")
                    nc.sync.dma_start(x2_t[:], x2_d[lt * P : (lt + 1) * P, :])
                    o_t = o_ts[tb]
                    lo_t = ffw.tile([P, C], F32, tag="lot", name=f"lot{tbh}_{tb}")
                    for cc in range(2):
                        nc.scalar.activation(
                            out=lo_t[:, cc * 512 : (cc + 1) * 512],
                            in_=grp[tb][cc][:],
                            func=mybir.ActivationFunctionType.Copy,
                            scale=float(1.0 / 256.0),
                        )
                    nc.vector.tensor_add(out=o_t[:], in0=o_t[:], in1=lo_t[:])
                    nc.vector.tensor_add(out=o_t[:], in0=o_t[:], in1=x2_t[:])
                    nc.sync.dma_start(out[lt * P : (lt + 1) * P, :], o_t[:])

            # aT(0) needs only LN2 of tiles 0-3; LN2 of 4-7 overlaps its
            # matmuls. aT(1) overlaps the hi(0) PSUM evacuation; lo(0) still
            # reads aT[0], so the two halves use separate aT buffers.
            emit_aT(0)
            for lt in range(4, NLOC):
                emit_ln2(lt)
            o0 = emit_ff_hi(0)
            emit_aT(1)
            emit_ff_lo(0, o0)
            o1 = emit_ff_hi(1)
            emit_ff_lo(1, o1)

    _split_multi_waits(nc)
    return nc


_NC_CACHE = None


def _get_nc():
    global _NC_CACHE
    if _NC_CACHE is None:
        _NC_CACHE = build_nc()
    return _NC_CACHE


def _prep_host(inputs):
    """Fold LN gains/biases into weights; build per-core input maps."""
    x = np.asarray(inputs["x"], dtype=np.float32)
    Wk = np.asarray(inputs["Wk"], dtype=np.float32)
    Wq = np.asarray(inputs["Wq"], dtype=np.float32)
    Wv = np.asarray(inputs["Wv"], dtype=np.float32)
    W1 = np.asarray(inputs["W1"], dtype=np.float32)
    b1 = np.asarray(inputs["b1"], dtype=np.float32)
    W2 = np.asarray(inputs["W2"], dtype=np.float32)
    b2 = np.asarray(inputs["b2"], dtype=np.float32)
    g1 = np.asarray(inputs["g1"], dtype=np.float32)
    be1 = np.asarray(inputs["be1"], dtype=np.float32)
    g2 = np.asarray(inputs["g2"], dtype=np.float32)
    be2 = np.asarray(inputs["be2"], dtype=np.float32)

    f8 = ml_dtypes.float8_e4m3
    bf = ml_dtypes.bfloat16
    wq_f = np.ascontiguousarray((g1[:, None] * Wq).astype(f8))
    wk_f = np.ascontiguousarray((g1[:, None] * Wk).astype(f8))
    wv_full = g1[:, None] * Wv
    wv_f = np.ascontiguousarray(wv_full.astype(f8))
    wv16_f = np.ascontiguousarray(wv_full.astype(bf))
    qb = be1 @ Wq
    kb = be1 @ Wk
    vb = be1 @ Wv
    w1_full = (g2[:, None] * W1).astype(f8)
    w1_f = np.ascontiguousarray(
        w1_full.reshape(NCT, P, NF, P).transpose(2, 1, 0, 3)
    )
    w2_hi = W2.astype(f8)
    w2_lo = (256.0 * (W2 - w2_hi.astype(np.float32))).astype(f8)
    w2_f8 = np.ascontiguousarray(np.stack([w2_hi, w2_lo], axis=0))
    b1_f = b1 + be2 @ W1

    ident = np.eye(P, dtype=ml_dtypes.bfloat16)

    # per-half masks against the PERMUTED key order: for own block lp (global
    # g), only the last processed 512-chunk needs masking; each 128-block in
    # it is fully allowed (gb < g), fully masked (gb > g), or diagonal.
    rr = np.arange(P)[:, None]
    sub_diag = np.where(np.arange(P)[None, :] <= rr, 0.0, NEG).astype(np.float32)
    masks_h = []
    for half in range(2):
        perm = PERMS[half]
        mk = np.empty((NLOC, P, 512), dtype=np.float32)
        for m_i, g in enumerate(L_HALF[half]):
            nch = NCHUNKS[m_i]
            for pos in range(4 * (nch - 1)):
                assert perm[pos] < g, (half, m_i, pos)
            for pos in range(4 * nch, NT):
                assert perm[pos] > g, (half, m_i, pos)
            base = 4 * (nch - 1)
            for jb in range(4):
                gb = perm[base + jb]
                if gb < g:
                    mk[m_i, :, jb * P : (jb + 1) * P] = 0.0
                elif gb > g:
                    mk[m_i, :, jb * P : (jb + 1) * P] = NEG
                else:
                    mk[m_i, :, jb * P : (jb + 1) * P] = sub_diag
        masks_h.append(mk)

    shared = {
        "wq": wq_f, "wk": wk_f, "wv": wv_f, "wv16": wv16_f,
        "w1": w1_f, "w2": w2_f8,
        "qb": qb, "kb": kb, "vb": vb, "b1": b1_f, "b2": b2,
        "ident": ident,
    }
    in_maps = []
    for core in range(8):
        b, half = core // 2, core % 2
        rows = np.concatenate(
            [np.arange(i * P, (i + 1) * P) for i in PERMS[half]]
        )
        xp = np.ascontiguousarray(x[b][rows])
        m = dict(shared)
        m["x"] = xp
        m["x_bf"] = xp.astype(bf)
        m["masks"] = masks_h[half]
        in_maps.append(m)
    return in_maps


def _scatter_out(results):
    out = np.empty((B, T, C), dtype=np.float32)
    for core in range(8):
        b, half = core // 2, core % 2
        L = L_HALF[half]
        o = results[core]["out"]
        for ppos, i in enumerate(L):
            out[b, i * P : (i + 1) * P, :] = o[ppos * P : (ppos + 1) * P, :]
    return out


def run(inputs, trace=False, **kw):
    nc = _get_nc()
    in_maps = _prep_host(inputs)
    res = run_bass_kernel_spmd(
        nc, in_maps, core_ids=list(range(8)), trace=trace, **kw
    )
    return _scatter_out(res.results), res


def kernel(**inputs) -> np.ndarray:
    out, _ = run(inputs, trace=False)
    return out


# revision 37
# speedup vs baseline: 1.2004x; 1.0016x over previous
"""Trainium2 Bass kernel for nn_Block (dense transformer block).

  out = x + FFN(LN2(x + Attn(LN1(x))))   with causal single-head attention,
  B=4, T=2048, C=H=1024, FF=4096, fp32 reference.

Distribution: 8 NeuronCores = (batch b in 0..3) x (query-half in 0..1).
Each core handles one batch element's keys/values and HALF its query rows
(causally balanced interleaved block split), plus LN2+FFN+residual for those
rows.  No collectives; the per-core programs are IDENTICAL (SPMD) - all
per-core variation is input data.

All matmul OPERANDS are fp8e4m3 driven in DoubleRow perf mode (2 fp8
weights per PE cell, K=256 contraction per matmul); every accumulation is
f32 in PSUM, and LN stats / softmax denominators / residual adds are f32.
LN gains/biases are folded into the weight matrices host-side.
"""

import sys
import types

import numpy as np

# ---------------------------------------------------------------------------
# antenv.axon_hooks shim: the image's antenv lacks this module and
# run_bass_kernel_spmd imports it under axon when trace=True.
import antenv

if "antenv.axon_hooks" not in sys.modules:
    _mod = types.ModuleType("antenv.axon_hooks")
    _mod._hook = None
    _mod.set_axon_ntff_profile_hook = lambda h: setattr(_mod, "_hook", h)
    _mod.get_axon_ntff_profile_hook = lambda: _mod._hook
    sys.modules["antenv.axon_hooks"] = _mod
    antenv.axon_hooks = _mod

import ml_dtypes

import concourse.bass as bass
import concourse.mybir as mybir
import concourse.tile as tile
from concourse.bass_utils import run_bass_kernel_spmd

F32 = mybir.dt.float32
BF16 = mybir.dt.bfloat16
E4 = mybir.dt.float8e4
DR = mybir.MatmulPerfMode.DoubleRow

B, T, C = 4, 2048, 1024
H, FF = 1024, 4096
P = 128
NT = T // P  # 16 token blocks per batch element
NCT = C // P  # 8 contraction tiles
NH = H // P  # 8 head-dim tiles
NF = FF // P  # 32 ff tiles
TOWN = T // 2  # own tokens per core (1024)
NLOC = TOWN // P  # 8 own blocks
EPS = 1e-5
SCALE = 1.0 / np.sqrt(np.float32(C))  # 1/32
NEG = -1.0e30

# Causally balanced query-block assignment (sum of chunk counts = 20 each).
L_HALF = [
    [0, 2, 4, 6, 9, 11, 13, 15],
    [1, 3, 5, 7, 8, 10, 12, 14],
]
# ceil((i+1)/4) for i in L_HALF[h] - same sequence for both halves.
NCHUNKS = [1, 1, 2, 2, 3, 3, 4, 4]
# Per-core BLOCK PERMUTATION of x: own blocks sit at even positions, the
# other half's at odd positions.  The program then addresses own tokens at
# fixed (core-independent) offsets; all per-core variation stays in data.
PERMS = [
    [L_HALF[h][m // 2] if m % 2 == 0 else L_HALF[1 - h][m // 2] for m in range(NT)]
    for h in range(2)
]


def _split_multi_waits(nc):
    """walrus here accepts at most ONE sync-wait per instruction; hoist
    extras onto injected same-engine NoOps."""
    for fn in nc.m.functions:
        for blk in fn.blocks:
            new_insts = []
            changed = False
            for inst in blk.instructions:
                si = getattr(inst, "sync_info", None)
                ow = list(si.on_wait) if si is not None and si.on_wait else []
                if len(ow) > 1:
                    for i, cond in enumerate(ow[:-1]):
                        new_insts.append(
                            mybir.InstNoOp(
                                name=f"{inst.name}-wn{i}",
                                engine=inst.engine,
                                ins=[],
                                outs=[],
                                sync_info=mybir.SyncInfo(
                                    on_wait=[cond], on_update=[]
                                ),
                            )
                        )
                    inst.sync_info = mybir.SyncInfo(
                        on_wait=[ow[-1]], on_update=list(si.on_update or [])
                    )
                    changed = True
                new_insts.append(inst)
            if changed:
                blk.instructions = new_insts


def _layernorm(nc, pool, x_t, h_t, eps_t):
    """h_t = (x_t - mean) * rsqrt(var + eps), stats along the free dim."""
    xg = x_t[:].rearrange("p (s f) -> p s f", f=512)
    stats = pool.tile([P, 2, nc.vector.BN_STATS_DIM], F32, tag="ln_stats")
    for sg in range(2):
        nc.vector.bn_stats(out=stats[:, sg], in_=xg[:, sg])
    mv = pool.tile([P, nc.vector.BN_AGGR_DIM], F32, tag="ln_mv")
    nc.vector.bn_aggr(out=mv[:], in_=stats[:])
    rstd = pool.tile([P, 1], F32, tag="ln_rstd")
    nc.scalar.activation(
        out=rstd[:],
        in_=mv[:, 1:2],
        func=mybir.ActivationFunctionType.Sqrt,
        bias=eps_t[:],
        scale=1.0,
    )
    nc.vector.reciprocal(out=rstd[:], in_=rstd[:])
    nc.vector.tensor_scalar(
        out=h_t[:],
        in0=x_t[:],
        scalar1=mv[:, 0:1],
        scalar2=rstd[:],
        op0=mybir.AluOpType.subtract,
        op1=mybir.AluOpType.mult,
    )


def build_nc():
    from contextlib import ExitStack

    nc = bass.Bass()

    x = nc.declare_dram_parameter("x", [T, C], F32, isOutput=False)
    x_bf = nc.declare_dram_parameter("x_bf", [T, C], BF16, isOutput=False)
    wq = nc.declare_dram_parameter("wq", [C, H], E4, isOutput=False)
    wk = nc.declare_dram_parameter("wk", [C, H], E4, isOutput=False)
    wv = nc.declare_dram_parameter("wv", [C, H], E4, isOutput=False)
    wv16 = nc.declare_dram_parameter("wv16", [C, H], BF16, isOutput=False)
    w1 = nc.declare_dram_parameter("w1", [NF, P, NCT, P], E4, isOutput=False)
    # w2[0] = e4m3(W2), w2[1] = e4m3(256*(W2 - w2[0])) - two-digit fp8
    w2 = nc.declare_dram_parameter("w2", [2, FF, C], E4, isOutput=False)
    qb = nc.declare_dram_parameter("qb", [H], F32, isOutput=False)
    kb = nc.declare_dram_parameter("kb", [H], F32, isOutput=False)
    vb = nc.declare_dram_parameter("vb", [H], F32, isOutput=False)
    b1 = nc.declare_dram_parameter("b1", [FF], F32, isOutput=False)
    b2 = nc.declare_dram_parameter("b2", [C], F32, isOutput=False)
    ident = nc.declare_dram_parameter("ident", [P, P], BF16, isOutput=False)
    masks = nc.declare_dram_parameter("masks", [NLOC, P, 512], F32, isOutput=False)
    out = nc.declare_dram_parameter("out", [TOWN, C], F32, isOutput=True)

    x2_d = nc.dram_tensor("x2_d", [TOWN, C], F32)

    wq_r = wq.rearrange("(ko p) h -> p ko h", p=P)
    wk_r = wk.rearrange("(ko p) h -> p ko h", p=P)
    wv_r = wv.rearrange("(ko p) h -> p ko h", p=P)
    wv16_r = wv16.rearrange("(ko p) h -> p ko h", p=P)
    w2_r = w2.rearrange("t (ko p) c -> p t ko c", p=P)

    with tile.TileContext(nc) as tc, ExitStack() as top:
        cn = top.enter_context(tc.tile_pool(name="cn", bufs=1))
        ps = top.enter_context(tc.tile_pool(name="ps", bufs=1, space="PSUM"))
        ln = top.enter_context(tc.tile_pool(name="ln", bufs=4))
        # big resident tensors spanning several phases
        res = top.enter_context(tc.tile_pool(name="res", bufs=1))

        # critical-path-first: the very first LN tile and the transpose
        # identity go ahead of the bulk constant loads.
        x0_t = ln.tile([P, C], BF16, tag="xt", name="x0t")
        nc.sync.dma_start(x0_t[:], x_bf[0:P, :])
        id_t = cn.tile([P, P], BF16)
        nc.sync.dma_start(id_t[:], ident[:])
        # ---- constants
        qb_t = cn.tile([P, NH], F32)
        nc.sync.dma_start(qb_t[:], qb.rearrange("(m p) -> p m", p=P))
        kb_t = cn.tile([P, NH], F32)
        nc.sync.dma_start(kb_t[:], kb.rearrange("(m p) -> p m", p=P))
        b1_t = cn.tile([P, NF], F32)
        nc.sync.dma_start(b1_t[:], b1.rearrange("(m p) -> p m", p=P))
        vb_b = cn.tile([P, H], F32)
        nc.sync.dma_start(vb_b[:], vb[None, :].partition_broadcast(P))
        b2_b = cn.tile([P, C], F32)
        nc.sync.dma_start(b2_b[:], b2[None, :].partition_broadcast(P))
        eps_t = cn.tile([P, 1], F32)
        nc.vector.memset(eps_t, EPS)

        _ctr = [0]

        def psum(tag, shape=(P, 512), dt=F32, bufs=2):
            _ctr[0] += 1
            return ps.tile(list(shape), dt, tag=tag, bufs=bufs, name=f"ps{_ctr[0]}")

        qTo = res.tile([P, NH, TOWN], E4)   # q^T own tokens (local order)
        kT = res.tile([P, NH, T], E4)       # k^T all keys
        v_sb = res.tile([P, NT, H], E4)     # v token-major, all keys
        h2T = res.tile([P, NCT, TOWN], E4)  # LN2 output transposed
        # precise (bf16) head-block path: first own block attends few keys, so
        # fp8 noise passes straight through - keep that slice in bf16.
        qTb = res.tile([P, NH, P], BF16)    # q^T for local block 0
        kTb = res.tile([P, NH, 512], BF16)  # k^T for key chunk 0
        v_bf = res.tile([P, 2, H], BF16)    # v for key blocks 0-1 (bf16 matmul)

        # ===== Phase B: LN1 over permuted blocks -> hT/hto; v; q^T; k^T ====
        # x arrives block-PERMUTED per core (own blocks at even positions),
        # so one LN pass feeds hT (all keys) and hto (own tokens, gathered by
        # the scalar engine) - no duplicated LN/transposes, and the v matmuls
        # give the PE work from the first tile on.
        with ExitStack() as sB:
            big_b = sB.enter_context(tc.tile_pool(name="bigb", bufs=1))
            wvp = sB.enter_context(tc.tile_pool(name="wvp", bufs=1))
            hT = big_b.tile([P, NCT, T], E4)
            hto = big_b.tile([P, NCT, TOWN], E4)
            hT_bf = big_b.tile([P, NCT, 256], BF16)
            # x tiles must BEAT the 5MB of weight loads to the DMA queues -
            # only wv is needed early; the rest are staggered into the loop.
            wv_t = wvp.tile([P, NCT, H], E4)
            nc.sync.dma_start(wv_t[:], wv_r[:])
            wq_t = wvp.tile([P, NCT, H], E4)
            wk_t = wvp.tile([P, NCT, H], E4)
            wv16_t = wvp.tile([P, NCT, H], BF16)
            # LN per permuted token tile; v row-block right after its tile
            for st in range(NT):
                if st == 0:
                    x_t = x0_t
                else:
                    x_t = ln.tile([P, C], BF16, tag="xb", bufs=6)
                    nc.sync.dma_start(x_t[:], x_bf[st * P : (st + 1) * P, :])
                if st == 1:
                    nc.sync.dma_start(wv16_t[:], wv16_r[:])
                elif st == 5:
                    nc.sync.dma_start(wq_t[:], wq_r[:])
                elif st == 9:
                    nc.sync.dma_start(wk_t[:], wk_r[:])
                h_t = ln.tile([P, C], BF16, tag="ht")
                _layernorm(nc, ln, x_t, h_t, eps_t)
                for c in range(NCT):
                    tp = psum("b", (P, P), BF16)
                    nc.tensor.transpose(tp[:], h_t[:, c * P : (c + 1) * P], id_t[:])
                    nc.vector.tensor_copy(hT[:, c, st * P : (st + 1) * P], tp[:])
                    if st % 2 == 0:
                        nc.scalar.activation(
                            out=hto[:, c, (st // 2) * P : (st // 2 + 1) * P],
                            in_=tp[:],
                            func=mybir.ActivationFunctionType.Copy,
                            scale=1.0,
                        )
                    if st < 2:
                        nc.vector.tensor_copy(
                            hT_bf[:, c, st * P : (st + 1) * P], tp[:]
                        )
                accs = [psum("a") for _ in range(2)]
                for k in range(NCT // 2):
                    for hh in range(2):
                        nc.tensor.matmul(
                            accs[hh][:],
                            hT[:, 2 * k : 2 * k + 2, st * P : (st + 1) * P],
                            wv_t[:, 2 * k : 2 * k + 2, hh * 512 : (hh + 1) * 512],
                            start=(k == 0),
                            stop=(k == NCT // 2 - 1),
                            perf_mode=DR,
                        )
                for hh in range(2):
                    nc.vector.tensor_add(
                        out=v_sb[:, st, hh * 512 : (hh + 1) * 512],
                        in0=accs[hh][:],
                        in1=vb_b[:, hh * 512 : (hh + 1) * 512],
                    )
                if st == 3:
                    # precise bf16 v for key blocks 0-1 - emitted here so the
                    # PE has work while the LN stream (DVE-bound) warms up
                    for blk in range(2):
                        accs = [psum("a") for _ in range(2)]
                        for k in range(NCT):
                            for hh in range(2):
                                nc.tensor.matmul(
                                    accs[hh][:],
                                    hT_bf[:, k, blk * P : (blk + 1) * P],
                                    wv16_t[:, k, hh * 512 : (hh + 1) * 512],
                                    start=(k == 0),
                                    stop=(k == NCT - 1),
                                )
                        for hh in range(2):
                            nc.vector.tensor_add(
                                out=v_bf[:, blk, hh * 512 : (hh + 1) * 512],
                                in0=accs[hh][:],
                                in1=vb_b[:, hh * 512 : (hh + 1) * 512],
                            )
            # q^T (own tokens, from hto)
            for m in range(NH):
                accs = [psum("a") for _ in range(2)]
                for k in range(NCT // 2):
                    for g in range(2):
                        nc.tensor.matmul(
                            accs[g][:],
                            wq_t[:, 2 * k : 2 * k + 2, m * P : (m + 1) * P],
                            hto[:, 2 * k : 2 * k + 2, g * 512 : (g + 1) * 512],
                            start=(k == 0),
                            stop=(k == NCT // 2 - 1),
                            perf_mode=DR,
                        )
                for g in range(2):
                    nc.vector.tensor_scalar_add(
                        out=qTo[:, m, g * 512 : (g + 1) * 512],
                        in0=accs[g][:],
                        scalar1=qb_t[:, m : m + 1],
                    )
                nc.vector.tensor_scalar_add(
                    out=qTb[:, m, :],
                    in0=accs[0][:, 0:P],
                    scalar1=qb_t[:, m : m + 1],
                )
            # k^T (all keys)
            for m in range(NH):
                accs = [psum("c", bufs=4) for _ in range(4)]
                for k in range(NCT // 2):
                    for ch in range(4):
                        nc.tensor.matmul(
                            accs[ch][:],
                            wk_t[:, 2 * k : 2 * k + 2, m * P : (m + 1) * P],
                            hT[:, 2 * k : 2 * k + 2, ch * 512 : (ch + 1) * 512],
                            start=(k == 0),
                            stop=(k == NCT // 2 - 1),
                            perf_mode=DR,
                        )
                for ch in range(4):
                    nc.vector.tensor_scalar_add(
                        out=kT[:, m, ch * 512 : (ch + 1) * 512],
                        in0=accs[ch][:],
                        scalar1=kb_t[:, m : m + 1],
                    )
                nc.vector.tensor_scalar_add(
                    out=kTb[:, m, :],
                    in0=accs[0][:],
                    scalar1=kb_t[:, m : m + 1],
                )

        # ============== Phase C: attention (software-pipelined) ============
        with ExitStack() as sC:
            att = sC.enter_context(tc.tile_pool(name="att", bufs=2))
            wtl = sC.enter_context(tc.tile_pool(name="wtl", bufs=16))
            state = {}

            def emit_scores(lp):
                nch = NCHUNKS[lp]
                mask_t = att.tile([P, 512], F32, tag="mask")
                nc.sync.dma_start(mask_t[:], masks[lp])
                p_t = att.tile([P, T], BF16, tag="pt", bufs=3)
                den = att.tile([P, 4], F32, tag="den")
                scs = [psum("c", bufs=4) for _ in range(nch)]
                if lp == 0:
                    # precise bf16 scores for the head block
                    for m in range(NH):
                        nc.tensor.matmul(
                            scs[0][:],
                            qTb[:, m, :],
                            kTb[:, m, :],
                            start=(m == 0),
                            stop=(m == NH - 1),
                        )
                else:
                    for m in range(NH // 2):
                        for j in range(nch):
                            nc.tensor.matmul(
                                scs[j][:],
                                qTo[:, 2 * m : 2 * m + 2, lp * P : (lp + 1) * P],
                                kT[:, 2 * m : 2 * m + 2, j * 512 : (j + 1) * 512],
                                start=(m == 0),
                                stop=(m == NH // 2 - 1),
                                perf_mode=DR,
                            )
                for j in range(nch):
                    if j == nch - 1:
                        nc.vector.tensor_add(
                            out=scs[j][:], in0=scs[j][:], in1=mask_t[:]
                        )
                    nc.scalar.activation(
                        out=p_t[:, j * 512 : (j + 1) * 512],
                        in_=scs[j][:],
                        func=mybir.ActivationFunctionType.Exp,
                        scale=float(SCALE),
                        accum_out=den[:, j : j + 1],
                    )
                state[lp] = (p_t, den)

            def emit_tail(lp):
                nch = NCHUNKS[lp]
                nst = 4 * nch
                p_t, den = state.pop(lp)
                dsum = att.tile([P, 1], F32, tag="dsum")
                nc.vector.reduce_sum(
                    out=dsum[:], in_=den[:, :nch], axis=mybir.AxisListType.X
                )
                nc.vector.reciprocal(out=dsum[:], in_=dsum[:])
                sa0 = psum("a")
                sa1 = psum("a")
                if lp == 0:
                    # precise bf16 p@v over key blocks 0-1 (rest masked to zero)
                    wtb = []
                    for st in range(2):
                        tp = psum("b", (P, P), BF16)
                        nc.tensor.transpose(
                            tp[:], p_t[:, st * P : (st + 1) * P], id_t[:]
                        )
                        wt = wtl.tile([P, P], BF16, tag="wtb")
                        nc.vector.tensor_copy(wt[:], tp[:])
                        wtb.append(wt)
                    for st in range(2):
                        nc.tensor.matmul(
                            sa0[:], wtb[st][:], v_bf[:, st, 0:512],
                            start=(st == 0), stop=(st == 1),
                        )
                        nc.tensor.matmul(
                            sa1[:], wtb[st][:], v_bf[:, st, 512:1024],
                            start=(st == 0), stop=(st == 1),
                        )
                else:
                    wtp = []
                    for sp in range(nst // 2):
                        wt = wtl.tile([P, 2, P], E4, tag="wt")
                        for u in range(2):
                            st = 2 * sp + u
                            tp = psum("b", (P, P), BF16)
                            nc.tensor.transpose(
                                tp[:], p_t[:, st * P : (st + 1) * P], id_t[:]
                            )
                            nc.vector.tensor_copy(wt[:, u], tp[:])
                        wtp.append(wt)
                    for sp in range(nst // 2):
                        nc.tensor.matmul(
                            sa0[:], wtp[sp][:], v_sb[:, 2 * sp : 2 * sp + 2, 0:512],
                            start=(sp == 0), stop=(sp == nst // 2 - 1),
                            perf_mode=DR,
                        )
                        nc.tensor.matmul(
                            sa1[:], wtp[sp][:],
                            v_sb[:, 2 * sp : 2 * sp + 2, 512:1024],
                            start=(sp == 0), stop=(sp == nst // 2 - 1),
                            perf_mode=DR,
                        )
                x_t = att.tile([P, C], F32, tag="xo")
                nc.sync.dma_start(x_t[:], x[2 * lp * P : (2 * lp + 1) * P, :])
                x2_t = att.tile([P, C], F32, tag="x2")
                # sa/den scaling on the scalar engine - DVE gates the PE here
                nc.scalar.activation(
                    out=x2_t[:, 0:512], in_=sa0[:],
                    func=mybir.ActivationFunctionType.Copy, scale=dsum[:],
                )
                nc.scalar.activation(
                    out=x2_t[:, 512:1024], in_=sa1[:],
                    func=mybir.ActivationFunctionType.Copy, scale=dsum[:],
                )
                nc.vector.tensor_add(out=x2_t[:], in0=x2_t[:], in1=x_t[:])
                nc.sync.dma_start(x2_d[lp * P : (lp + 1) * P, :], x2_t[:])

            emit_scores(0)
            emit_scores(1)
            for lp in range(2, NLOC):
                emit_scores(lp)
                emit_tail(lp - 2)
            emit_tail(NLOC - 2)
            emit_tail(NLOC - 1)

        # ============== Phase C2: LN2 + h2^T ===============================
        def emit_ln2(lt):
            x2_t = ln.tile([P, C], F32, tag="xt")
            nc.sync.dma_start(x2_t[:], x2_d[lt * P : (lt + 1) * P, :])
            h2_t = ln.tile([P, C], BF16, tag="ht")
            _layernorm(nc, ln, x2_t, h2_t, eps_t)
            for c in range(NCT):
                tp = psum("b", (P, P), BF16)
                nc.tensor.transpose(tp[:], h2_t[:, c * P : (c + 1) * P], id_t[:])
                nc.scalar.activation(
                    out=h2T[:, c, lt * P : (lt + 1) * P],
                    in_=tp[:],
                    func=mybir.ActivationFunctionType.Copy,
                    scale=1.0,
                )

        for lt in range(4):
            emit_ln2(lt)

        # ================= Phase D: FFN (fp8 DoubleRow) ====================
        with ExitStack() as sD:
            big_d = sD.enter_context(tc.tile_pool(name="bigd", bufs=1))
            ffw = sD.enter_context(tc.tile_pool(name="ffw", bufs=3))
            aT = [
                big_d.tile([P, NF, 512], E4, name=f"aT{i}") for i in range(2)
            ]

            def emit_aT(tch):
                # a^T half = relu(W1^T h2^T + b1) for 512 local tokens
                for ft in range(NF):
                    w1_t = ffw.tile(
                        [P, NCT, P], E4, tag="w1t", bufs=4, name=f"w1t{tch}_{ft}"
                    )
                    nc.sync.dma_start(w1_t[:], w1[ft])
                    acc = psum("a")
                    for k in range(NCT // 2):
                        nc.tensor.matmul(
                            acc[:],
                            w1_t[:, 2 * k : 2 * k + 2],
                            h2T[:, 2 * k : 2 * k + 2, tch * 512 : (tch + 1) * 512],
                            start=(k == 0),
                            stop=(k == NCT // 2 - 1),
                            perf_mode=DR,
                        )
                    nc.scalar.activation(
                        out=aT[tch][:, ft, :],
                        in_=acc[:],
                        func=mybir.ActivationFunctionType.Relu,
                        bias=b1_t[:, ft : ft + 1],
                        scale=1.0,
                    )

            def mk_grp():
                return [
                    [
                        psum(
                            ("a" if tb < 1 else "b" if tb < 2 else "c"),
                            bufs=(2 if tb < 2 else 4),
                        )
                        for cc in range(2)
                    ]
                    for tb in range(4)
                ]

            def ff_pass(tbh, dig, grp):
                for ft in range(NF // 2):
                    w2_t = ffw.tile(
                        [P, 2, C], E4, tag="w2t", bufs=4,
                        name=f"w2t{tbh}_{dig}_{ft}",
                    )
                    nc.sync.dma_start(
                        w2_t[:], w2_r[:, dig, 2 * ft : 2 * ft + 2, :]
                    )
                    for tb in range(4):
                        for cc in range(2):
                            nc.tensor.matmul(
                                grp[tb][cc][:],
                                aT[tbh][:, 2 * ft : 2 * ft + 2,
                                        tb * P : (tb + 1) * P],
                                w2_t[:, :, cc * 512 : (cc + 1) * 512],
                                start=(ft == 0),
                                stop=(ft == NF // 2 - 1),
                                perf_mode=DR,
                            )

            def emit_ff_hi(tbh):
                # hi-digit pass, evacuated (+b2) into o_t
                grp = mk_grp()
                ff_pass(tbh, 0, grp)
                o_ts = []
                for tb in range(4):
                    o_t = ffw.tile(
                        [P, C], F32, tag="ot", bufs=5, name=f"ot{tbh}_{tb}"
                    )
                    for cc in range(2):
                        nc.vector.tensor_add(
                            out=o_t[:, cc * 512 : (cc + 1) * 512],
                            in0=grp[tb][cc][:],
                            in1=b2_b[:, cc * 512 : (cc + 1) * 512],
                        )
                    o_ts.append(o_t)
                return o_ts

            def emit_ff_lo(tbh, o_ts):
                # lo-digit pass (weights pre-scaled x256), combined at 1/256
                grp = mk_grp()
                ff_pass(tbh, 1, grp)
                for tb in range(4):
                    lt = tbh * 4 + tb
                    x2_t = ffw.tile([P, C], F32, tag="x2r", name=f"x2r{tbh}_{tb}")
                    nc.sync.dma_start(x2_t[:], x2_d[lt * P : (lt + 1) * P, :])
                    o_t = o_ts[tb]
                    lo_t = ffw.tile([P, C], F32, tag="lot", name=f"lot{tbh}_{tb}")
                    for cc in range(2):
                        nc.scalar.activation(
                            out=lo_t[:, cc * 512 : (cc + 1) * 512],
                            in_=grp[tb][cc][:],
                            func=mybir.ActivationFunctionType.Copy,
                            scale=float(1.0 / 256.0),
                        )
                    nc.vector.tensor_add(out=o_t[:], in0=o_t[:], in1=lo_t[:])
                    nc.vector.tensor_add(out=o_t[:], in0=o_t[:], in1=x2_t[:])
                    nc.sync.dma_start(out[lt * P : (lt + 1) * P, :], o_t[:])

            # aT(0) needs only LN2 of tiles 0-3; LN2 of 4-7 overlaps its
            # matmuls. aT(1) overlaps the hi(0) PSUM evacuation; lo(0) still
            # reads aT[0], so the two halves use separate aT buffers.
            emit_aT(0)
            for lt in range(4, NLOC):
                emit_ln2(lt)
            o0 = emit_ff_hi(0)
            emit_aT(1)
            emit_ff_lo(0, o0)
            o1 = emit_ff_hi(1)
            emit_ff_lo(1, o1)

    _split_multi_waits(nc)
    return nc


_NC_CACHE = None


def _get_nc():
    global _NC_CACHE
    if _NC_CACHE is None:
        _NC_CACHE = build_nc()
    return _NC_CACHE


def _prep_host(inputs):
    """Fold LN gains/biases into weights; build per-core input maps."""
    x = np.asarray(inputs["x"], dtype=np.float32)
    Wk = np.asarray(inputs["Wk"], dtype=np.float32)
    Wq = np.asarray(inputs["Wq"], dtype=np.float32)
    Wv = np.asarray(inputs["Wv"], dtype=np.float32)
    W1 = np.asarray(inputs["W1"], dtype=np.float32)
    b1 = np.asarray(inputs["b1"], dtype=np.float32)
    W2 = np.asarray(inputs["W2"], dtype=np.float32)
    b2 = np.asarray(inputs["b2"], dtype=np.float32)
    g1 = np.asarray(inputs["g1"], dtype=np.float32)
    be1 = np.asarray(inputs["be1"], dtype=np.float32)
    g2 = np.asarray(inputs["g2"], dtype=np.float32)
    be2 = np.asarray(inputs["be2"], dtype=np.float32)

    f8 = ml_dtypes.float8_e4m3
    bf = ml_dtypes.bfloat16
    wq_f = np.ascontiguousarray((g1[:, None] * Wq).astype(f8))
    wk_f = np.ascontiguousarray((g1[:, None] * Wk).astype(f8))
    wv_full = g1[:, None] * Wv
    wv_f = np.ascontiguousarray(wv_full.astype(f8))
    wv16_f = np.ascontiguousarray(wv_full.astype(bf))
    qb = be1 @ Wq
    kb = be1 @ Wk
    vb = be1 @ Wv
    w1_full = (g2[:, None] * W1).astype(f8)
    w1_f = np.ascontiguousarray(
        w1_full.reshape(NCT, P, NF, P).transpose(2, 1, 0, 3)
    )
    w2_hi = W2.astype(f8)
    w2_lo = (256.0 * (W2 - w2_hi.astype(np.float32))).astype(f8)
    w2_f8 = np.ascontiguousarray(np.stack([w2_hi, w2_lo], axis=0))
    b1_f = b1 + be2 @ W1

    ident = np.eye(P, dtype=ml_dtypes.bfloat16)

    # per-half masks against the PERMUTED key order: for own block lp (global
    # g), only the last processed 512-chunk needs masking; each 128-block in
    # it is fully allowed (gb < g), fully masked (gb > g), or diagonal.
    rr = np.arange(P)[:, None]
    sub_diag = np.where(np.arange(P)[None, :] <= rr, 0.0, NEG).astype(np.float32)
    masks_h = []
    for half in range(2):
        perm = PERMS[half]
        mk = np.empty((NLOC, P, 512), dtype=np.float32)
        for m_i, g in enumerate(L_HALF[half]):
            nch = NCHUNKS[m_i]
            for pos in range(4 * (nch - 1)):
                assert perm[pos] < g, (half, m_i, pos)
            for pos in range(4 * nch, NT):
                assert perm[pos] > g, (half, m_i, pos)
            base = 4 * (nch - 1)
            for jb in range(4):
                gb = perm[base + jb]
                if gb < g:
                    mk[m_i, :, jb * P : (jb + 1) * P] = 0.0
                elif gb > g:
                    mk[m_i, :, jb * P : (jb + 1) * P] = NEG
                else:
                    mk[m_i, :, jb * P : (jb + 1) * P] = sub_diag
        masks_h.append(mk)

    shared = {
        "wq": wq_f, "wk": wk_f, "wv": wv_f, "wv16": wv16_f,
        "w1": w1_f, "w2": w2_f8,
        "qb": qb, "kb": kb, "vb": vb, "b1": b1_f, "b2": b2,
        "ident": ident,
    }
    in_maps = []
    for core in range(8):
        b, half = core // 2, core % 2
        rows = np.concatenate(
            [np.arange(i * P, (i + 1) * P) for i in PERMS[half]]
        )
        xp = np.ascontiguousarray(x[b][rows])
        m = dict(shared)
        m["x"] = xp
        m["x_bf"] = xp.astype(bf)
        m["masks"] = masks_h[half]
        in_maps.append(m)
    return in_maps


def _scatter_out(results):
    out = np.empty((B, T, C), dtype=np.float32)
    for core in range(8):
        b, half = core // 2, core % 2
        L = L_HALF[half]
        o = results[core]["out"]
        for ppos, i in enumerate(L):
            out[b, i * P : (i + 1) * P, :] = o[ppos * P : (ppos + 1) * P, :]
    return out


def run(inputs, trace=False, **kw):
    nc = _get_nc()
    in_maps = _prep_host(inputs)
    res = run_bass_kernel_spmd(
        nc, in_maps, core_ids=list(range(8)), trace=trace, **kw
    )
    return _scatter_out(res.results), res


def kernel(**inputs) -> np.ndarray:
    out, _ = run(inputs, trace=False)
    return out


# revision 39
# speedup vs baseline: 1.2159x; 1.0129x over previous
"""Trainium2 Bass kernel for nn_Block (dense transformer block).

  out = x + FFN(LN2(x + Attn(LN1(x))))   with causal single-head attention,
  B=4, T=2048, C=H=1024, FF=4096, fp32 reference.

Distribution: 8 NeuronCores = (batch b in 0..3) x (query-half in 0..1).
Each core handles one batch element's keys/values and HALF its query rows
(causally balanced interleaved block split), plus LN2+FFN+residual for those
rows.  No collectives; the per-core programs are IDENTICAL (SPMD) - all
per-core variation is input data.

All matmul OPERANDS are fp8e4m3 driven in DoubleRow perf mode (2 fp8
weights per PE cell, K=256 contraction per matmul); every accumulation is
f32 in PSUM, and LN stats / softmax denominators / residual adds are f32.
LN gains/biases are folded into the weight matrices host-side.
"""

import sys
import types

import numpy as np

# ---------------------------------------------------------------------------
# antenv.axon_hooks shim: the image's antenv lacks this module and
# run_bass_kernel_spmd imports it under axon when trace=True.
import antenv

if "antenv.axon_hooks" not in sys.modules:
    _mod = types.ModuleType("antenv.axon_hooks")
    _mod._hook = None
    _mod.set_axon_ntff_profile_hook = lambda h: setattr(_mod, "_hook", h)
    _mod.get_axon_ntff_profile_hook = lambda: _mod._hook
    sys.modules["antenv.axon_hooks"] = _mod
    antenv.axon_hooks = _mod

import ml_dtypes

import concourse.bass as bass
import concourse.mybir as mybir
import concourse.tile as tile
from concourse.bass_utils import run_bass_kernel_spmd

F32 = mybir.dt.float32
BF16 = mybir.dt.bfloat16
E4 = mybir.dt.float8e4
DR = mybir.MatmulPerfMode.DoubleRow

B, T, C = 4, 2048, 1024
H, FF = 1024, 4096
P = 128
NT = T // P  # 16 token blocks per batch element
NCT = C // P  # 8 contraction tiles
NH = H // P  # 8 head-dim tiles
NF = FF // P  # 32 ff tiles
TOWN = T // 2  # own tokens per core (1024)
NLOC = TOWN // P  # 8 own blocks
EPS = 1e-5
SCALE = 1.0 / np.sqrt(np.float32(C))  # 1/32
NEG = -1.0e30

# Causally balanced query-block assignment (sum of chunk counts = 20 each).
L_HALF = [
    [0, 2, 4, 6, 9, 11, 13, 15],
    [1, 3, 5, 7, 8, 10, 12, 14],
]
# ceil((i+1)/4) for i in L_HALF[h] - same sequence for both halves.
NCHUNKS = [1, 1, 2, 2, 3, 3, 4, 4]
# Per-core BLOCK PERMUTATION of x: own blocks sit at even positions, the
# other half's at odd positions.  The program then addresses own tokens at
# fixed (core-independent) offsets; all per-core variation stays in data.
PERMS = [
    [L_HALF[h][m // 2] if m % 2 == 0 else L_HALF[1 - h][m // 2] for m in range(NT)]
    for h in range(2)
]


def _split_multi_waits(nc):
    """walrus here accepts at most ONE sync-wait per instruction; hoist
    extras onto injected same-engine NoOps."""
    for fn in nc.m.functions:
        for blk in fn.blocks:
            new_insts = []
            changed = False
            for inst in blk.instructions:
                si = getattr(inst, "sync_info", None)
                ow = list(si.on_wait) if si is not None and si.on_wait else []
                if len(ow) > 1:
                    for i, cond in enumerate(ow[:-1]):
                        new_insts.append(
                            mybir.InstNoOp(
                                name=f"{inst.name}-wn{i}",
                                engine=inst.engine,
                                ins=[],
                                outs=[],
                                sync_info=mybir.SyncInfo(
                                    on_wait=[cond], on_update=[]
                                ),
                            )
                        )
                    inst.sync_info = mybir.SyncInfo(
                        on_wait=[ow[-1]], on_update=list(si.on_update or [])
                    )
                    changed = True
                new_insts.append(inst)
            if changed:
                blk.instructions = new_insts


def _layernorm(nc, pool, x_t, h_t, eps_t):
    """h_t = (x_t - mean) * rsqrt(var + eps), stats along the free dim."""
    xg = x_t[:].rearrange("p (s f) -> p s f", f=512)
    stats = pool.tile([P, 2, nc.vector.BN_STATS_DIM], F32, tag="ln_stats")
    for sg in range(2):
        nc.vector.bn_stats(out=stats[:, sg], in_=xg[:, sg])
    mv = pool.tile([P, nc.vector.BN_AGGR_DIM], F32, tag="ln_mv")
    nc.vector.bn_aggr(out=mv[:], in_=stats[:])
    rstd = pool.tile([P, 1], F32, tag="ln_rstd")
    nc.scalar.activation(
        out=rstd[:],
        in_=mv[:, 1:2],
        func=mybir.ActivationFunctionType.Sqrt,
        bias=eps_t[:],
        scale=1.0,
    )
    nc.vector.reciprocal(out=rstd[:], in_=rstd[:])
    nc.vector.tensor_scalar(
        out=h_t[:],
        in0=x_t[:],
        scalar1=mv[:, 0:1],
        scalar2=rstd[:],
        op0=mybir.AluOpType.subtract,
        op1=mybir.AluOpType.mult,
    )


def build_nc():
    from contextlib import ExitStack

    nc = bass.Bass()

    x = nc.declare_dram_parameter("x", [T, C], F32, isOutput=False)
    x_bf = nc.declare_dram_parameter("x_bf", [T, C], BF16, isOutput=False)
    wq = nc.declare_dram_parameter("wq", [C, H], E4, isOutput=False)
    wk = nc.declare_dram_parameter("wk", [C, H], E4, isOutput=False)
    wv = nc.declare_dram_parameter("wv", [C, H], E4, isOutput=False)
    wv16 = nc.declare_dram_parameter("wv16", [C, H], BF16, isOutput=False)
    w1 = nc.declare_dram_parameter("w1", [NF, P, NCT, P], E4, isOutput=False)
    # w2[0] = e4m3(W2), w2[1] = e4m3(256*(W2 - w2[0])) - two-digit fp8
    w2 = nc.declare_dram_parameter("w2", [2, FF, C], E4, isOutput=False)
    qb = nc.declare_dram_parameter("qb", [H], F32, isOutput=False)
    kb = nc.declare_dram_parameter("kb", [H], F32, isOutput=False)
    vb = nc.declare_dram_parameter("vb", [H], F32, isOutput=False)
    b1 = nc.declare_dram_parameter("b1", [FF], F32, isOutput=False)
    b2 = nc.declare_dram_parameter("b2", [C], F32, isOutput=False)
    ident = nc.declare_dram_parameter("ident", [P, P], BF16, isOutput=False)
    masks = nc.declare_dram_parameter("masks", [NLOC, P, 512], F32, isOutput=False)
    out = nc.declare_dram_parameter("out", [TOWN, C], F32, isOutput=True)

    x2_d = nc.dram_tensor("x2_d", [TOWN, C], F32)

    wq_r = wq.rearrange("(ko p) h -> p ko h", p=P)
    wk_r = wk.rearrange("(ko p) h -> p ko h", p=P)
    wv_r = wv.rearrange("(ko p) h -> p ko h", p=P)
    wv16_r = wv16.rearrange("(ko p) h -> p ko h", p=P)
    w2_r = w2.rearrange("t (ko p) c -> p t ko c", p=P)

    with tile.TileContext(nc) as tc, ExitStack() as top:
        cn = top.enter_context(tc.tile_pool(name="cn", bufs=1))
        ps = top.enter_context(tc.tile_pool(name="ps", bufs=1, space="PSUM"))
        ln = top.enter_context(tc.tile_pool(name="ln", bufs=4))
        # big resident tensors spanning several phases
        res = top.enter_context(tc.tile_pool(name="res", bufs=1))

        # critical-path-first: the very first LN tile and the transpose
        # identity go ahead of the bulk constant loads.
        x0_t = ln.tile([P, C], BF16, tag="xb", bufs=6, name="x0t")
        nc.sync.dma_start(x0_t[:], x_bf[0:P, :])
        id_t = cn.tile([P, P], BF16)
        nc.sync.dma_start(id_t[:], ident[:])
        xpre = [x0_t]
        for stp in range(1, 4):
            t = ln.tile([P, C], BF16, tag="xb", bufs=6, name=f"xp{stp}")
            nc.sync.dma_start(t[:], x_bf[stp * P : (stp + 1) * P, :])
            xpre.append(t)
        # ---- constants
        qb_t = cn.tile([P, NH], F32)
        nc.sync.dma_start(qb_t[:], qb.rearrange("(m p) -> p m", p=P))
        kb_t = cn.tile([P, NH], F32)
        nc.sync.dma_start(kb_t[:], kb.rearrange("(m p) -> p m", p=P))
        b1_t = cn.tile([P, NF], F32)
        nc.sync.dma_start(b1_t[:], b1.rearrange("(m p) -> p m", p=P))
        vb_b = cn.tile([P, H], F32)
        nc.sync.dma_start(vb_b[:], vb[None, :].partition_broadcast(P))
        b2_b = cn.tile([P, C], F32)
        nc.sync.dma_start(b2_b[:], b2[None, :].partition_broadcast(P))
        eps_t = cn.tile([P, 1], F32)
        nc.vector.memset(eps_t, EPS)

        _ctr = [0]

        def psum(tag, shape=(P, 512), dt=F32, bufs=2):
            _ctr[0] += 1
            return ps.tile(list(shape), dt, tag=tag, bufs=bufs, name=f"ps{_ctr[0]}")

        qTo = res.tile([P, NH, TOWN], E4)   # q^T own tokens (local order)
        kT = res.tile([P, NH, T], E4)       # k^T all keys
        v_sb = res.tile([P, NT, H], E4)     # v token-major, all keys
        h2T = res.tile([P, NCT, TOWN], E4)  # LN2 output transposed
        # precise (bf16) head-block path: first own block attends few keys, so
        # fp8 noise passes straight through - keep that slice in bf16.
        qTb = res.tile([P, NH, P], BF16)    # q^T for local block 0
        kTb = res.tile([P, NH, 512], BF16)  # k^T for key chunk 0
        v_bf = res.tile([P, 2, H], BF16)    # v for key blocks 0-1 (bf16 matmul)

        # ===== Phase B: LN1 over permuted blocks -> hT/hto; v; q^T; k^T ====
        # x arrives block-PERMUTED per core (own blocks at even positions),
        # so one LN pass feeds hT (all keys) and hto (own tokens, gathered by
        # the scalar engine) - no duplicated LN/transposes, and the v matmuls
        # give the PE work from the first tile on.
        with ExitStack() as sB:
            big_b = sB.enter_context(tc.tile_pool(name="bigb", bufs=1))
            wvp = sB.enter_context(tc.tile_pool(name="wvp", bufs=1))
            hT = big_b.tile([P, NCT, T], E4)
            hto = big_b.tile([P, NCT, TOWN], E4)
            hT_bf = big_b.tile([P, NCT, 256], BF16)
            # x tiles must BEAT the 5MB of weight loads to the DMA queues -
            # only wv is needed early; the rest are staggered into the loop.
            wv_t = wvp.tile([P, NCT, H], E4)
            nc.sync.dma_start(wv_t[:], wv_r[:])
            wq_t = wvp.tile([P, NCT, H], E4)
            wk_t = wvp.tile([P, NCT, H], E4)
            wv16_t = wvp.tile([P, NCT, H], BF16)
            # LN per permuted token tile; v row-block right after its tile
            for st in range(NT):
                if st < 4:
                    x_t = xpre[st]
                else:
                    x_t = ln.tile([P, C], BF16, tag="xb", bufs=6)
                    nc.sync.dma_start(x_t[:], x_bf[st * P : (st + 1) * P, :])
                if st == 1:
                    nc.sync.dma_start(wv16_t[:], wv16_r[:])
                elif st == 5:
                    nc.sync.dma_start(wq_t[:], wq_r[:])
                elif st == 9:
                    nc.sync.dma_start(wk_t[:], wk_r[:])
                h_t = ln.tile([P, C], BF16, tag="ht")
                _layernorm(nc, ln, x_t, h_t, eps_t)
                for c in range(NCT):
                    tp = psum("b", (P, P), BF16)
                    nc.tensor.transpose(tp[:], h_t[:, c * P : (c + 1) * P], id_t[:])
                    nc.vector.tensor_copy(hT[:, c, st * P : (st + 1) * P], tp[:])
                    if st % 2 == 0:
                        nc.scalar.activation(
                            out=hto[:, c, (st // 2) * P : (st // 2 + 1) * P],
                            in_=tp[:],
                            func=mybir.ActivationFunctionType.Copy,
                            scale=1.0,
                        )
                    if st < 2:
                        nc.vector.tensor_copy(
                            hT_bf[:, c, st * P : (st + 1) * P], tp[:]
                        )
                accs = [psum("a") for _ in range(2)]
                for k in range(NCT // 2):
                    for hh in range(2):
                        nc.tensor.matmul(
                            accs[hh][:],
                            hT[:, 2 * k : 2 * k + 2, st * P : (st + 1) * P],
                            wv_t[:, 2 * k : 2 * k + 2, hh * 512 : (hh + 1) * 512],
                            start=(k == 0),
                            stop=(k == NCT // 2 - 1),
                            perf_mode=DR,
                        )
                for hh in range(2):
                    nc.vector.tensor_add(
                        out=v_sb[:, st, hh * 512 : (hh + 1) * 512],
                        in0=accs[hh][:],
                        in1=vb_b[:, hh * 512 : (hh + 1) * 512],
                    )
                if st == 3:
                    # precise bf16 v for key blocks 0-1 - emitted here so the
                    # PE has work while the LN stream (DVE-bound) warms up
                    for blk in range(2):
                        accs = [psum("a") for _ in range(2)]
                        for k in range(NCT):
                            for hh in range(2):
                                nc.tensor.matmul(
                                    accs[hh][:],
                                    hT_bf[:, k, blk * P : (blk + 1) * P],
                                    wv16_t[:, k, hh * 512 : (hh + 1) * 512],
                                    start=(k == 0),
                                    stop=(k == NCT - 1),
                                )
                        for hh in range(2):
                            nc.vector.tensor_add(
                                out=v_bf[:, blk, hh * 512 : (hh + 1) * 512],
                                in0=accs[hh][:],
                                in1=vb_b[:, hh * 512 : (hh + 1) * 512],
                            )
            # q^T (own tokens, from hto)
            for m in range(NH):
                accs = [psum("a") for _ in range(2)]
                for k in range(NCT // 2):
                    for g in range(2):
                        nc.tensor.matmul(
                            accs[g][:],
                            wq_t[:, 2 * k : 2 * k + 2, m * P : (m + 1) * P],
                            hto[:, 2 * k : 2 * k + 2, g * 512 : (g + 1) * 512],
                            start=(k == 0),
                            stop=(k == NCT // 2 - 1),
                            perf_mode=DR,
                        )
                for g in range(2):
                    nc.vector.tensor_scalar_add(
                        out=qTo[:, m, g * 512 : (g + 1) * 512],
                        in0=accs[g][:],
                        scalar1=qb_t[:, m : m + 1],
                    )
                nc.vector.tensor_scalar_add(
                    out=qTb[:, m, :],
                    in0=accs[0][:, 0:P],
                    scalar1=qb_t[:, m : m + 1],
                )
            # k^T (all keys)
            for m in range(NH):
                accs = [psum("c", bufs=4) for _ in range(4)]
                for k in range(NCT // 2):
                    for ch in range(4):
                        nc.tensor.matmul(
                            accs[ch][:],
                            wk_t[:, 2 * k : 2 * k + 2, m * P : (m + 1) * P],
                            hT[:, 2 * k : 2 * k + 2, ch * 512 : (ch + 1) * 512],
                            start=(k == 0),
                            stop=(k == NCT // 2 - 1),
                            perf_mode=DR,
                        )
                for ch in range(4):
                    nc.vector.tensor_scalar_add(
                        out=kT[:, m, ch * 512 : (ch + 1) * 512],
                        in0=accs[ch][:],
                        scalar1=kb_t[:, m : m + 1],
                    )
                nc.vector.tensor_scalar_add(
                    out=kTb[:, m, :],
                    in0=accs[0][:],
                    scalar1=kb_t[:, m : m + 1],
                )

        # ============== Phase C: attention (software-pipelined) ============
        with ExitStack() as sC:
            att = sC.enter_context(tc.tile_pool(name="att", bufs=2))
            wtl = sC.enter_context(tc.tile_pool(name="wtl", bufs=16))
            state = {}

            def emit_scores(lp):
                nch = NCHUNKS[lp]
                mask_t = att.tile([P, 512], F32, tag="mask")
                nc.sync.dma_start(mask_t[:], masks[lp])
                p_t = att.tile([P, T], BF16, tag="pt", bufs=3)
                den = att.tile([P, 4], F32, tag="den")
                scs = [psum("c", bufs=4) for _ in range(nch)]
                if lp == 0:
                    # precise bf16 scores for the head block
                    for m in range(NH):
                        nc.tensor.matmul(
                            scs[0][:],
                            qTb[:, m, :],
                            kTb[:, m, :],
                            start=(m == 0),
                            stop=(m == NH - 1),
                        )
                else:
                    for m in range(NH // 2):
                        for j in range(nch):
                            nc.tensor.matmul(
                                scs[j][:],
                                qTo[:, 2 * m : 2 * m + 2, lp * P : (lp + 1) * P],
                                kT[:, 2 * m : 2 * m + 2, j * 512 : (j + 1) * 512],
                                start=(m == 0),
                                stop=(m == NH // 2 - 1),
                                perf_mode=DR,
                            )
                for j in range(nch):
                    if j == nch - 1:
                        nc.vector.tensor_add(
                            out=scs[j][:], in0=scs[j][:], in1=mask_t[:]
                        )
                    nc.scalar.activation(
                        out=p_t[:, j * 512 : (j + 1) * 512],
                        in_=scs[j][:],
                        func=mybir.ActivationFunctionType.Exp,
                        scale=float(SCALE),
                        accum_out=den[:, j : j + 1],
                    )
                state[lp] = (p_t, den)

            def emit_tail(lp):
                nch = NCHUNKS[lp]
                nst = 4 * nch
                p_t, den = state.pop(lp)
                dsum = att.tile([P, 1], F32, tag="dsum")
                nc.vector.reduce_sum(
                    out=dsum[:], in_=den[:, :nch], axis=mybir.AxisListType.X
                )
                nc.vector.reciprocal(out=dsum[:], in_=dsum[:])
                sa0 = psum("a")
                sa1 = psum("a")
                if lp == 0:
                    # precise bf16 p@v over key blocks 0-1 (rest masked to zero)
                    wtb = []
                    for st in range(2):
                        tp = psum("b", (P, P), BF16)
                        nc.tensor.transpose(
                            tp[:], p_t[:, st * P : (st + 1) * P], id_t[:]
                        )
                        wt = wtl.tile([P, P], BF16, tag="wtb")
                        nc.vector.tensor_copy(wt[:], tp[:])
                        wtb.append(wt)
                    for st in range(2):
                        nc.tensor.matmul(
                            sa0[:], wtb[st][:], v_bf[:, st, 0:512],
                            start=(st == 0), stop=(st == 1),
                        )
                        nc.tensor.matmul(
                            sa1[:], wtb[st][:], v_bf[:, st, 512:1024],
                            start=(st == 0), stop=(st == 1),
                        )
                else:
                    wtp = []
                    for sp in range(nst // 2):
                        wt = wtl.tile([P, 2, P], E4, tag="wt")
                        for u in range(2):
                            st = 2 * sp + u
                            tp = psum("b", (P, P), BF16)
                            nc.tensor.transpose(
                                tp[:], p_t[:, st * P : (st + 1) * P], id_t[:]
                            )
                            nc.vector.tensor_copy(wt[:, u], tp[:])
                        wtp.append(wt)
                    for sp in range(nst // 2):
                        nc.tensor.matmul(
                            sa0[:], wtp[sp][:], v_sb[:, 2 * sp : 2 * sp + 2, 0:512],
                            start=(sp == 0), stop=(sp == nst // 2 - 1),
                            perf_mode=DR,
                        )
                        nc.tensor.matmul(
                            sa1[:], wtp[sp][:],
                            v_sb[:, 2 * sp : 2 * sp + 2, 512:1024],
                            start=(sp == 0), stop=(sp == nst // 2 - 1),
                            perf_mode=DR,
                        )
                x_t = att.tile([P, C], F32, tag="xo")
                nc.sync.dma_start(x_t[:], x[2 * lp * P : (2 * lp + 1) * P, :])
                x2_t = att.tile([P, C], F32, tag="x2")
                # sa/den scaling on the scalar engine - DVE gates the PE here
                nc.scalar.activation(
                    out=x2_t[:, 0:512], in_=sa0[:],
                    func=mybir.ActivationFunctionType.Copy, scale=dsum[:],
                )
                nc.scalar.activation(
                    out=x2_t[:, 512:1024], in_=sa1[:],
                    func=mybir.ActivationFunctionType.Copy, scale=dsum[:],
                )
                nc.vector.tensor_add(out=x2_t[:], in0=x2_t[:], in1=x_t[:])
                nc.sync.dma_start(x2_d[lp * P : (lp + 1) * P, :], x2_t[:])

            emit_scores(0)
            emit_scores(1)
            for lp in range(2, NLOC):
                emit_scores(lp)
                emit_tail(lp - 2)
            emit_tail(NLOC - 2)
            emit_tail(NLOC - 1)

        # ============== Phase C2: LN2 + h2^T ===============================
        def emit_ln2(lt):
            x2_t = ln.tile([P, C], F32, tag="xt")
            nc.sync.dma_start(x2_t[:], x2_d[lt * P : (lt + 1) * P, :])
            h2_t = ln.tile([P, C], BF16, tag="ht")
            _layernorm(nc, ln, x2_t, h2_t, eps_t)
            for c in range(NCT):
                tp = psum("b", (P, P), BF16)
                nc.tensor.transpose(tp[:], h2_t[:, c * P : (c + 1) * P], id_t[:])
                nc.scalar.activation(
                    out=h2T[:, c, lt * P : (lt + 1) * P],
                    in_=tp[:],
                    func=mybir.ActivationFunctionType.Copy,
                    scale=1.0,
                )

        for lt in range(4):
            emit_ln2(lt)

        # ================= Phase D: FFN (fp8 DoubleRow) ====================
        with ExitStack() as sD:
            big_d = sD.enter_context(tc.tile_pool(name="bigd", bufs=1))
            ffw = sD.enter_context(tc.tile_pool(name="ffw", bufs=3))
            aT = [
                big_d.tile([P, NF, 512], E4, name=f"aT{i}") for i in range(2)
            ]

            def emit_aT(tch):
                # a^T half = relu(W1^T h2^T + b1) for 512 local tokens
                for ft in range(NF):
                    w1_t = ffw.tile(
                        [P, NCT, P], E4, tag="w1t", bufs=4, name=f"w1t{tch}_{ft}"
                    )
                    nc.sync.dma_start(w1_t[:], w1[ft])
                    acc = psum("a")
                    for k in range(NCT // 2):
                        nc.tensor.matmul(
                            acc[:],
                            w1_t[:, 2 * k : 2 * k + 2],
                            h2T[:, 2 * k : 2 * k + 2, tch * 512 : (tch + 1) * 512],
                            start=(k == 0),
                            stop=(k == NCT // 2 - 1),
                            perf_mode=DR,
                        )
                    nc.scalar.activation(
                        out=aT[tch][:, ft, :],
                        in_=acc[:],
                        func=mybir.ActivationFunctionType.Relu,
                        bias=b1_t[:, ft : ft + 1],
                        scale=1.0,
                    )

            def mk_grp():
                return [
                    [
                        psum(
                            ("a" if tb < 1 else "b" if tb < 2 else "c"),
                            bufs=(2 if tb < 2 else 4),
                        )
                        for cc in range(2)
                    ]
                    for tb in range(4)
                ]

            def ff_pass(tbh, dig, grp):
                for ft in range(NF // 2):
                    w2_t = ffw.tile(
                        [P, 2, C], E4, tag="w2t", bufs=4,
                        name=f"w2t{tbh}_{dig}_{ft}",
                    )
                    nc.sync.dma_start(
                        w2_t[:], w2_r[:, dig, 2 * ft : 2 * ft + 2, :]
                    )
                    for tb in range(4):
                        for cc in range(2):
                            nc.tensor.matmul(
                                grp[tb][cc][:],
                                aT[tbh][:, 2 * ft : 2 * ft + 2,
                                        tb * P : (tb + 1) * P],
                                w2_t[:, :, cc * 512 : (cc + 1) * 512],
                                start=(ft == 0),
                                stop=(ft == NF // 2 - 1),
                                perf_mode=DR,
                            )

            def emit_ff_hi(tbh):
                # hi-digit pass, evacuated (+b2) into o_t
                grp = mk_grp()
                ff_pass(tbh, 0, grp)
                o_ts = []
                for tb in range(4):
                    o_t = ffw.tile(
                        [P, C], F32, tag="ot", bufs=5, name=f"ot{tbh}_{tb}"
                    )
                    for cc in range(2):
                        nc.vector.tensor_add(
                            out=o_t[:, cc * 512 : (cc + 1) * 512],
                            in0=grp[tb][cc][:],
                            in1=b2_b[:, cc * 512 : (cc + 1) * 512],
                        )
                    o_ts.append(o_t)
                return o_ts

            def emit_ff_lo(tbh, o_ts):
                # lo-digit pass (weights pre-scaled x256), combined at 1/256
                grp = mk_grp()
                ff_pass(tbh, 1, grp)
                for tb in range(4):
                    lt = tbh * 4 + tb
                    x2_t = ffw.tile([P, C], F32, tag="x2r", name=f"x2r{tbh}_{tb}")
                    nc.sync.dma_start(x2_t[:], x2_d[lt * P : (lt + 1) * P, :])
                    o_t = o_ts[tb]
                    lo_t = ffw.tile([P, C], F32, tag="lot", name=f"lot{tbh}_{tb}")
                    for cc in range(2):
                        nc.scalar.activation(
                            out=lo_t[:, cc * 512 : (cc + 1) * 512],
                            in_=grp[tb][cc][:],
                            func=mybir.ActivationFunctionType.Copy,
                            scale=float(1.0 / 256.0),
                        )
                    nc.vector.tensor_add(out=o_t[:], in0=o_t[:], in1=lo_t[:])
                    nc.vector.tensor_add(out=o_t[:], in0=o_t[:], in1=x2_t[:])
                    nc.sync.dma_start(out[lt * P : (lt + 1) * P, :], o_t[:])

            # aT(0) needs only LN2 of tiles 0-3; LN2 of 4-7 overlaps its
            # matmuls. aT(1) overlaps the hi(0) PSUM evacuation; lo(0) still
            # reads aT[0], so the two halves use separate aT buffers.
            emit_aT(0)
            for lt in range(4, NLOC):
                emit_ln2(lt)
            o0 = emit_ff_hi(0)
            emit_aT(1)
            emit_ff_lo(0, o0)
            o1 = emit_ff_hi(1)
            emit_ff_lo(1, o1)

    _split_multi_waits(nc)
    return nc


_NC_CACHE = None


def _get_nc():
    global _NC_CACHE
    if _NC_CACHE is None:
        _NC_CACHE = build_nc()
    return _NC_CACHE


def _prep_host(inputs):
    """Fold LN gains/biases into weights; build per-core input maps."""
    x = np.asarray(inputs["x"], dtype=np.float32)
    Wk = np.asarray(inputs["Wk"], dtype=np.float32)
    Wq = np.asarray(inputs["Wq"], dtype=np.float32)
    Wv = np.asarray(inputs["Wv"], dtype=np.float32)
    W1 = np.asarray(inputs["W1"], dtype=np.float32)
    b1 = np.asarray(inputs["b1"], dtype=np.float32)
    W2 = np.asarray(inputs["W2"], dtype=np.float32)
    b2 = np.asarray(inputs["b2"], dtype=np.float32)
    g1 = np.asarray(inputs["g1"], dtype=np.float32)
    be1 = np.asarray(inputs["be1"], dtype=np.float32)
    g2 = np.asarray(inputs["g2"], dtype=np.float32)
    be2 = np.asarray(inputs["be2"], dtype=np.float32)

    f8 = ml_dtypes.float8_e4m3
    bf = ml_dtypes.bfloat16
    wq_f = np.ascontiguousarray((g1[:, None] * Wq).astype(f8))
    wk_f = np.ascontiguousarray((g1[:, None] * Wk).astype(f8))
    wv_full = g1[:, None] * Wv
    wv_f = np.ascontiguousarray(wv_full.astype(f8))
    wv16_f = np.ascontiguousarray(wv_full.astype(bf))
    qb = be1 @ Wq
    kb = be1 @ Wk
    vb = be1 @ Wv
    w1_full = (g2[:, None] * W1).astype(f8)
    w1_f = np.ascontiguousarray(
        w1_full.reshape(NCT, P, NF, P).transpose(2, 1, 0, 3)
    )
    w2_hi = W2.astype(f8)
    w2_lo = (256.0 * (W2 - w2_hi.astype(np.float32))).astype(f8)
    w2_f8 = np.ascontiguousarray(np.stack([w2_hi, w2_lo], axis=0))
    b1_f = b1 + be2 @ W1

    ident = np.eye(P, dtype=ml_dtypes.bfloat16)

    # per-half masks against the PERMUTED key order: for own block lp (global
    # g), only the last processed 512-chunk needs masking; each 128-block in
    # it is fully allowed (gb < g), fully masked (gb > g), or diagonal.
    rr = np.arange(P)[:, None]
    sub_diag = np.where(np.arange(P)[None, :] <= rr, 0.0, NEG).astype(np.float32)
    masks_h = []
    for half in range(2):
        perm = PERMS[half]
        mk = np.empty((NLOC, P, 512), dtype=np.float32)
        for m_i, g in enumerate(L_HALF[half]):
            nch = NCHUNKS[m_i]
            for pos in range(4 * (nch - 1)):
                assert perm[pos] < g, (half, m_i, pos)
            for pos in range(4 * nch, NT):
                assert perm[pos] > g, (half, m_i, pos)
            base = 4 * (nch - 1)
            for jb in range(4):
                gb = perm[base + jb]
                if gb < g:
                    mk[m_i, :, jb * P : (jb + 1) * P] = 0.0
                elif gb > g:
                    mk[m_i, :, jb * P : (jb + 1) * P] = NEG
                else:
                    mk[m_i, :, jb * P : (jb + 1) * P] = sub_diag
        masks_h.append(mk)

    shared = {
        "wq": wq_f, "wk": wk_f, "wv": wv_f, "wv16": wv16_f,
        "w1": w1_f, "w2": w2_f8,
        "qb": qb, "kb": kb, "vb": vb, "b1": b1_f, "b2": b2,
        "ident": ident,
    }
    in_maps = []
    for core in range(8):
        b, half = core // 2, core % 2
        rows = np.concatenate(
            [np.arange(i * P, (i + 1) * P) for i in PERMS[half]]
        )
        xp = np.ascontiguousarray(x[b][rows])
        m = dict(shared)
        m["x"] = xp
        m["x_bf"] = xp.astype(bf)
        m["masks"] = masks_h[half]
        in_maps.append(m)
    return in_maps


def _scatter_out(results):
    out = np.empty((B, T, C), dtype=np.float32)
    for core in range(8):
        b, half = core // 2, core % 2
        L = L_HALF[half]
        o = results[core]["out"]
        for ppos, i in enumerate(L):
            out[b, i * P : (i + 1) * P, :] = o[ppos * P : (ppos + 1) * P, :]
    return out


def run(inputs, trace=False, **kw):
    nc = _get_nc()
    in_maps = _prep_host(inputs)
    res = run_bass_kernel_spmd(
        nc, in_maps, core_ids=list(range(8)), trace=trace, **kw
    )
    return _scatter_out(res.results), res


def kernel(**inputs) -> np.ndarray:
    out, _ = run(inputs, trace=False)
    return out


# revision 44
# speedup vs baseline: 1.2946x; 1.0647x over previous
"""Trainium2 Bass kernel for nn_Block (dense transformer block).

  out = x + FFN(LN2(x + Attn(LN1(x))))   with causal single-head attention,
  B=4, T=2048, C=H=1024, FF=4096, fp32 reference.

Distribution: 8 NeuronCores = (batch b in 0..3) x (query-half in 0..1).
Each core handles one batch element's keys/values and HALF its query rows
(causally balanced interleaved block split), plus LN2+FFN+residual for those
rows.  No collectives; the per-core programs are IDENTICAL (SPMD) - all
per-core variation is input data.

All matmul OPERANDS are fp8e4m3 driven in DoubleRow perf mode (2 fp8
weights per PE cell, K=256 contraction per matmul); every accumulation is
f32 in PSUM, and LN stats / softmax denominators / residual adds are f32.
LN gains/biases are folded into the weight matrices host-side.
"""

import sys
import types

import numpy as np

# ---------------------------------------------------------------------------
# antenv.axon_hooks shim: the image's antenv lacks this module and
# run_bass_kernel_spmd imports it under axon when trace=True.
import antenv

if "antenv.axon_hooks" not in sys.modules:
    _mod = types.ModuleType("antenv.axon_hooks")
    _mod._hook = None
    _mod.set_axon_ntff_profile_hook = lambda h: setattr(_mod, "_hook", h)
    _mod.get_axon_ntff_profile_hook = lambda: _mod._hook
    sys.modules["antenv.axon_hooks"] = _mod
    antenv.axon_hooks = _mod

import ml_dtypes

import concourse.bass as bass
import concourse.mybir as mybir
import concourse.tile as tile
from concourse.bass_utils import run_bass_kernel_spmd

F32 = mybir.dt.float32
BF16 = mybir.dt.bfloat16
E4 = mybir.dt.float8e4
DR = mybir.MatmulPerfMode.DoubleRow

B, T, C = 4, 2048, 1024
H, FF = 1024, 4096
P = 128
NT = T // P  # 16 token blocks per batch element
NCT = C // P  # 8 contraction tiles
NH = H // P  # 8 head-dim tiles
NF = FF // P  # 32 ff tiles
TOWN = T // 2  # own tokens per core (1024)
NLOC = TOWN // P  # 8 own blocks
EPS = 1e-5
SCALE = 1.0 / np.sqrt(np.float32(C))  # 1/32
NEG = -1.0e30

# Causally balanced query-block assignment (sum of chunk counts = 20 each).
L_HALF = [
    [0, 2, 4, 6, 9, 11, 13, 15],
    [1, 3, 5, 7, 8, 10, 12, 14],
]
# ceil((i+1)/4) for i in L_HALF[h] - same sequence for both halves.
NCHUNKS = [1, 1, 2, 2, 3, 3, 4, 4]
# Per-core BLOCK PERMUTATION of x: own blocks sit at even positions, the
# other half's at odd positions.  The program then addresses own tokens at
# fixed (core-independent) offsets; all per-core variation stays in data.
PERMS = [
    [L_HALF[h][m // 2] if m % 2 == 0 else L_HALF[1 - h][m // 2] for m in range(NT)]
    for h in range(2)
]


def _split_multi_waits(nc):
    """walrus here accepts at most ONE sync-wait per instruction; hoist
    extras onto injected same-engine NoOps."""
    for fn in nc.m.functions:
        for blk in fn.blocks:
            new_insts = []
            changed = False
            for inst in blk.instructions:
                si = getattr(inst, "sync_info", None)
                ow = list(si.on_wait) if si is not None and si.on_wait else []
                if len(ow) > 1:
                    for i, cond in enumerate(ow[:-1]):
                        new_insts.append(
                            mybir.InstNoOp(
                                name=f"{inst.name}-wn{i}",
                                engine=inst.engine,
                                ins=[],
                                outs=[],
                                sync_info=mybir.SyncInfo(
                                    on_wait=[cond], on_update=[]
                                ),
                            )
                        )
                    inst.sync_info = mybir.SyncInfo(
                        on_wait=[ow[-1]], on_update=list(si.on_update or [])
                    )
                    changed = True
                new_insts.append(inst)
            if changed:
                blk.instructions = new_insts


def _layernorm(nc, pool, x_t, h_t, eps_t):
    """h_t = (x_t - mean) * rsqrt(var + eps), stats along the free dim."""
    xg = x_t[:].rearrange("p (s f) -> p s f", f=512)
    stats = pool.tile([P, 2, nc.vector.BN_STATS_DIM], F32, tag="ln_stats")
    for sg in range(2):
        nc.vector.bn_stats(out=stats[:, sg], in_=xg[:, sg])
    mv = pool.tile([P, nc.vector.BN_AGGR_DIM], F32, tag="ln_mv")
    nc.vector.bn_aggr(out=mv[:], in_=stats[:])
    rstd = pool.tile([P, 1], F32, tag="ln_rstd")
    nc.scalar.activation(
        out=rstd[:],
        in_=mv[:, 1:2],
        func=mybir.ActivationFunctionType.Sqrt,
        bias=eps_t[:],
        scale=1.0,
    )
    nc.vector.reciprocal(out=rstd[:], in_=rstd[:])
    nc.vector.tensor_scalar(
        out=h_t[:],
        in0=x_t[:],
        scalar1=mv[:, 0:1],
        scalar2=rstd[:],
        op0=mybir.AluOpType.subtract,
        op1=mybir.AluOpType.mult,
    )


def build_nc():
    from contextlib import ExitStack

    nc = bass.Bass()

    x = nc.declare_dram_parameter("x", [T, C], F32, isOutput=False)
    x_bf = nc.declare_dram_parameter("x_bf", [T, C], BF16, isOutput=False)
    wq = nc.declare_dram_parameter("wq", [C, H], E4, isOutput=False)
    wk = nc.declare_dram_parameter("wk", [C, H], E4, isOutput=False)
    wv = nc.declare_dram_parameter("wv", [C, H], E4, isOutput=False)
    wv16 = nc.declare_dram_parameter("wv16", [C, H], BF16, isOutput=False)
    w1 = nc.declare_dram_parameter("w1", [NF, P, NCT, P], E4, isOutput=False)
    # w2[0] = e4m3(W2), w2[1] = e4m3(256*(W2 - w2[0])) - two-digit fp8
    w2 = nc.declare_dram_parameter("w2", [2, FF, C], E4, isOutput=False)
    qb = nc.declare_dram_parameter("qb", [H], F32, isOutput=False)
    kb = nc.declare_dram_parameter("kb", [H], F32, isOutput=False)
    vb = nc.declare_dram_parameter("vb", [H], F32, isOutput=False)
    b1 = nc.declare_dram_parameter("b1", [FF], F32, isOutput=False)
    b2 = nc.declare_dram_parameter("b2", [C], F32, isOutput=False)
    ident = nc.declare_dram_parameter("ident", [P, P], BF16, isOutput=False)
    masks = nc.declare_dram_parameter("masks", [NLOC, P, 512], F32, isOutput=False)
    out = nc.declare_dram_parameter("out", [TOWN, C], F32, isOutput=True)

    x2_d = nc.dram_tensor("x2_d", [TOWN, C], F32)

    wq_r = wq.rearrange("(ko p) h -> p ko h", p=P)
    wk_r = wk.rearrange("(ko p) h -> p ko h", p=P)
    wv_r = wv.rearrange("(ko p) h -> p ko h", p=P)
    wv16_r = wv16.rearrange("(ko p) h -> p ko h", p=P)
    w2_r = w2.rearrange("t (ko p) c -> p t ko c", p=P)

    with tile.TileContext(nc) as tc, ExitStack() as top:
        cn = top.enter_context(tc.tile_pool(name="cn", bufs=1))
        ps = top.enter_context(tc.tile_pool(name="ps", bufs=1, space="PSUM"))
        ln = top.enter_context(tc.tile_pool(name="ln", bufs=4))
        # big resident tensors spanning several phases
        res = top.enter_context(tc.tile_pool(name="res", bufs=1))

        # critical-path-first: the very first LN tile and the transpose
        # identity go ahead of the bulk constant loads.
        x0_t = ln.tile([P, C], BF16, tag="xb", bufs=6, name="x0t")
        nc.sync.dma_start(x0_t[:], x_bf[0:P, :])
        id_t = cn.tile([P, P], BF16)
        nc.sync.dma_start(id_t[:], ident[:])
        xpre = [x0_t]
        for stp in range(1, 4):
            t = ln.tile([P, C], BF16, tag="xb", bufs=6, name=f"xp{stp}")
            nc.sync.dma_start(t[:], x_bf[stp * P : (stp + 1) * P, :])
            xpre.append(t)
        # ---- constants
        qb_t = cn.tile([P, NH], F32)
        nc.sync.dma_start(qb_t[:], qb.rearrange("(m p) -> p m", p=P))
        kb_t = cn.tile([P, NH], F32)
        nc.sync.dma_start(kb_t[:], kb.rearrange("(m p) -> p m", p=P))
        b1_t = cn.tile([P, NF], F32)
        nc.sync.dma_start(b1_t[:], b1.rearrange("(m p) -> p m", p=P))
        vb_b = cn.tile([P, H], F32)
        nc.sync.dma_start(vb_b[:], vb[None, :].partition_broadcast(P))
        b2_b = cn.tile([P, C], F32)
        nc.sync.dma_start(b2_b[:], b2[None, :].partition_broadcast(P))
        eps_t = cn.tile([P, 1], F32)
        nc.vector.memset(eps_t, EPS)

        _ctr = [0]

        def psum(tag, shape=(P, 512), dt=F32, bufs=2):
            _ctr[0] += 1
            return ps.tile(list(shape), dt, tag=tag, bufs=bufs, name=f"ps{_ctr[0]}")

        qTo = res.tile([P, NH, TOWN], E4)   # q^T own tokens (local order)
        kT = res.tile([P, NH, T], E4)       # k^T all keys
        v_sb = res.tile([P, NT, H], E4)     # v token-major, all keys
        h2T = res.tile([P, NCT, TOWN], E4)  # LN2 output transposed
        # precise (bf16) head-block path: first own block attends few keys, so
        # fp8 noise passes straight through - keep that slice in bf16.
        qTb = res.tile([P, NH, P], BF16)    # q^T for local block 0
        kTb = res.tile([P, NH, 512], BF16)  # k^T for key chunk 0
        v_bf = res.tile([P, 2, H], BF16)    # v for key blocks 0-1 (bf16 matmul)

        # ===== Phase B: LN1 over permuted blocks -> hT/hto; v; q^T; k^T ====
        # x arrives block-PERMUTED per core (own blocks at even positions),
        # so one LN pass feeds hT (all keys) and hto (own tokens, gathered by
        # the scalar engine) - no duplicated LN/transposes, and the v matmuls
        # give the PE work from the first tile on.
        with ExitStack() as sB:
            big_b = sB.enter_context(tc.tile_pool(name="bigb", bufs=1))
            wvp = sB.enter_context(tc.tile_pool(name="wvp", bufs=1))
            hT = big_b.tile([P, NCT, T], E4)
            hto = big_b.tile([P, NCT, TOWN], E4)
            hT_bf = big_b.tile([P, NCT, 256], BF16)
            # x tiles must BEAT the 5MB of weight loads to the DMA queues -
            # only wv is needed early; the rest are staggered into the loop.
            wv_t = wvp.tile([P, NCT, H], E4)
            nc.sync.dma_start(wv_t[:], wv_r[:])
            wq_t = wvp.tile([P, NCT, H], E4)
            wk_t = wvp.tile([P, NCT, H], E4)
            wv16_t = wvp.tile([P, NCT, H], BF16)
            # LN per permuted token tile; v row-block right after its tile
            for st in range(NT):
                if st < 4:
                    x_t = xpre[st]
                else:
                    x_t = ln.tile([P, C], BF16, tag="xb", bufs=6)
                    nc.sync.dma_start(x_t[:], x_bf[st * P : (st + 1) * P, :])
                if st == 1:
                    nc.sync.dma_start(wv16_t[:], wv16_r[:])
                elif st == 5:
                    nc.sync.dma_start(wq_t[:], wq_r[:])
                elif st == 9:
                    nc.sync.dma_start(wk_t[:], wk_r[:])
                h_t = ln.tile([P, C], BF16, tag="ht")
                _layernorm(nc, ln, x_t, h_t, eps_t)
                for c in range(NCT):
                    tp = psum("b", (P, P), BF16)
                    nc.tensor.transpose(tp[:], h_t[:, c * P : (c + 1) * P], id_t[:])
                    # balance the PSUM evacuations across DVE and ACT
                    if st % 2 == 1 and c >= 6:
                        nc.scalar.activation(
                            out=hT[:, c, st * P : (st + 1) * P],
                            in_=tp[:],
                            func=mybir.ActivationFunctionType.Copy,
                            scale=1.0,
                        )
                    else:
                        nc.vector.tensor_copy(
                            hT[:, c, st * P : (st + 1) * P], tp[:]
                        )
                    if st % 2 == 0:
                        nc.scalar.activation(
                            out=hto[:, c, (st // 2) * P : (st // 2 + 1) * P],
                            in_=tp[:],
                            func=mybir.ActivationFunctionType.Copy,
                            scale=1.0,
                        )
                    if st < 2:
                        nc.vector.tensor_copy(
                            hT_bf[:, c, st * P : (st + 1) * P], tp[:]
                        )
                accs = [psum("a") for _ in range(2)]
                for k in range(NCT // 2):
                    for hh in range(2):
                        nc.tensor.matmul(
                            accs[hh][:],
                            hT[:, 2 * k : 2 * k + 2, st * P : (st + 1) * P],
                            wv_t[:, 2 * k : 2 * k + 2, hh * 512 : (hh + 1) * 512],
                            start=(k == 0),
                            stop=(k == NCT // 2 - 1),
                            perf_mode=DR,
                        )
                # vb is folded into the host-side residual x (softmax weights
                # sum to 1), so the v evacuation is a pure cast - on ACT.
                for hh in range(2):
                    nc.scalar.activation(
                        out=v_sb[:, st, hh * 512 : (hh + 1) * 512],
                        in_=accs[hh][:],
                        func=mybir.ActivationFunctionType.Copy,
                        scale=1.0,
                    )
                if st == 3:
                    # precise bf16 v for key blocks 0-1 - emitted here so the
                    # PE has work while the LN stream (DVE-bound) warms up
                    for blk in range(2):
                        accs = [psum("a") for _ in range(2)]
                        for k in range(NCT):
                            for hh in range(2):
                                nc.tensor.matmul(
                                    accs[hh][:],
                                    hT_bf[:, k, blk * P : (blk + 1) * P],
                                    wv16_t[:, k, hh * 512 : (hh + 1) * 512],
                                    start=(k == 0),
                                    stop=(k == NCT - 1),
                                )
                        for hh in range(2):
                            nc.scalar.activation(
                                out=v_bf[:, blk, hh * 512 : (hh + 1) * 512],
                                in_=accs[hh][:],
                                func=mybir.ActivationFunctionType.Copy,
                                scale=1.0,
                            )
            # q^T (own tokens, from hto)
            for m in range(NH):
                accs = [psum("a") for _ in range(2)]
                for k in range(NCT // 2):
                    for g in range(2):
                        nc.tensor.matmul(
                            accs[g][:],
                            wq_t[:, 2 * k : 2 * k + 2, m * P : (m + 1) * P],
                            hto[:, 2 * k : 2 * k + 2, g * 512 : (g + 1) * 512],
                            start=(k == 0),
                            stop=(k == NCT // 2 - 1),
                            perf_mode=DR,
                        )
                for g in range(2):
                    nc.vector.tensor_scalar_add(
                        out=qTo[:, m, g * 512 : (g + 1) * 512],
                        in0=accs[g][:],
                        scalar1=qb_t[:, m : m + 1],
                    )
                nc.vector.tensor_scalar_add(
                    out=qTb[:, m, :],
                    in0=accs[0][:, 0:P],
                    scalar1=qb_t[:, m : m + 1],
                )
            # k^T (all keys)
            for m in range(NH):
                accs = [psum("c", bufs=4) for _ in range(4)]
                for k in range(NCT // 2):
                    for ch in range(4):
                        nc.tensor.matmul(
                            accs[ch][:],
                            wk_t[:, 2 * k : 2 * k + 2, m * P : (m + 1) * P],
                            hT[:, 2 * k : 2 * k + 2, ch * 512 : (ch + 1) * 512],
                            start=(k == 0),
                            stop=(k == NCT // 2 - 1),
                            perf_mode=DR,
                        )
                for ch in range(4):
                    nc.vector.tensor_scalar_add(
                        out=kT[:, m, ch * 512 : (ch + 1) * 512],
                        in0=accs[ch][:],
                        scalar1=kb_t[:, m : m + 1],
                    )
                nc.vector.tensor_scalar_add(
                    out=kTb[:, m, :],
                    in0=accs[0][:],
                    scalar1=kb_t[:, m : m + 1],
                )

        # ============== Phase C: attention (software-pipelined) ============
        with ExitStack() as sC:
            att = sC.enter_context(tc.tile_pool(name="att", bufs=2))
            wtl = sC.enter_context(tc.tile_pool(name="wtl", bufs=16))
            state = {}

            def emit_scores(lp):
                nch = NCHUNKS[lp]
                mask_t = att.tile([P, 512], F32, tag="mask")
                nc.sync.dma_start(mask_t[:], masks[lp])
                p_t = att.tile([P, T], BF16, tag="pt", bufs=3)
                den = att.tile([P, 4], F32, tag="den")
                scs = [psum("c", bufs=4) for _ in range(nch)]
                if lp == 0:
                    # precise bf16 scores for the head block
                    for m in range(NH):
                        nc.tensor.matmul(
                            scs[0][:],
                            qTb[:, m, :],
                            kTb[:, m, :],
                            start=(m == 0),
                            stop=(m == NH - 1),
                        )
                else:
                    for m in range(NH // 2):
                        for j in range(nch):
                            nc.tensor.matmul(
                                scs[j][:],
                                qTo[:, 2 * m : 2 * m + 2, lp * P : (lp + 1) * P],
                                kT[:, 2 * m : 2 * m + 2, j * 512 : (j + 1) * 512],
                                start=(m == 0),
                                stop=(m == NH // 2 - 1),
                                perf_mode=DR,
                            )
                for j in range(nch):
                    if j == nch - 1:
                        nc.vector.tensor_add(
                            out=scs[j][:], in0=scs[j][:], in1=mask_t[:]
                        )
                    nc.scalar.activation(
                        out=p_t[:, j * 512 : (j + 1) * 512],
                        in_=scs[j][:],
                        func=mybir.ActivationFunctionType.Exp,
                        scale=float(SCALE),
                        accum_out=den[:, j : j + 1],
                    )
                state[lp] = (p_t, den)

            def emit_tail(lp):
                nch = NCHUNKS[lp]
                nst = 4 * nch
                p_t, den = state.pop(lp)
                dsum = att.tile([P, 1], F32, tag="dsum")
                nc.vector.reduce_sum(
                    out=dsum[:], in_=den[:, :nch], axis=mybir.AxisListType.X
                )
                nc.vector.reciprocal(out=dsum[:], in_=dsum[:])
                sa0 = psum("a")
                sa1 = psum("a")
                if lp == 0:
                    # precise bf16 p@v over key blocks 0-1 (rest masked to zero)
                    wtb = []
                    for st in range(2):
                        tp = psum("b", (P, P), BF16)
                        nc.tensor.transpose(
                            tp[:], p_t[:, st * P : (st + 1) * P], id_t[:]
                        )
                        wt = wtl.tile([P, P], BF16, tag="wtb")
                        nc.vector.tensor_copy(wt[:], tp[:])
                        wtb.append(wt)
                    for st in range(2):
                        nc.tensor.matmul(
                            sa0[:], wtb[st][:], v_bf[:, st, 0:512],
                            start=(st == 0), stop=(st == 1),
                        )
                        nc.tensor.matmul(
                            sa1[:], wtb[st][:], v_bf[:, st, 512:1024],
                            start=(st == 0), stop=(st == 1),
                        )
                else:
                    wtp = []
                    for sp in range(nst // 2):
                        wt = wtl.tile([P, 2, P], E4, tag="wt")
                        for u in range(2):
                            st = 2 * sp + u
                            tp = psum("b", (P, P), BF16)
                            nc.tensor.transpose(
                                tp[:], p_t[:, st * P : (st + 1) * P], id_t[:]
                            )
                            nc.vector.tensor_copy(wt[:, u], tp[:])
                        wtp.append(wt)
                    for sp in range(nst // 2):
                        nc.tensor.matmul(
                            sa0[:], wtp[sp][:], v_sb[:, 2 * sp : 2 * sp + 2, 0:512],
                            start=(sp == 0), stop=(sp == nst // 2 - 1),
                            perf_mode=DR,
                        )
                        nc.tensor.matmul(
                            sa1[:], wtp[sp][:],
                            v_sb[:, 2 * sp : 2 * sp + 2, 512:1024],
                            start=(sp == 0), stop=(sp == nst // 2 - 1),
                            perf_mode=DR,
                        )
                x_t = att.tile([P, C], F32, tag="xo")
                nc.sync.dma_start(x_t[:], x[2 * lp * P : (2 * lp + 1) * P, :])
                x2_t = att.tile([P, C], F32, tag="x2")
                # sa/den scaling on the scalar engine - DVE gates the PE here
                nc.scalar.activation(
                    out=x2_t[:, 0:512], in_=sa0[:],
                    func=mybir.ActivationFunctionType.Copy, scale=dsum[:],
                )
                nc.scalar.activation(
                    out=x2_t[:, 512:1024], in_=sa1[:],
                    func=mybir.ActivationFunctionType.Copy, scale=dsum[:],
                )
                nc.vector.tensor_add(out=x2_t[:], in0=x2_t[:], in1=x_t[:])
                nc.sync.dma_start(x2_d[lp * P : (lp + 1) * P, :], x2_t[:])

            emit_scores(0)
            emit_scores(1)
            for lp in range(2, NLOC):
                emit_scores(lp)
                emit_tail(lp - 2)
            emit_tail(NLOC - 2)
            emit_tail(NLOC - 1)

        # ============== Phase C2: LN2 + h2^T ===============================
        def emit_ln2(lt):
            x2_t = ln.tile([P, C], F32, tag="xt")
            nc.sync.dma_start(x2_t[:], x2_d[lt * P : (lt + 1) * P, :])
            h2_t = ln.tile([P, C], BF16, tag="ht")
            _layernorm(nc, ln, x2_t, h2_t, eps_t)
            for c in range(NCT):
                tp = psum("b", (P, P), BF16)
                nc.tensor.transpose(tp[:], h2_t[:, c * P : (c + 1) * P], id_t[:])
                nc.scalar.activation(
                    out=h2T[:, c, lt * P : (lt + 1) * P],
                    in_=tp[:],
                    func=mybir.ActivationFunctionType.Copy,
                    scale=1.0,
                )

        for lt in range(4):
            emit_ln2(lt)

        # ================= Phase D: FFN (fp8 DoubleRow) ====================
        with ExitStack() as sD:
            big_d = sD.enter_context(tc.tile_pool(name="bigd", bufs=1))
            ffw = sD.enter_context(tc.tile_pool(name="ffw", bufs=3))
            aT = [
                big_d.tile([P, NF, 512], E4, name=f"aT{i}") for i in range(2)
            ]

            def emit_aT(tch):
                # a^T half = relu(W1^T h2^T + b1) for 512 local tokens
                for ft in range(NF):
                    w1_t = ffw.tile(
                        [P, NCT, P], E4, tag="w1t", bufs=6, name=f"w1t{tch}_{ft}"
                    )
                    nc.sync.dma_start(w1_t[:], w1[ft])
                    acc = psum("a")
                    for k in range(NCT // 2):
                        nc.tensor.matmul(
                            acc[:],
                            w1_t[:, 2 * k : 2 * k + 2],
                            h2T[:, 2 * k : 2 * k + 2, tch * 512 : (tch + 1) * 512],
                            start=(k == 0),
                            stop=(k == NCT // 2 - 1),
                            perf_mode=DR,
                        )
                    nc.scalar.activation(
                        out=aT[tch][:, ft, :],
                        in_=acc[:],
                        func=mybir.ActivationFunctionType.Relu,
                        bias=b1_t[:, ft : ft + 1],
                        scale=1.0,
                    )

            def mk_grp():
                return [
                    [
                        psum(
                            ("a" if tb < 1 else "b" if tb < 2 else "c"),
                            bufs=(2 if tb < 2 else 4),
                        )
                        for cc in range(2)
                    ]
                    for tb in range(4)
                ]

            def ff_pass(tbh, dig, grp):
                for ft in range(NF // 2):
                    w2_t = ffw.tile(
                        [P, 2, C], E4, tag="w2t", bufs=6,
                        name=f"w2t{tbh}_{dig}_{ft}",
                    )
                    nc.sync.dma_start(
                        w2_t[:], w2_r[:, dig, 2 * ft : 2 * ft + 2, :]
                    )
                    for tb in range(4):
                        for cc in range(2):
                            nc.tensor.matmul(
                                grp[tb][cc][:],
                                aT[tbh][:, 2 * ft : 2 * ft + 2,
                                        tb * P : (tb + 1) * P],
                                w2_t[:, :, cc * 512 : (cc + 1) * 512],
                                start=(ft == 0),
                                stop=(ft == NF // 2 - 1),
                                perf_mode=DR,
                            )

            def emit_ff_hi(tbh):
                # hi-digit pass, evacuated (+b2) into o_t
                grp = mk_grp()
                ff_pass(tbh, 0, grp)
                o_ts = []
                for tb in range(4):
                    o_t = ffw.tile(
                        [P, C], F32, tag="ot", bufs=5, name=f"ot{tbh}_{tb}"
                    )
                    for cc in range(2):
                        nc.vector.tensor_add(
                            out=o_t[:, cc * 512 : (cc + 1) * 512],
                            in0=grp[tb][cc][:],
                            in1=b2_b[:, cc * 512 : (cc + 1) * 512],
                        )
                    o_ts.append(o_t)
                return o_ts

            def emit_ff_lo(tbh, o_ts):
                # lo-digit pass (weights pre-scaled x256), combined at 1/256
                grp = mk_grp()
                ff_pass(tbh, 1, grp)
                for tb in range(4):
                    lt = tbh * 4 + tb
                    x2_t = ffw.tile([P, C], F32, tag="x2r", name=f"x2r{tbh}_{tb}")
                    nc.sync.dma_start(x2_t[:], x2_d[lt * P : (lt + 1) * P, :])
                    o_t = o_ts[tb]
                    lo_t = ffw.tile([P, C], F32, tag="lot", name=f"lot{tbh}_{tb}")
                    for cc in range(2):
                        nc.scalar.activation(
                            out=lo_t[:, cc * 512 : (cc + 1) * 512],
                            in_=grp[tb][cc][:],
                            func=mybir.ActivationFunctionType.Copy,
                            scale=float(1.0 / 256.0),
                        )
                    nc.vector.tensor_add(out=o_t[:], in0=o_t[:], in1=lo_t[:])
                    nc.vector.tensor_add(out=o_t[:], in0=o_t[:], in1=x2_t[:])
                    nc.sync.dma_start(out[lt * P : (lt + 1) * P, :], o_t[:])

            # aT(0) needs only LN2 of tiles 0-3; LN2 of 4-7 overlaps its
            # matmuls. aT(1) overlaps the hi(0) PSUM evacuation; lo(0) still
            # reads aT[0], so the two halves use separate aT buffers.
            emit_aT(0)
            for lt in range(4, NLOC):
                emit_ln2(lt)
            o0 = emit_ff_hi(0)
            emit_aT(1)
            emit_ff_lo(0, o0)
            o1 = emit_ff_hi(1)
            emit_ff_lo(1, o1)

    _split_multi_waits(nc)
    return nc


_NC_CACHE = None


def _get_nc():
    global _NC_CACHE
    if _NC_CACHE is None:
        _NC_CACHE = build_nc()
    return _NC_CACHE


def _prep_host(inputs):
    """Fold LN gains/biases into weights; build per-core input maps."""
    x = np.asarray(inputs["x"], dtype=np.float32)
    Wk = np.asarray(inputs["Wk"], dtype=np.float32)
    Wq = np.asarray(inputs["Wq"], dtype=np.float32)
    Wv = np.asarray(inputs["Wv"], dtype=np.float32)
    W1 = np.asarray(inputs["W1"], dtype=np.float32)
    b1 = np.asarray(inputs["b1"], dtype=np.float32)
    W2 = np.asarray(inputs["W2"], dtype=np.float32)
    b2 = np.asarray(inputs["b2"], dtype=np.float32)
    g1 = np.asarray(inputs["g1"], dtype=np.float32)
    be1 = np.asarray(inputs["be1"], dtype=np.float32)
    g2 = np.asarray(inputs["g2"], dtype=np.float32)
    be2 = np.asarray(inputs["be2"], dtype=np.float32)

    f8 = ml_dtypes.float8_e4m3
    bf = ml_dtypes.bfloat16
    wq_f = np.ascontiguousarray((g1[:, None] * Wq).astype(f8))
    wk_f = np.ascontiguousarray((g1[:, None] * Wk).astype(f8))
    wv_full = g1[:, None] * Wv
    wv_f = np.ascontiguousarray(wv_full.astype(f8))
    wv16_f = np.ascontiguousarray(wv_full.astype(bf))
    qb = be1 @ Wq
    kb = be1 @ Wk
    vb = be1 @ Wv
    w1_full = (g2[:, None] * W1).astype(f8)
    w1_f = np.ascontiguousarray(
        w1_full.reshape(NCT, P, NF, P).transpose(2, 1, 0, 3)
    )
    w2_hi = W2.astype(f8)
    w2_lo = (256.0 * (W2 - w2_hi.astype(np.float32))).astype(f8)
    w2_f8 = np.ascontiguousarray(np.stack([w2_hi, w2_lo], axis=0))
    b1_f = b1 + be2 @ W1

    ident = np.eye(P, dtype=ml_dtypes.bfloat16)

    # per-half masks against the PERMUTED key order: for own block lp (global
    # g), only the last processed 512-chunk needs masking; each 128-block in
    # it is fully allowed (gb < g), fully masked (gb > g), or diagonal.
    rr = np.arange(P)[:, None]
    sub_diag = np.where(np.arange(P)[None, :] <= rr, 0.0, NEG).astype(np.float32)
    masks_h = []
    for half in range(2):
        perm = PERMS[half]
        mk = np.empty((NLOC, P, 512), dtype=np.float32)
        for m_i, g in enumerate(L_HALF[half]):
            nch = NCHUNKS[m_i]
            for pos in range(4 * (nch - 1)):
                assert perm[pos] < g, (half, m_i, pos)
            for pos in range(4 * nch, NT):
                assert perm[pos] > g, (half, m_i, pos)
            base = 4 * (nch - 1)
            for jb in range(4):
                gb = perm[base + jb]
                if gb < g:
                    mk[m_i, :, jb * P : (jb + 1) * P] = 0.0
                elif gb > g:
                    mk[m_i, :, jb * P : (jb + 1) * P] = NEG
                else:
                    mk[m_i, :, jb * P : (jb + 1) * P] = sub_diag
        masks_h.append(mk)

    shared = {
        "wq": wq_f, "wk": wk_f, "wv": wv_f, "wv16": wv16_f,
        "w1": w1_f, "w2": w2_f8,
        "qb": qb, "kb": kb, "vb": vb, "b1": b1_f, "b2": b2,
        "ident": ident,
    }
    in_maps = []
    for core in range(8):
        b, half = core // 2, core % 2
        rows = np.concatenate(
            [np.arange(i * P, (i + 1) * P) for i in PERMS[half]]
        )
        xp = np.ascontiguousarray(x[b][rows])
        m = dict(shared)
        # vb folded into the residual (softmax weights sum to 1)
        m["x"] = np.ascontiguousarray(xp + vb[None, :].astype(np.float32))
        m["x_bf"] = xp.astype(bf)
        m["masks"] = masks_h[half]
        in_maps.append(m)
    return in_maps


def _scatter_out(results):
    out = np.empty((B, T, C), dtype=np.float32)
    for core in range(8):
        b, half = core // 2, core % 2
        L = L_HALF[half]
        o = results[core]["out"]
        for ppos, i in enumerate(L):
            out[b, i * P : (i + 1) * P, :] = o[ppos * P : (ppos + 1) * P, :]
    return out


def run(inputs, trace=False, **kw):
    nc = _get_nc()
    in_maps = _prep_host(inputs)
    res = run_bass_kernel_spmd(
        nc, in_maps, core_ids=list(range(8)), trace=trace, **kw
    )
    return _scatter_out(res.results), res


def kernel(**inputs) -> np.ndarray:
    out, _ = run(inputs, trace=False)
    return out


# revision 47
# speedup vs baseline: 1.3063x; 1.0090x over previous
"""Trainium2 Bass kernel for nn_Block (dense transformer block).

  out = x + FFN(LN2(x + Attn(LN1(x))))   with causal single-head attention,
  B=4, T=2048, C=H=1024, FF=4096, fp32 reference.

Distribution: 8 NeuronCores = (batch b in 0..3) x (query-half in 0..1).
Each core handles one batch element's keys/values and HALF its query rows
(causally balanced interleaved block split), plus LN2+FFN+residual for those
rows.  No collectives; the per-core programs are IDENTICAL (SPMD) - all
per-core variation is input data.

All matmul OPERANDS are fp8e4m3 driven in DoubleRow perf mode (2 fp8
weights per PE cell, K=256 contraction per matmul); every accumulation is
f32 in PSUM, and LN stats / softmax denominators / residual adds are f32.
LN gains/biases are folded into the weight matrices host-side.
"""

import sys
import types

import numpy as np

# ---------------------------------------------------------------------------
# antenv.axon_hooks shim: the image's antenv lacks this module and
# run_bass_kernel_spmd imports it under axon when trace=True.
import antenv

if "antenv.axon_hooks" not in sys.modules:
    _mod = types.ModuleType("antenv.axon_hooks")
    _mod._hook = None
    _mod.set_axon_ntff_profile_hook = lambda h: setattr(_mod, "_hook", h)
    _mod.get_axon_ntff_profile_hook = lambda: _mod._hook
    sys.modules["antenv.axon_hooks"] = _mod
    antenv.axon_hooks = _mod

import ml_dtypes

import concourse.bass as bass
import concourse.mybir as mybir
import concourse.tile as tile
from concourse.bass_utils import run_bass_kernel_spmd

F32 = mybir.dt.float32
BF16 = mybir.dt.bfloat16
E4 = mybir.dt.float8e4
DR = mybir.MatmulPerfMode.DoubleRow

B, T, C = 4, 2048, 1024
H, FF = 1024, 4096
P = 128
NT = T // P  # 16 token blocks per batch element
NCT = C // P  # 8 contraction tiles
NH = H // P  # 8 head-dim tiles
NF = FF // P  # 32 ff tiles
TOWN = T // 2  # own tokens per core (1024)
NLOC = TOWN // P  # 8 own blocks
EPS = 1e-5
SCALE = 1.0 / np.sqrt(np.float32(C))  # 1/32
NEG = -1.0e30

# Causally balanced query-block assignment (sum of chunk counts = 20 each).
L_HALF = [
    [0, 2, 4, 6, 9, 11, 13, 15],
    [1, 3, 5, 7, 8, 10, 12, 14],
]
# ceil((i+1)/4) for i in L_HALF[h] - same sequence for both halves.
NCHUNKS = [1, 1, 2, 2, 3, 3, 4, 4]
# Per-core BLOCK PERMUTATION of x: own blocks sit at even positions, the
# other half's at odd positions.  The program then addresses own tokens at
# fixed (core-independent) offsets; all per-core variation stays in data.
PERMS = [
    [L_HALF[h][m // 2] if m % 2 == 0 else L_HALF[1 - h][m // 2] for m in range(NT)]
    for h in range(2)
]


def _split_multi_waits(nc):
    """walrus here accepts at most ONE sync-wait per instruction; hoist
    extras onto injected same-engine NoOps."""
    for fn in nc.m.functions:
        for blk in fn.blocks:
            new_insts = []
            changed = False
            for inst in blk.instructions:
                si = getattr(inst, "sync_info", None)
                ow = list(si.on_wait) if si is not None and si.on_wait else []
                if len(ow) > 1:
                    for i, cond in enumerate(ow[:-1]):
                        new_insts.append(
                            mybir.InstNoOp(
                                name=f"{inst.name}-wn{i}",
                                engine=inst.engine,
                                ins=[],
                                outs=[],
                                sync_info=mybir.SyncInfo(
                                    on_wait=[cond], on_update=[]
                                ),
                            )
                        )
                    inst.sync_info = mybir.SyncInfo(
                        on_wait=[ow[-1]], on_update=list(si.on_update or [])
                    )
                    changed = True
                new_insts.append(inst)
            if changed:
                blk.instructions = new_insts


def _layernorm(nc, pool, x_t, h_t, eps_t):
    """h_t = (x_t - mean) * rsqrt(var + eps), stats along the free dim."""
    xg = x_t[:].rearrange("p (s f) -> p s f", f=512)
    stats = pool.tile([P, 2, nc.vector.BN_STATS_DIM], F32, tag="ln_stats")
    for sg in range(2):
        nc.vector.bn_stats(out=stats[:, sg], in_=xg[:, sg])
    mv = pool.tile([P, nc.vector.BN_AGGR_DIM], F32, tag="ln_mv")
    nc.vector.bn_aggr(out=mv[:], in_=stats[:])
    rstd = pool.tile([P, 1], F32, tag="ln_rstd")
    nc.scalar.activation(
        out=rstd[:],
        in_=mv[:, 1:2],
        func=mybir.ActivationFunctionType.Sqrt,
        bias=eps_t[:],
        scale=1.0,
    )
    nc.vector.reciprocal(out=rstd[:], in_=rstd[:])
    nc.vector.tensor_scalar(
        out=h_t[:],
        in0=x_t[:],
        scalar1=mv[:, 0:1],
        scalar2=rstd[:],
        op0=mybir.AluOpType.subtract,
        op1=mybir.AluOpType.mult,
    )


def build_nc():
    from contextlib import ExitStack

    nc = bass.Bass()

    x = nc.declare_dram_parameter("x", [T, C], F32, isOutput=False)
    x_bf = nc.declare_dram_parameter("x_bf", [T, C], BF16, isOutput=False)
    wq = nc.declare_dram_parameter("wq", [C, H], E4, isOutput=False)
    wk = nc.declare_dram_parameter("wk", [C, H], E4, isOutput=False)
    wv = nc.declare_dram_parameter("wv", [C, H], E4, isOutput=False)
    wv16 = nc.declare_dram_parameter("wv16", [C, H], BF16, isOutput=False)
    w1 = nc.declare_dram_parameter("w1", [NF, P, NCT, P], E4, isOutput=False)
    # w2[0] = e4m3(W2), w2[1] = e4m3(256*(W2 - w2[0])) - two-digit fp8
    w2 = nc.declare_dram_parameter("w2", [2, FF, C], E4, isOutput=False)
    qb = nc.declare_dram_parameter("qb", [H], F32, isOutput=False)
    kb = nc.declare_dram_parameter("kb", [H], F32, isOutput=False)
    vb = nc.declare_dram_parameter("vb", [H], F32, isOutput=False)
    b1 = nc.declare_dram_parameter("b1", [FF], F32, isOutput=False)
    b2 = nc.declare_dram_parameter("b2", [C], F32, isOutput=False)
    ident = nc.declare_dram_parameter("ident", [P, P], BF16, isOutput=False)
    masks = nc.declare_dram_parameter("masks", [NLOC, P, 512], F32, isOutput=False)
    out = nc.declare_dram_parameter("out", [TOWN, C], F32, isOutput=True)

    x2_d = nc.dram_tensor("x2_d", [TOWN, C], F32)

    wq_r = wq.rearrange("(ko p) h -> p ko h", p=P)
    wk_r = wk.rearrange("(ko p) h -> p ko h", p=P)
    wv_r = wv.rearrange("(ko p) h -> p ko h", p=P)
    wv16_r = wv16.rearrange("(ko p) h -> p ko h", p=P)
    w2_r = w2.rearrange("t (ko p) c -> p t ko c", p=P)

    with tile.TileContext(nc) as tc, ExitStack() as top:
        cn = top.enter_context(tc.tile_pool(name="cn", bufs=1))
        ps = top.enter_context(tc.tile_pool(name="ps", bufs=1, space="PSUM"))
        ln = top.enter_context(tc.tile_pool(name="ln", bufs=4))
        # big resident tensors spanning several phases
        res = top.enter_context(tc.tile_pool(name="res", bufs=1))

        # critical-path-first: the very first LN tile and the transpose
        # identity go ahead of the bulk constant loads.
        x0_t = ln.tile([P, C], BF16, tag="xb", bufs=6, name="x0t")
        nc.sync.dma_start(x0_t[:], x_bf[0:P, :])
        id_t = cn.tile([P, P], BF16)
        nc.sync.dma_start(id_t[:], ident[:])
        xpre = [x0_t]
        for stp in range(1, 4):
            t = ln.tile([P, C], BF16, tag="xb", bufs=6, name=f"xp{stp}")
            nc.sync.dma_start(t[:], x_bf[stp * P : (stp + 1) * P, :])
            xpre.append(t)
        # ---- constants
        qb_t = cn.tile([P, NH], F32)
        nc.sync.dma_start(qb_t[:], qb.rearrange("(m p) -> p m", p=P))
        kb_t = cn.tile([P, NH], F32)
        nc.sync.dma_start(kb_t[:], kb.rearrange("(m p) -> p m", p=P))
        b1_t = cn.tile([P, NF], F32)
        nc.sync.dma_start(b1_t[:], b1.rearrange("(m p) -> p m", p=P))
        vb_b = cn.tile([P, H], F32)
        nc.sync.dma_start(vb_b[:], vb[None, :].partition_broadcast(P))
        b2_b = cn.tile([P, C], F32)
        nc.sync.dma_start(b2_b[:], b2[None, :].partition_broadcast(P))
        eps_t = cn.tile([P, 1], F32)
        nc.vector.memset(eps_t, EPS)

        _ctr = [0]

        def psum(tag, shape=(P, 512), dt=F32, bufs=2):
            _ctr[0] += 1
            return ps.tile(list(shape), dt, tag=tag, bufs=bufs, name=f"ps{_ctr[0]}")

        qTo = res.tile([P, NH, TOWN], E4)   # q^T own tokens (local order)
        kT = res.tile([P, NH, T], E4)       # k^T all keys
        v_sb = res.tile([P, NT, H], E4)     # v token-major, all keys
        h2T = res.tile([P, NCT, TOWN], E4)  # LN2 output transposed
        # precise (bf16) head-block path: first own block attends few keys, so
        # fp8 noise passes straight through - keep that slice in bf16.
        qTb = res.tile([P, NH, P], BF16)    # q^T for local block 0
        kTb = res.tile([P, NH, 512], BF16)  # k^T for key chunk 0
        v_bf = res.tile([P, 2, H], BF16)    # v for key blocks 0-1 (bf16 matmul)

        # ===== Phase B: LN1 over permuted blocks -> hT/hto; v; q^T; k^T ====
        # x arrives block-PERMUTED per core (own blocks at even positions),
        # so one LN pass feeds hT (all keys) and hto (own tokens, gathered by
        # the scalar engine) - no duplicated LN/transposes, and the v matmuls
        # give the PE work from the first tile on.
        with ExitStack() as sB:
            big_b = sB.enter_context(tc.tile_pool(name="bigb", bufs=1))
            wvp = sB.enter_context(tc.tile_pool(name="wvp", bufs=1))
            hT = big_b.tile([P, NCT, T], E4)
            hto = big_b.tile([P, NCT, TOWN], E4)
            hT_bf = big_b.tile([P, NCT, 256], BF16)
            # x tiles must BEAT the 5MB of weight loads to the DMA queues -
            # only wv is needed early; the rest are staggered into the loop.
            wv_t = wvp.tile([P, NCT, H], E4)
            nc.sync.dma_start(wv_t[:], wv_r[:])
            wq_t = wvp.tile([P, NCT, H], E4)
            wk_t = wvp.tile([P, NCT, H], E4)
            wv16_t = wvp.tile([P, NCT, H], BF16)
            # LN per permuted token tile; v row-block right after its tile
            for st in range(NT):
                if st < 4:
                    x_t = xpre[st]
                else:
                    x_t = ln.tile([P, C], BF16, tag="xb", bufs=6)
                    nc.sync.dma_start(x_t[:], x_bf[st * P : (st + 1) * P, :])
                if st == 1:
                    nc.sync.dma_start(wv16_t[:], wv16_r[:])
                elif st == 5:
                    nc.sync.dma_start(wq_t[:], wq_r[:])
                elif st == 9:
                    nc.sync.dma_start(wk_t[:], wk_r[:])
                h_t = ln.tile([P, C], BF16, tag="ht")
                _layernorm(nc, ln, x_t, h_t, eps_t)
                for c in range(NCT):
                    tp = psum("b", (P, P), BF16)
                    nc.tensor.transpose(tp[:], h_t[:, c * P : (c + 1) * P], id_t[:])
                    if st < 2:
                        # first two tiles: write bf16 only; the fp8 hT slice
                        # is backfilled in bulk below, off the critical path
                        nc.vector.tensor_copy(
                            hT_bf[:, c, st * P : (st + 1) * P], tp[:]
                        )
                    elif st % 2 == 1 and c >= 6:
                        # balance the PSUM evacuations across DVE and ACT
                        nc.scalar.activation(
                            out=hT[:, c, st * P : (st + 1) * P],
                            in_=tp[:],
                            func=mybir.ActivationFunctionType.Copy,
                            scale=1.0,
                        )
                    else:
                        nc.vector.tensor_copy(
                            hT[:, c, st * P : (st + 1) * P], tp[:]
                        )
                    if st % 2 == 0:
                        nc.scalar.activation(
                            out=hto[:, c, (st // 2) * P : (st // 2 + 1) * P],
                            in_=tp[:],
                            func=mybir.ActivationFunctionType.Copy,
                            scale=1.0,
                        )
                def emit_v(vst):
                    accs = [psum("a") for _ in range(2)]
                    for k in range(NCT // 2):
                        for hh in range(2):
                            nc.tensor.matmul(
                                accs[hh][:],
                                hT[:, 2 * k : 2 * k + 2, vst * P : (vst + 1) * P],
                                wv_t[:, 2 * k : 2 * k + 2,
                                     hh * 512 : (hh + 1) * 512],
                                start=(k == 0),
                                stop=(k == NCT // 2 - 1),
                                perf_mode=DR,
                            )
                    # vb is folded into the host-side residual x (softmax
                    # weights sum to 1), so the evacuation is a cast - on ACT.
                    for hh in range(2):
                        nc.scalar.activation(
                            out=v_sb[:, vst, hh * 512 : (hh + 1) * 512],
                            in_=accs[hh][:],
                            func=mybir.ActivationFunctionType.Copy,
                            scale=1.0,
                        )

                if st == 2:
                    # deferred bulk cast: hT tokens 0-255 from hT_bf, then
                    # the postponed v matmuls for those two blocks
                    nc.vector.tensor_copy(hT[:, :, 0:256], hT_bf[:])
                    emit_v(0)
                    emit_v(1)
                if st >= 2:
                    emit_v(st)
                if st == 3:
                    # precise bf16 v for key blocks 0-1 - emitted here so the
                    # PE has work while the LN stream (DVE-bound) warms up
                    for blk in range(2):
                        accs = [psum("a") for _ in range(2)]
                        for k in range(NCT):
                            for hh in range(2):
                                nc.tensor.matmul(
                                    accs[hh][:],
                                    hT_bf[:, k, blk * P : (blk + 1) * P],
                                    wv16_t[:, k, hh * 512 : (hh + 1) * 512],
                                    start=(k == 0),
                                    stop=(k == NCT - 1),
                                )
                        for hh in range(2):
                            nc.scalar.activation(
                                out=v_bf[:, blk, hh * 512 : (hh + 1) * 512],
                                in_=accs[hh][:],
                                func=mybir.ActivationFunctionType.Copy,
                                scale=1.0,
                            )
            # q^T (own tokens, from hto)
            for m in range(NH):
                accs = [psum("a") for _ in range(2)]
                for k in range(NCT // 2):
                    for g in range(2):
                        nc.tensor.matmul(
                            accs[g][:],
                            wq_t[:, 2 * k : 2 * k + 2, m * P : (m + 1) * P],
                            hto[:, 2 * k : 2 * k + 2, g * 512 : (g + 1) * 512],
                            start=(k == 0),
                            stop=(k == NCT // 2 - 1),
                            perf_mode=DR,
                        )
                for g in range(2):
                    nc.vector.tensor_scalar_add(
                        out=qTo[:, m, g * 512 : (g + 1) * 512],
                        in0=accs[g][:],
                        scalar1=qb_t[:, m : m + 1],
                    )
                nc.vector.tensor_scalar_add(
                    out=qTb[:, m, :],
                    in0=accs[0][:, 0:P],
                    scalar1=qb_t[:, m : m + 1],
                )
            # k^T (all keys)
            for m in range(NH):
                accs = [psum("c", bufs=4) for _ in range(4)]
                for k in range(NCT // 2):
                    for ch in range(4):
                        nc.tensor.matmul(
                            accs[ch][:],
                            wk_t[:, 2 * k : 2 * k + 2, m * P : (m + 1) * P],
                            hT[:, 2 * k : 2 * k + 2, ch * 512 : (ch + 1) * 512],
                            start=(k == 0),
                            stop=(k == NCT // 2 - 1),
                            perf_mode=DR,
                        )
                for ch in range(4):
                    nc.vector.tensor_scalar_add(
                        out=kT[:, m, ch * 512 : (ch + 1) * 512],
                        in0=accs[ch][:],
                        scalar1=kb_t[:, m : m + 1],
                    )
                nc.vector.tensor_scalar_add(
                    out=kTb[:, m, :],
                    in0=accs[0][:],
                    scalar1=kb_t[:, m : m + 1],
                )

        # ============== Phase C: attention (software-pipelined) ============
        with ExitStack() as sC:
            att = sC.enter_context(tc.tile_pool(name="att", bufs=2))
            wtl = sC.enter_context(tc.tile_pool(name="wtl", bufs=16))
            state = {}

            def emit_scores(lp):
                nch = NCHUNKS[lp]
                mask_t = att.tile([P, 512], F32, tag="mask")
                nc.sync.dma_start(mask_t[:], masks[lp])
                p_t = att.tile([P, T], BF16, tag="pt", bufs=3)
                den = att.tile([P, 4], F32, tag="den")
                scs = [psum("c", bufs=4) for _ in range(nch)]
                if lp == 0:
                    # precise bf16 scores for the head block
                    for m in range(NH):
                        nc.tensor.matmul(
                            scs[0][:],
                            qTb[:, m, :],
                            kTb[:, m, :],
                            start=(m == 0),
                            stop=(m == NH - 1),
                        )
                else:
                    # chunk-outer: each chunk's exp can fire right after its
                    # 4 matmuls, freeing the PSUM bank earlier
                    for j in range(nch):
                        for m in range(NH // 2):
                            nc.tensor.matmul(
                                scs[j][:],
                                qTo[:, 2 * m : 2 * m + 2, lp * P : (lp + 1) * P],
                                kT[:, 2 * m : 2 * m + 2, j * 512 : (j + 1) * 512],
                                start=(m == 0),
                                stop=(m == NH // 2 - 1),
                                perf_mode=DR,
                            )
                for j in range(nch):
                    if j == nch - 1:
                        nc.vector.tensor_add(
                            out=scs[j][:], in0=scs[j][:], in1=mask_t[:]
                        )
                    nc.scalar.activation(
                        out=p_t[:, j * 512 : (j + 1) * 512],
                        in_=scs[j][:],
                        func=mybir.ActivationFunctionType.Exp,
                        scale=float(SCALE),
                        accum_out=den[:, j : j + 1],
                    )
                state[lp] = (p_t, den)

            def emit_tail(lp):
                nch = NCHUNKS[lp]
                nst = 4 * nch
                p_t, den = state.pop(lp)
                dsum = att.tile([P, 1], F32, tag="dsum")
                nc.vector.reduce_sum(
                    out=dsum[:], in_=den[:, :nch], axis=mybir.AxisListType.X
                )
                nc.vector.reciprocal(out=dsum[:], in_=dsum[:])
                sa0 = psum("a")
                sa1 = psum("a")
                if lp == 0:
                    # precise bf16 p@v over key blocks 0-1 (rest masked to zero)
                    wtb = []
                    for st in range(2):
                        tp = psum("b", (P, P), BF16)
                        nc.tensor.transpose(
                            tp[:], p_t[:, st * P : (st + 1) * P], id_t[:]
                        )
                        wt = wtl.tile([P, P], BF16, tag="wtb")
                        nc.vector.tensor_copy(wt[:], tp[:])
                        wtb.append(wt)
                    for st in range(2):
                        nc.tensor.matmul(
                            sa0[:], wtb[st][:], v_bf[:, st, 0:512],
                            start=(st == 0), stop=(st == 1),
                        )
                        nc.tensor.matmul(
                            sa1[:], wtb[st][:], v_bf[:, st, 512:1024],
                            start=(st == 0), stop=(st == 1),
                        )
                else:
                    wtp = []
                    for sp in range(nst // 2):
                        wt = wtl.tile([P, 2, P], E4, tag="wt")
                        for u in range(2):
                            st = 2 * sp + u
                            tp = psum("b", (P, P), BF16)
                            nc.tensor.transpose(
                                tp[:], p_t[:, st * P : (st + 1) * P], id_t[:]
                            )
                            nc.vector.tensor_copy(wt[:, u], tp[:])
                        wtp.append(wt)
                    for sp in range(nst // 2):
                        nc.tensor.matmul(
                            sa0[:], wtp[sp][:], v_sb[:, 2 * sp : 2 * sp + 2, 0:512],
                            start=(sp == 0), stop=(sp == nst // 2 - 1),
                            perf_mode=DR,
                        )
                        nc.tensor.matmul(
                            sa1[:], wtp[sp][:],
                            v_sb[:, 2 * sp : 2 * sp + 2, 512:1024],
                            start=(sp == 0), stop=(sp == nst // 2 - 1),
                            perf_mode=DR,
                        )
                x_t = att.tile([P, C], F32, tag="xo")
                nc.sync.dma_start(x_t[:], x[2 * lp * P : (2 * lp + 1) * P, :])
                x2_t = att.tile([P, C], F32, tag="x2")
                # sa/den scaling on the scalar engine - DVE gates the PE here
                nc.scalar.activation(
                    out=x2_t[:, 0:512], in_=sa0[:],
                    func=mybir.ActivationFunctionType.Copy, scale=dsum[:],
                )
                nc.scalar.activation(
                    out=x2_t[:, 512:1024], in_=sa1[:],
                    func=mybir.ActivationFunctionType.Copy, scale=dsum[:],
                )
                nc.vector.tensor_add(out=x2_t[:], in0=x2_t[:], in1=x_t[:])
                nc.sync.dma_start(x2_d[lp * P : (lp + 1) * P, :], x2_t[:])

            emit_scores(0)
            emit_scores(1)
            for lp in range(2, NLOC):
                emit_scores(lp)
                emit_tail(lp - 2)
            emit_tail(NLOC - 2)
            emit_tail(NLOC - 1)

        # ============== Phase C2: LN2 + h2^T ===============================
        def emit_ln2(lt):
            x2_t = ln.tile([P, C], F32, tag="xt")
            nc.sync.dma_start(x2_t[:], x2_d[lt * P : (lt + 1) * P, :])
            h2_t = ln.tile([P, C], BF16, tag="ht")
            _layernorm(nc, ln, x2_t, h2_t, eps_t)
            for c in range(NCT):
                tp = psum("b", (P, P), BF16)
                nc.tensor.transpose(tp[:], h2_t[:, c * P : (c + 1) * P], id_t[:])
                nc.scalar.activation(
                    out=h2T[:, c, lt * P : (lt + 1) * P],
                    in_=tp[:],
                    func=mybir.ActivationFunctionType.Copy,
                    scale=1.0,
                )

        for lt in range(4):
            emit_ln2(lt)

        # ================= Phase D: FFN (fp8 DoubleRow) ====================
        with ExitStack() as sD:
            big_d = sD.enter_context(tc.tile_pool(name="bigd", bufs=1))
            ffw = sD.enter_context(tc.tile_pool(name="ffw", bufs=3))
            aT = [
                big_d.tile([P, NF, 512], E4, name=f"aT{i}") for i in range(2)
            ]

            def emit_aT(tch):
                # a^T half = relu(W1^T h2^T + b1) for 512 local tokens
                for ft in range(NF):
                    w1_t = ffw.tile(
                        [P, NCT, P], E4, tag="w1t", bufs=6, name=f"w1t{tch}_{ft}"
                    )
                    nc.sync.dma_start(w1_t[:], w1[ft])
                    acc = psum("a")
                    for k in range(NCT // 2):
                        nc.tensor.matmul(
                            acc[:],
                            w1_t[:, 2 * k : 2 * k + 2],
                            h2T[:, 2 * k : 2 * k + 2, tch * 512 : (tch + 1) * 512],
                            start=(k == 0),
                            stop=(k == NCT // 2 - 1),
                            perf_mode=DR,
                        )
                    nc.scalar.activation(
                        out=aT[tch][:, ft, :],
                        in_=acc[:],
                        func=mybir.ActivationFunctionType.Relu,
                        bias=b1_t[:, ft : ft + 1],
                        scale=1.0,
                    )

            def mk_grp():
                return [
                    [
                        psum(
                            ("a" if tb < 1 else "b" if tb < 2 else "c"),
                            bufs=(2 if tb < 2 else 4),
                        )
                        for cc in range(2)
                    ]
                    for tb in range(4)
                ]

            def ff_pass(tbh, dig, grp):
                for ft in range(NF // 2):
                    w2_t = ffw.tile(
                        [P, 2, C], E4, tag="w2t", bufs=6,
                        name=f"w2t{tbh}_{dig}_{ft}",
                    )
                    nc.sync.dma_start(
                        w2_t[:], w2_r[:, dig, 2 * ft : 2 * ft + 2, :]
                    )
                    for tb in range(4):
                        for cc in range(2):
                            nc.tensor.matmul(
                                grp[tb][cc][:],
                                aT[tbh][:, 2 * ft : 2 * ft + 2,
                                        tb * P : (tb + 1) * P],
                                w2_t[:, :, cc * 512 : (cc + 1) * 512],
                                start=(ft == 0),
                                stop=(ft == NF // 2 - 1),
                                perf_mode=DR,
                            )

            def emit_ff_hi(tbh):
                # hi-digit pass, evacuated (+b2) into o_t
                grp = mk_grp()
                ff_pass(tbh, 0, grp)
                o_ts = []
                for tb in range(4):
                    o_t = ffw.tile(
                        [P, C], F32, tag="ot", bufs=5, name=f"ot{tbh}_{tb}"
                    )
                    for cc in range(2):
                        nc.vector.tensor_add(
                            out=o_t[:, cc * 512 : (cc + 1) * 512],
                            in0=grp[tb][cc][:],
                            in1=b2_b[:, cc * 512 : (cc + 1) * 512],
                        )
                    o_ts.append(o_t)
                return o_ts

            def emit_ff_lo(tbh, o_ts):
                # lo-digit pass (weights pre-scaled x256), combined at 1/256
                grp = mk_grp()
                ff_pass(tbh, 1, grp)
                for tb in range(4):
                    lt = tbh * 4 + tb
                    x2_t = ffw.tile([P, C], F32, tag="x2r", name=f"x2r{tbh}_{tb}")
                    nc.sync.dma_start(x2_t[:], x2_d[lt * P : (lt + 1) * P, :])
                    o_t = o_ts[tb]
                    lo_t = ffw.tile([P, C], F32, tag="lot", name=f"lot{tbh}_{tb}")
                    for cc in range(2):
                        nc.scalar.activation(
                            out=lo_t[:, cc * 512 : (cc + 1) * 512],
                            in_=grp[tb][cc][:],
                            func=mybir.ActivationFunctionType.Copy,
                            scale=float(1.0 / 256.0),
                        )
                    nc.vector.tensor_add(out=o_t[:], in0=o_t[:], in1=lo_t[:])
                    nc.vector.tensor_add(out=o_t[:], in0=o_t[:], in1=x2_t[:])
                    nc.sync.dma_start(out[lt * P : (lt + 1) * P, :], o_t[:])

            # aT(0) needs only LN2 of tiles 0-3; LN2 of 4-7 overlaps its
            # matmuls. aT(1) overlaps the hi(0) PSUM evacuation; lo(0) still
            # reads aT[0], so the two halves use separate aT buffers.
            emit_aT(0)
            for lt in range(4, NLOC):
                emit_ln2(lt)
            o0 = emit_ff_hi(0)
            emit_aT(1)
            emit_ff_lo(0, o0)
            o1 = emit_ff_hi(1)
            emit_ff_lo(1, o1)

    _split_multi_waits(nc)
    return nc


_NC_CACHE = None


def _get_nc():
    global _NC_CACHE
    if _NC_CACHE is None:
        _NC_CACHE = build_nc()
    return _NC_CACHE


def _prep_host(inputs):
    """Fold LN gains/biases into weights; build per-core input maps."""
    x = np.asarray(inputs["x"], dtype=np.float32)
    Wk = np.asarray(inputs["Wk"], dtype=np.float32)
    Wq = np.asarray(inputs["Wq"], dtype=np.float32)
    Wv = np.asarray(inputs["Wv"], dtype=np.float32)
    W1 = np.asarray(inputs["W1"], dtype=np.float32)
    b1 = np.asarray(inputs["b1"], dtype=np.float32)
    W2 = np.asarray(inputs["W2"], dtype=np.float32)
    b2 = np.asarray(inputs["b2"], dtype=np.float32)
    g1 = np.asarray(inputs["g1"], dtype=np.float32)
    be1 = np.asarray(inputs["be1"], dtype=np.float32)
    g2 = np.asarray(inputs["g2"], dtype=np.float32)
    be2 = np.asarray(inputs["be2"], dtype=np.float32)

    f8 = ml_dtypes.float8_e4m3
    bf = ml_dtypes.bfloat16
    wq_f = np.ascontiguousarray((g1[:, None] * Wq).astype(f8))
    wk_f = np.ascontiguousarray((g1[:, None] * Wk).astype(f8))
    wv_full = g1[:, None] * Wv
    wv_f = np.ascontiguousarray(wv_full.astype(f8))
    wv16_f = np.ascontiguousarray(wv_full.astype(bf))
    qb = be1 @ Wq
    kb = be1 @ Wk
    vb = be1 @ Wv
    w1_full = (g2[:, None] * W1).astype(f8)
    w1_f = np.ascontiguousarray(
        w1_full.reshape(NCT, P, NF, P).transpose(2, 1, 0, 3)
    )
    w2_hi = W2.astype(f8)
    w2_lo = (256.0 * (W2 - w2_hi.astype(np.float32))).astype(f8)
    w2_f8 = np.ascontiguousarray(np.stack([w2_hi, w2_lo], axis=0))
    b1_f = b1 + be2 @ W1

    ident = np.eye(P, dtype=ml_dtypes.bfloat16)

    # per-half masks against the PERMUTED key order: for own block lp (global
    # g), only the last processed 512-chunk needs masking; each 128-block in
    # it is fully allowed (gb < g), fully masked (gb > g), or diagonal.
    rr = np.arange(P)[:, None]
    sub_diag = np.where(np.arange(P)[None, :] <= rr, 0.0, NEG).astype(np.float32)
    masks_h = []
    for half in range(2):
        perm = PERMS[half]
        mk = np.empty((NLOC, P, 512), dtype=np.float32)
        for m_i, g in enumerate(L_HALF[half]):
            nch = NCHUNKS[m_i]
            for pos in range(4 * (nch - 1)):
                assert perm[pos] < g, (half, m_i, pos)
            for pos in range(4 * nch, NT):
                assert perm[pos] > g, (half, m_i, pos)
            base = 4 * (nch - 1)
            for jb in range(4):
                gb = perm[base + jb]
                if gb < g:
                    mk[m_i, :, jb * P : (jb + 1) * P] = 0.0
                elif gb > g:
                    mk[m_i, :, jb * P : (jb + 1) * P] = NEG
                else:
                    mk[m_i, :, jb * P : (jb + 1) * P] = sub_diag
        masks_h.append(mk)

    shared = {
        "wq": wq_f, "wk": wk_f, "wv": wv_f, "wv16": wv16_f,
        "w1": w1_f, "w2": w2_f8,
        "qb": qb, "kb": kb, "vb": vb, "b1": b1_f, "b2": b2,
        "ident": ident,
    }
    in_maps = []
    for core in range(8):
        b, half = core // 2, core % 2
        rows = np.concatenate(
            [np.arange(i * P, (i + 1) * P) for i in PERMS[half]]
        )
        xp = np.ascontiguousarray(x[b][rows])
        m = dict(shared)
        # vb folded into the residual (softmax weights sum to 1)
        m["x"] = np.ascontiguousarray(xp + vb[None, :].astype(np.float32))
        m["x_bf"] = xp.astype(bf)
        m["masks"] = masks_h[half]
        in_maps.append(m)
    return in_maps


def _scatter_out(results):
    out = np.empty((B, T, C), dtype=np.float32)
    for core in range(8):
        b, half = core // 2, core % 2
        L = L_HALF[half]
        o = results[core]["out"]
        for ppos, i in enumerate(L):
            out[b, i * P : (i + 1) * P, :] = o[ppos * P : (ppos + 1) * P, :]
    return out


def run(inputs, trace=False, **kw):
    nc = _get_nc()
    in_maps = _prep_host(inputs)
    res = run_bass_kernel_spmd(
        nc, in_maps, core_ids=list(range(8)), trace=trace, **kw
    )
    return _scatter_out(res.results), res


def kernel(**inputs) -> np.ndarray:
    out, _ = run(inputs, trace=False)
    return out
